# revision 1
# baseline (speedup 1.0000x reference)
"""Trainium2 Bass kernel for the Gudi UpProj block.

Reference computation (per image, NCHW):
    xu  = zero_stuff_2x(x)                    # [B,1024,32,32], nonzero only at even (h,w)
    c1  = conv5x5(xu, w1, pad=2);  out1 = relu(BN(c1))
    c2  = conv3x3(out1, w2, pad=1)
    csc = conv5x5(xu, wsc, pad=2)
    out = relu(BN(c2) + BN(csc))              # BN: training-mode batch stats over (N,H,W)

Strategy:
  * Data-parallel over batch: 16 images -> 2 per NeuronCore (8 cores).
  * Zero-stuffing exploited: a 5x5 conv on the zero-stuffed 32x32 grid decomposes
    into 4 parity phases, each a small conv (3x3 / 3x2 / 2x3 / 2x2) on the original
    16x16 grid -> 4x FLOP reduction.
  * All convs as tap-decomposed matmuls on the PE array in float32r
    (TF32-like: full PE rate at N>=256, ~1e-4 rel err). Weights / x are
    pre-rounded host-side (RNE to 11 mantissa bits), regrouped into the exact
    consumption order, and DMA'd directly in multi-tap batches.
  * BN batch stats (sum, sumsq per channel) need cross-core reduction:
    two small AllReduces (stats of c1; stats of c2+csc together).
"""

import numpy as np

NCORES = 8
B = 16
B_LOC = B // NCORES          # 2 images per core
CIN, COUT = 1024, 512
NCI, NCO = CIN // 128, COUT // 128   # 8, 4 partition tiles
H = 16                        # input spatial
OH = 32                       # output spatial
EPS = 1e-5
CNT = float(B * OH * OH)      # BN element count per channel = 16384
PHASES = [(0, 0), (0, 1), (1, 0), (1, 1)]

_CACHE = {}


def _round_f32r(a: np.ndarray) -> np.ndarray:
    """Round fp32 to float32r (RNE to 11 mantissa bits) - matches TRN2 HW rounding."""
    bits = np.ascontiguousarray(a, dtype=np.float32).view(np.uint32)
    shift = 12
    lsb = (bits >> shift) & np.uint32(1)
    out = ((bits + np.uint32((1 << (shift - 1)) - 1) + lsb) >> shift) << shift
    return out.view(np.float32)


def _taps(p):
    """Taps of a parity phase along one dim: list of (input shift, 5-tap kernel idx)."""
    if p == 0:
        return [(-1, 0), (0, 2), (1, 4)]
    return [(0, 1), (1, 3)]


def _w5_groups():
    """Weight-block groups for the phase-decomposed 5x5 conv, in consumption
    order: one group per (phase, cin-tile, kernel-row) holding len(kws) blocks."""
    groups = []
    for (p, q) in PHASES:
        for ci in range(NCI):
            for (ah, kh) in _taps(p):
                groups.append((p, q, ci, ah, kh, _taps(q)))
    return groups


def _phase_view(ap2048, p, q):
    """[128, 2048] tile viewed as [128, b, i, j] at output positions (2i+p, 2j+q)."""
    v = ap2048.rearrange("c (b i p2 j q2) -> c b i p2 j q2", b=2, i=16, p2=2, j=16, q2=2)
    return v[:, :, :, p, :, q]


def _build_nc():
    import concourse.mybir as mybir
    import concourse.tile as tile
    from concourse import bacc

    f32 = mybir.dt.float32
    f32r = mybir.dt.float32r
    ALU = mybir.AluOpType
    AFT = mybir.ActivationFunctionType

    nc = bacc.Bacc("TRN2", target_bir_lowering=False, debug=False)

    # ---- kernel I/O ----
    xpad_d = nc.dram_tensor("xpad", [B_LOC, CIN, 18, 18], f32r, kind="ExternalInput").ap()
    w1g_d = nc.dram_tensor("w1g", [200, 128, COUT], f32r, kind="ExternalInput").ap()
    wscg_d = nc.dram_tensor("wscg", [200, 128, COUT], f32r, kind="ExternalInput").ap()
    w2t_d = nc.dram_tensor("w2t", [9, COUT, COUT], f32r, kind="ExternalInput").ap()
    gb_d = nc.dram_tensor("gb", [128, 6, 4], f32, kind="ExternalInput").ap()
    zpad_d = nc.dram_tensor("zpad", [2, 34, 34], f32r, kind="ExternalInput").ap()
    out_d = nc.dram_tensor("out", [B_LOC, COUT, OH, OH], f32, kind="ExternalOutput").ap()

    with tile.TileContext(nc) as tc:
        # collective buffers (internal DRAM)
        _frees = []
        ar1_in, _f = tc.tile([2, COUT], f32, space="DRAM", name="ar1_in"); _frees.append(_f)
        ar1_out, _f = tc.tile([2, COUT], f32, space="DRAM", addr_space="Shared", name="ar1_out"); _frees.append(_f)
        ar2_in, _f = tc.tile([4, COUT], f32, space="DRAM", name="ar2_in"); _frees.append(_f)
        ar2_out, _f = tc.tile([4, COUT], f32, space="DRAM", addr_space="Shared", name="ar2_out"); _frees.append(_f)

        with tc.tile_pool(name="xp", bufs=1) as xp_pool, \
             tc.tile_pool(name="acts", bufs=1) as acts, \
             tc.tile_pool(name="op1", bufs=1) as op1_pool, \
             tc.tile_pool(name="wts", bufs=4) as wts, \
             tc.tile_pool(name="w2p", bufs=2) as w2p, \
             tc.tile_pool(name="scr", bufs=1) as scr_pool, \
             tc.tile_pool(name="small", bufs=1) as small, \
             tc.tile_pool(name="ps", bufs=8, space="PSUM") as ps:

            # ---- persistent SBUF tensors ----
            XP = [xp_pool.tile([128, 2, 18, 18], f32r, name=f"xp{i}", tag=f"xp{i}")
                  for i in range(NCI)]
            C1 = [acts.tile([128, 2048], f32, name=f"c1_{i}", tag=f"c1_{i}") for i in range(NCO)]
            CSC = [acts.tile([128, 2048], f32, name=f"csc_{i}", tag=f"csc_{i}") for i in range(NCO)]
            C2 = [acts.tile([128, 2048], f32, name=f"c2_{i}", tag=f"c2_{i}") for i in range(NCO)]
            OP1 = [op1_pool.tile([128, 2, 34, 34], f32r, name=f"op1_{i}", tag=f"op1_{i}")
                   for i in range(NCO)]

            # stat columns: sums/sumsqs per (tensor, co, phase-or-quarter)
            sums1 = small.tile([128, 16], f32, name="sums1")
            sq1 = small.tile([128, 16], f32, name="sq1")
            sums2 = small.tile([128, 16], f32, name="sums2")
            sq2 = small.tile([128, 16], f32, name="sq2")
            sumssc = small.tile([128, 16], f32, name="sumssc")
            sqsc = small.tile([128, 16], f32, name="sqsc")
            pack1 = small.tile([128, 2, 4], f32, name="pack1")
            pack2 = small.tile([128, 4, 4], f32, name="pack2")
            st1 = small.tile([128, 2, 4], f32, name="st1")
            st2 = small.tile([128, 2, 2, 4], f32, name="st2")   # [c, grp(c2,sc), (sum,sq), co]
            gbv = small.tile([128, 6, 4], f32, name="gbv")      # rows: g1,b1,g2,b2,gsc,bsc
            scale1 = small.tile([128, 4], f32, name="scale1")
            shift1 = small.tile([128, 4], f32, name="shift1")
            m2b = small.tile([128, 2, 4], f32, name="m2b")
            varb = small.tile([128, 2, 4], f32, name="varb")
            invb = small.tile([128, 2, 4], f32, name="invb")
            scaleb = small.tile([128, 2, 4], f32, name="scaleb")  # [.,0,:]=scale2 [.,1,:]=scalesc
            shiftb2 = small.tile([128, 2, 4], f32, name="shiftb2")
            shiftB = small.tile([128, 4], f32, name="shiftB")
            rmix = small.tile([128, 4], f32, name="rmix")
            tmpa = small.tile([128, 4], f32, name="tmpa")
            tmpb = small.tile([128, 4], f32, name="tmpb")
            epsc = small.tile([128, 1], f32, name="epsc")

            # ---- input DMAs (x first: the PE's first dependency) ----
            def emit_xp_dma(ci):
                nc.sync.dma_start(
                    XP[ci][:].rearrange("c b h w -> c b (h w)"),
                    xpad_d[:, ci * 128:(ci + 1) * 128].rearrange("b c h w -> c b (h w)"),
                )
            emit_xp_dma(0)
            emit_xp_dma(1)
            nc.vector.memset(epsc[:], EPS)

            # ---- helper: one 5x5-phase-decomposed conv (conv1 / convsc) ----
            def conv5(wg_d, dst, sums, sqs, wtag, prefetch_xp=False):
                gofs = 0
                for iph, (p, q) in enumerate(PHASES):
                    pps = [ps.tile([128, 512], f32, name=f"{wtag}ps{iph}_{co}", tag="psb")
                           for co in range(NCO)]
                    kws = _taps(q)
                    n_acc = NCI * len(_taps(p)) * len(kws)
                    k = 0
                    for ci in range(NCI):
                        if prefetch_xp and iph == 0 and ci + 2 < NCI:
                            emit_xp_dma(ci + 2)
                        for (ah, kh) in _taps(p):
                            L = len(kws)
                            wt = wts.tile([128, 3, 512], f32r, name=f"{wtag}w", tag="w5")
                            nc.sync.dma_start(
                                wt[:, :L, :],
                                wg_d[gofs:gofs + L].rearrange("l c m -> c l m"))
                            gofs += L
                            for kwi, (aw, kw) in enumerate(kws):
                                rhs = XP[ci][:, :, 1 + ah:17 + ah, 1 + aw:17 + aw]
                                for co in range(NCO):
                                    nc.tensor.matmul(
                                        pps[co][:], wt[:, kwi, co * 128:(co + 1) * 128], rhs,
                                        start=(k == 0), stop=(k == n_acc - 1))
                                k += 1
                    for co in range(NCO):
                        icol = co * 4 + iph
                        nc.vector.tensor_scalar(
                            dst[co][:, iph * 512:(iph + 1) * 512], pps[co][:],
                            0.0, 0.0, op0=ALU.add, op1=ALU.add,
                            accum_out=sums[:, icol:icol + 1])
                        scr = scr_pool.tile([128, 512], f32, name=f"{wtag}scr", tag="scr")
                        nc.scalar.activation(
                            scr[:], pps[co][:], AFT.Square,
                            accum_out=sqs[:, icol:icol + 1])

            # ================= conv1 =================
            conv5(w1g_d, C1, sums1, sq1, "c1", prefetch_xp=True)

            # aux DMAs (needed from BN1-apply onward; emitted late to keep the
            # startup DMA path clear)
            nc.sync.dma_start(gbv[:], gb_d)
            for co in range(NCO):
                nc.sync.dma_start(OP1[co][:], zpad_d.unsqueeze(0).partition_broadcast(128))

            # ---- c1 stats -> AllReduce #1 (overlaps with convsc compute) ----
            nc.vector.tensor_reduce(
                pack1[:, 0, :], sums1[:].rearrange("c (co ph) -> c co ph", ph=4),
                axis=mybir.AxisListType.X, op=ALU.add)
            nc.vector.tensor_reduce(
                pack1[:, 1, :], sq1[:].rearrange("c (co ph) -> c co ph", ph=4),
                axis=mybir.AxisListType.X, op=ALU.add)
            nc.sync.dma_start(ar1_in[:].rearrange("s (co c) -> c s co", c=128), pack1[:])
            nc.gpsimd.collective_compute(
                "AllReduce", ALU.add,
                replica_groups=[list(range(NCORES))],
                ins=[ar1_in.opt()], outs=[ar1_out.opt()])
            nc.sync.dma_start(st1[:], ar1_out[:].rearrange("s (co c) -> c s co", c=128))

            # ================= convsc (independent of BN1) =================
            conv5(wscg_d, CSC, sumssc, sqsc, "sc")

            # ---- BN1 scale/shift from global stats ----
            nc.vector.tensor_scalar_mul(st1[:], st1[:], 1.0 / CNT)
            m1 = st1[:, 0, :]
            nc.vector.tensor_tensor(tmpa[:], m1, m1, op=ALU.mult)
            nc.vector.tensor_tensor(tmpb[:], st1[:, 1, :], tmpa[:], op=ALU.subtract)
            nc.scalar.activation(tmpb[:], tmpb[:], AFT.Sqrt, bias=epsc[:])
            nc.vector.reciprocal(tmpa[:], tmpb[:])
            nc.vector.tensor_tensor(scale1[:], gbv[:, 0, :], tmpa[:], op=ALU.mult)
            nc.vector.tensor_tensor(tmpa[:], m1, scale1[:], op=ALU.mult)
            nc.vector.tensor_tensor(shift1[:], gbv[:, 1, :], tmpa[:], op=ALU.subtract)

            # ---- BN1 apply + ReLU -> padded conv2 input (interleave phases) ----
            for co in range(NCO):
                for iph, (p, q) in enumerate(PHASES):
                    dst = OP1[co][:, :, 1:33, 1:33] \
                        .rearrange("c b (i p2) (j q2) -> c b i p2 j q2", p2=2, q2=2)[:, :, :, p, :, q]
                    src = C1[co][:, iph * 512:(iph + 1) * 512] \
                        .rearrange("c (b h w) -> c b h w", b=2, h=16)
                    nc.scalar.activation(dst, src, AFT.Relu,
                                         bias=shift1[:, co:co + 1], scale=scale1[:, co:co + 1])

            # ================= conv2 (3x3, pad 1, on OP1) =================
            for co in range(NCO):
                pps = [ps.tile([128, 512], f32, name=f"c2ps{co}_{qq}", tag="psb")
                       for qq in range(4)]
                n_acc = NCO * 9
                k = 0
                for ci in range(NCO):
                    wt = w2p.tile([128, 9, 128], f32r, name="c2w", tag="w2")
                    nc.sync.dma_start(
                        wt[:],
                        w2t_d[:, ci * 128:(ci + 1) * 128, co * 128:(co + 1) * 128]
                        .rearrange("t c m -> c t m"))
                    for dh in (-1, 0, 1):
                        for dw in (-1, 0, 1):
                            t = (dh + 1) * 3 + (dw + 1)
                            for qq in range(4):
                                rhs = OP1[ci][:, :, 1 + 8 * qq + dh:9 + 8 * qq + dh, 1 + dw:33 + dw]
                                nc.tensor.matmul(pps[qq][:], wt[:, t, :], rhs,
                                                 start=(k == 0), stop=(k == n_acc - 1))
                            k += 1
                for qq in range(4):
                    icol = co * 4 + qq
                    dst = C2[co][:].rearrange("c (b h w) -> c b h w", b=2, h=32)[:, :, 8 * qq:8 * qq + 8, :]
                    nc.vector.tensor_scalar(
                        dst, pps[qq][:].rearrange("c (b h w) -> c b h w", b=2, h=8),
                        0.0, 0.0, op0=ALU.add, op1=ALU.add,
                        accum_out=sums2[:, icol:icol + 1])
                    scr = scr_pool.tile([128, 512], f32, name="c2scr", tag="scr")
                    nc.scalar.activation(
                        scr[:], pps[qq][:], AFT.Square,
                        accum_out=sq2[:, icol:icol + 1])

            # ---- stats of c2 + csc -> AllReduce #2 ----
            for row, src in ((0, sums2), (1, sq2), (2, sumssc), (3, sqsc)):
                nc.vector.tensor_reduce(
                    pack2[:, row, :], src[:].rearrange("c (co x) -> c co x", x=4),
                    axis=mybir.AxisListType.X, op=ALU.add)
            nc.sync.dma_start(ar2_in[:].rearrange("s (co c) -> c s co", c=128), pack2[:])
            nc.gpsimd.collective_compute(
                "AllReduce", ALU.add,
                replica_groups=[list(range(NCORES))],
                ins=[ar2_in.opt()], outs=[ar2_out.opt()])
            nc.sync.dma_start(
                st2[:].rearrange("c g s co -> c (g s) co"),
                ar2_out[:].rearrange("s (co c) -> c s co", c=128))

            # ---- BN2 / BNsc scale+shift (vectorized over both tensors) ----
            # final = relu(s2*c2 + t2 + ssc*csc + tsc)
            #       = relu( s2 * (c2 + (ssc/s2)*csc) + (t2 + tsc) )
            nc.vector.tensor_scalar_mul(st2[:], st2[:], 1.0 / CNT)
            means = st2[:, :, 0, :]    # [c, 2, 4]
            e2s = st2[:, :, 1, :]
            gpair = gbv[:, 2:, :].rearrange("c (g s) co -> c g s co", s=2)  # rows g2,b2,gsc,bsc
            nc.vector.tensor_tensor(m2b[:], means, means, op=ALU.mult)
            nc.vector.tensor_tensor(varb[:], e2s, m2b[:], op=ALU.subtract)
            nc.scalar.activation(varb[:], varb[:], AFT.Sqrt, bias=epsc[:])
            nc.vector.reciprocal(invb[:], varb[:])
            nc.vector.tensor_tensor(scaleb[:], gpair[:, :, 0, :], invb[:], op=ALU.mult)
            nc.vector.tensor_tensor(m2b[:], means, scaleb[:], op=ALU.mult)
            nc.vector.tensor_tensor(shiftb2[:], gpair[:, :, 1, :], m2b[:], op=ALU.subtract)
            nc.vector.tensor_tensor(shiftB[:], shiftb2[:, 0, :], shiftb2[:, 1, :], op=ALU.add)
            nc.vector.reciprocal(tmpa[:], scaleb[:, 0, :])
            nc.vector.tensor_tensor(rmix[:], scaleb[:, 1, :], tmpa[:], op=ALU.mult)

            # ---- final fuse: c2 += rmix*csc ; out = relu(scale2*c2 + shiftB) ----
            # (final output staged through the dead C1 tiles)
            for co in range(NCO):
                for iph, (p, q) in enumerate(PHASES):
                    nc.vector.scalar_tensor_tensor(
                        _phase_view(C2[co][:], p, q),
                        CSC[co][:, iph * 512:(iph + 1) * 512]
                        .rearrange("c (b h w) -> c b h w", b=2, h=16),
                        rmix[:, co:co + 1],
                        _phase_view(C2[co][:], p, q),
                        op0=ALU.mult, op1=ALU.add)
                fin = C1[co]
                nc.scalar.activation(fin[:], C2[co][:], AFT.Relu,
                                     bias=shiftB[:, co:co + 1], scale=scaleb[:, 0, co:co + 1])
                for b in range(B_LOC):
                    nc.sync.dma_start(
                        out_d[b, co * 128:(co + 1) * 128].rearrange("c h w -> c (h w)"),
                        fin[:, b * 1024:(b + 1) * 1024])

            for _f in _frees:
                _f()

    nc.compile()
    return nc


def _get_nc():
    if "nc" not in _CACHE:
        _CACHE["nc"] = _build_nc()
    return _CACHE["nc"]


def _regroup_w5(wt_full: np.ndarray) -> np.ndarray:
    """[5,5,CIN,COUT] -> [200,128,COUT] blocks in kernel consumption order."""
    blocks = np.empty((200, 128, COUT), dtype=np.float32)
    g = 0
    for (p, q, ci, ah, kh, kws) in _w5_groups():
        for (aw, kw) in kws:
            blocks[g] = wt_full[kh, kw, ci * 128:(ci + 1) * 128, :]
            g += 1
    assert g == 200
    return blocks


def _prep_inputs(x, w1, w2, wsc, g1, b1, g2, b2, gsc, bsc):
    xpad = np.zeros((B, CIN, 18, 18), dtype=np.float32)
    xpad[:, :, 1:17, 1:17] = x
    xpad = _round_f32r(xpad)
    w1g = _round_f32r(_regroup_w5(w1.transpose(2, 3, 1, 0)))
    wscg = _round_f32r(_regroup_w5(wsc.transpose(2, 3, 1, 0)))
    w2t = _round_f32r(np.ascontiguousarray(w2.transpose(2, 3, 1, 0)).reshape(9, COUT, COUT))
    gb = np.stack([g1, b1, g2, b2, gsc, bsc]).astype(np.float32)   # [6, 512]
    gbt = np.ascontiguousarray(gb.reshape(6, 4, 128).transpose(2, 0, 1))  # [128, 6, 4]
    return xpad, w1g, wscg, w2t, gbt


def kernel(x, w1, g1, b1, w2, g2, b2, wsc, gsc, bsc, _trace=False, **_kw):
    from concourse.bass_utils import run_bass_kernel_spmd

    x = np.asarray(x, dtype=np.float32)
    xpad, w1g, wscg, w2t, gbt = _prep_inputs(
        np.asarray(x), np.asarray(w1), np.asarray(w2), np.asarray(wsc),
        np.asarray(g1), np.asarray(b1), np.asarray(g2), np.asarray(b2),
        np.asarray(gsc), np.asarray(bsc))

    nc = _get_nc()
    in_maps = []
    for core in range(NCORES):
        in_maps.append({
            "xpad": xpad[core * B_LOC:(core + 1) * B_LOC],
            "w1g": w1g, "wscg": wscg, "w2t": w2t, "gb": gbt,
            "zpad": np.zeros((2, 34, 34), dtype=np.float32),
        })
    res = run_bass_kernel_spmd(nc, in_maps, list(range(NCORES)), trace=_trace)
    out = np.concatenate([res.results[i]["out"] for i in range(NCORES)], axis=0)
    if _trace:
        _CACHE["last_result"] = res
    return out



# revision 24
# speedup vs baseline: 1.2973x; 1.2973x over previous
"""Trainium2 Bass kernel for the Gudi UpProj block — Winograd + bf16.

Reference computation (per image, NCHW):
    xu  = zero_stuff_2x(x)                    # [B,1024,32,32]
    c1  = conv5x5(xu, w1, pad=2);  out1 = relu(BN(c1))
    c2  = conv3x3(out1, w2, pad=1)
    csc = conv5x5(xu, wsc, pad=2)
    out = relu(BN(c2) + BN(csc))              # BN: batch stats over (N,H,W)

Strategy:
  * Data-parallel over batch: 16 images -> 2 per core (8 cores).
  * Zero-stuffing: the 5x5 conv decomposes into 4 parity phases with
    3x3 / 3x2 / 2x3 / 2x2 kernels on the 16x16 grid.
  * 1D Winograd along h on each phase: F(2,3) for 3-tap rows, F(2,2) for
    2-tap rows; w taps stay direct -> 25 -> 17.5 effective taps/quad.
  * conv2 (3x3) via full 2D Winograd F(2x2,3x3): 9 -> 4 eff taps.
  * All matmuls in bf16 (full PE rate at any N; PSUM accumulates f32).
  * BN batch stats cross-core via two small AllReduces (c1; c2+csc).
"""

import numpy as np
import ml_dtypes

NCORES = 8
B = 16
B_LOC = B // NCORES
CIN, COUT = 1024, 512
NCI, NCO = CIN // 128, COUT // 128   # 8, 4
EPS = 1e-5
CNT = float(B * 32 * 32)
PHASES = [(0, 0), (0, 1), (1, 0), (1, 1)]
BF16 = ml_dtypes.bfloat16

# F(2,3): 3-tap kernel, 2 outputs, 4 positions; F(2,2): 2-tap, 2 out, 3 pos.
G3 = np.array([[1, 0, 0], [.5, .5, .5], [.5, -.5, .5], [0, 0, 1]], np.float64)
G2 = np.array([[1, 0], [1, 1], [0, 1]], np.float64)
# Output transforms hardcoded in drain code:
#   F(2,3): y0 = m0+m1+m2 ; y1 = m1-m2-m3
#   F(2,2): y0 = m0+m1    ; y1 = m1+m2

_CACHE = {}


def _aws(q):
    return (-1, 0, 1) if q == 0 else (0, 1)


def _ht(p):
    return 4 if p == 0 else 3


def _w5_order():
    """(phase, s, ci, aw) emission order for conv5 weight blocks."""
    order = []
    for (p, q) in PHASES:
        for s in range(_ht(p)):
            for ci in range(NCI):
                for aw in _aws(q):
                    order.append((p, q, ci, s, aw))
    return order


N_W5 = len(_w5_order())  # 280


def _w5_blocks(w):
    """w [Cout, Cin, 5, 5] f32 -> [280, 128, COUT] transformed blocks (f32)."""
    w = np.asarray(w, np.float64)
    out = np.empty((N_W5, 128, COUT), np.float32)
    for g, (p, q, ci, s, aw) in enumerate(_w5_order()):
        Gm = G3 if p == 0 else G2
        nk = 3 - p
        acc = np.zeros((COUT, 128), np.float64)
        for k in range(nk):
            kh = 2 * k + p
            kw = 2 * (aw + 1) if q == 0 else 2 * aw + 1
            acc += Gm[s, k] * w[:, ci * 128:(ci + 1) * 128, kh, kw]
        out[g] = acc.T
    return out


def _w2_blocks(w2):
    """w2 [Cout, Cin, 3, 3] -> [16, 128, 16*128]: per (co, s): [c, (ci sw), m].

    DMA'd per (co, s) with per-partition-contiguous 4KB lines.
    """
    W2p = np.einsum('sr,ocrv,wv->swoc', G3, np.asarray(w2, np.float64), G3)
    out = np.empty((16, 128, 16, 128), np.float32)
    for co in range(NCO):
        for s in range(4):
            for ci in range(4):
                for sw in range(4):
                    blk = W2p[s, sw, co * 128:(co + 1) * 128,
                              ci * 128:(ci + 1) * 128]   # [m, c]
                    out[co * 4 + s, :, ci * 4 + sw, :] = blk.T
    return out.reshape(16, 128, 16 * 128)


def _build_nc():
    import concourse.mybir as mybir
    import concourse.tile as tile
    from concourse import bacc

    f32 = mybir.dt.float32
    bf = mybir.dt.bfloat16
    ALU = mybir.AluOpType
    AFT = mybir.ActivationFunctionType

    nc = bacc.Bacc("TRN2", target_bir_lowering=False, debug=False)

    xpad_d = nc.dram_tensor("xpad", [B_LOC, CIN, 18, 18], bf, kind="ExternalInput").ap()
    w1g_d = nc.dram_tensor("w1g", [N_W5, 128, COUT], bf, kind="ExternalInput").ap()
    wscg_d = nc.dram_tensor("wscg", [N_W5, 128, COUT], bf, kind="ExternalInput").ap()
    w2g_d = nc.dram_tensor("w2g", [16, 128, 16 * 128], bf, kind="ExternalInput").ap()
    gb_d = nc.dram_tensor("gb", [128, 6, 4], f32, kind="ExternalInput").ap()
    out_d = nc.dram_tensor("out", [B_LOC, COUT, 32, 32], f32, kind="ExternalOutput").ap()

    def r256(ap):
        return ap.rearrange("c (b t w) -> c b t w", b=2, t=8, w=16)

    def m128b(t, half, b):
        return t[:, half * 256 + b * 128:half * 256 + (b + 1) * 128].rearrange(
            "c (t w) -> c t w", t=8)

    with tile.TileContext(nc) as tc:
        _frees = []
        ar1_in, _f = tc.tile([2, COUT], f32, space="DRAM", name="ar1_in"); _frees.append(_f)
        ar1_out, _f = tc.tile([2, COUT], f32, space="DRAM", addr_space="Shared", name="ar1_out"); _frees.append(_f)
        ar2_in, _f = tc.tile([4, COUT], f32, space="DRAM", name="ar2_in"); _frees.append(_f)
        ar2_out, _f = tc.tile([4, COUT], f32, space="DRAM", addr_space="Shared", name="ar2_out"); _frees.append(_f)

        with tc.tile_pool(name="xp", bufs=1) as xp_pool, \
             tc.tile_pool(name="uv", bufs=1) as uv_pool, \
             tc.tile_pool(name="acts", bufs=1) as acts, \
             tc.tile_pool(name="op1", bufs=1) as op1_pool, \
             tc.tile_pool(name="u2", bufs=1) as u2_pool, \
             tc.tile_pool(name="wts", bufs=4) as wts, \
             tc.tile_pool(name="w2p", bufs=2) as w2p, \
             tc.tile_pool(name="scr", bufs=1) as scr_pool, \
             tc.tile_pool(name="small", bufs=1) as small, \
             tc.tile_pool(name="ps", bufs=8, space="PSUM") as ps:

            # ---- persistent SBUF tensors ----
            XP = [xp_pool.tile([128, 2, 18, 18], bf, name=f"xp{i}", tag=f"xp{i}")
                  for i in range(NCI)]
            # U: h-Winograd F(2,3) positions for p=0 phases; V: F(2,2) (v0,v2)
            U = [uv_pool.tile([128, 4, 2, 8, 18], bf, name=f"u{i}", tag=f"u{i}")
                 for i in range(NCI)]
            V = [uv_pool.tile([128, 2, 2, 8, 18], bf, name=f"v{i}", tag=f"v{i}")
                 for i in range(NCI)]
            # OP1: BN1(relu) output, padded, parity-split cols:
            # [c, b, h(0..33), par, wp]; par0 = even cols 0..32, par1 = odd 1..33
            OP1 = [op1_pool.tile([128, 2, 34, 2, 17], bf, name=f"op1_{i}", tag=f"op1_{i}")
                   for i in range(NCO)]
            # CSC / C2: phase-major [c, ph(2p+q), b, i, j]
            CSC = [acts.tile([128, 4, 2, 16, 16], bf, name=f"csc{i}", tag=f"csc{i}")
                   for i in range(NCO)]
            C2 = [acts.tile([128, 4, 2, 16, 16], bf, name=f"c2_{i}", tag=f"c2_{i}")
                  for i in range(NCO)]
            # U2: conv2 2D-Winograd input transform, per OP1 tile (=ci2)
            U2 = [u2_pool.tile([128, 16, 2, 16, 16], bf, name=f"u2_{i}", tag=f"u2_{i}")
                  for i in range(NCO)]

            # stats: conv5 sums col = ((co*4 + iph)*2 + j)*2 + b (per-image,
            # since the split drain ops each carry their own accum); sq col =
            # (co*4 + iph)*2 + j; conv2 col = co*4 + ph
            sums1 = small.tile([128, 64], f32, name="sums1")
            sq1 = small.tile([128, 64], f32, name="sq1")
            sumssc = small.tile([128, 64], f32, name="sumssc")
            sqsc = small.tile([128, 64], f32, name="sqsc")
            sums2 = small.tile([128, 16], f32, name="sums2")
            sq2 = small.tile([128, 16], f32, name="sq2")
            pack1 = small.tile([128, 2, 4], f32, name="pack1")
            pack2 = small.tile([128, 4, 4], f32, name="pack2")
            st1 = small.tile([128, 2, 4], f32, name="st1")
            st2 = small.tile([128, 2, 2, 4], f32, name="st2")
            gbv = small.tile([128, 6, 4], f32, name="gbv")
            scale1 = small.tile([128, 4], f32, name="scale1")
            shift1 = small.tile([128, 4], f32, name="shift1")
            m2b = small.tile([128, 2, 4], f32, name="m2b")
            varb = small.tile([128, 2, 4], f32, name="varb")
            invb = small.tile([128, 2, 4], f32, name="invb")
            scaleb = small.tile([128, 2, 4], f32, name="scaleb")
            shiftb2 = small.tile([128, 2, 4], f32, name="shiftb2")
            shiftB = small.tile([128, 4], f32, name="shiftB")
            rmix = small.tile([128, 4], f32, name="rmix")
            tmpa = small.tile([128, 4], f32, name="tmpa")
            tmpb = small.tile([128, 4], f32, name="tmpb")
            epsc = small.tile([128, 1], f32, name="epsc")
            sqscr = small.tile([128, 512], f32, name="sqscr")

            # ---- helpers ----
            def rowpair(xp, lo, parity):
                """rows lo..lo+15 of an 18-row dim, split in pairs, pick one."""
                return xp[:, :, lo:lo + 16].rearrange(
                    "c b (t two) w -> c b t two w", two=2)[:, :, :, parity, :]

            def emit_xp(ci):
                nc.sync.dma_start(
                    XP[ci][:].rearrange("c b h w -> c b (h w)"),
                    xpad_d[:, ci * 128:(ci + 1) * 128].rearrange("b c h w -> c b (h w)"))

            def emit_uv(ci):
                # per-image ops: walrus caps DVE APs at 3 canonical dims
                xp = XP[ci]
                for b in range(B_LOC):
                    d0 = rowpair(xp, 0, 0)[:, b]   # xpad rows 0,2..14
                    d1 = rowpair(xp, 1, 0)[:, b]   # 1,3..15
                    d2 = rowpair(xp, 2, 0)[:, b]   # 2,4..16
                    d3 = rowpair(xp, 2, 1)[:, b]   # 3,5..17
                    nc.vector.tensor_tensor(U[ci][:, 0, b], d0, d2, op=ALU.subtract)
                    nc.vector.tensor_tensor(U[ci][:, 1, b], d1, d2, op=ALU.add)
                    nc.vector.tensor_tensor(U[ci][:, 2, b], d2, d1, op=ALU.subtract)
                    nc.vector.tensor_tensor(U[ci][:, 3, b], d1, d3, op=ALU.subtract)
                    # p=1 window rows: d0'=d1, d1'=d2, d2'=d3:
                    # v0 = d0'-d1', v1 = d1' (XP view), v2 = d2'-d1'
                    nc.vector.tensor_tensor(V[ci][:, 0, b], d1, d2, op=ALU.subtract)
                    nc.vector.tensor_tensor(V[ci][:, 1, b], d3, d2, op=ALU.subtract)

            def v_rhs(ci, s, wsl):
                if s == 0:
                    return V[ci][:, 0, :, :, wsl]
                if s == 1:
                    return rowpair(XP[ci], 2, 0)[:, :, :, wsl]
                return V[ci][:, 1, :, :, wsl]

            def u_rhs(ci, s, wsl):
                return U[ci][:, s, :, :, wsl]

            def op1_dst(co, p, q, j):
                """conv1 drain dest: OP1 interior phase view [c, b, t, 16]."""
                par = 1 - q          # q=0 -> odd cols (1..31), q=1 -> even (2..32)
                wsl = slice(0, 16) if q == 0 else slice(1, 17)
                return OP1[co][:, :, 1:33].rearrange(
                    "c b (t four) par w -> c b t four par w", four=4)[
                        :, :, :, 2 * j + p, par, wsl]

            def csc_dst(co, p, q, j):
                iph = 2 * p + q
                return CSC[co][:, iph].rearrange(
                    "c b (t two) w -> c b t two w", two=2)[:, :, :, j, :]

            # ---- startup DMAs / init ----
            emit_xp(0)
            emit_xp(1)
            nc.vector.memset(epsc[:], EPS)
            for co in range(NCO):
                nc.gpsimd.memset(OP1[co][:].rearrange("c b h p w -> c b (h p w)"), 0.0)
            emit_uv(0)
            emit_uv(1)

            # ---- conv5: phase / h-position / ci loops ----
            # Per s-position: 4 PSUM chains (one per co), accumulated over
            # (ci, aw). Drains fold the h-inverse incrementally, in place in
            # the bf16 destination:
            #   p=0: y0 = m0+m1+m2 ; y1 = m1-m2-m3
            #   p=1: y0 = m0+m1    ; y1 = m1+m2
            def conv5(wg_d, dst_fn, sums, sqs, wtag, prefetch=False):
                gofs = 0
                for iph, (p, q) in enumerate(PHASES):
                    aws = _aws(q)
                    nblk = len(aws)
                    rhs_fn = u_rhs if p == 0 else v_rhs
                    for s in range(_ht(p)):
                        pps = [ps.tile([128, 256], f32, name=f"{wtag}ps{iph}{s}_{co}",
                                       tag="psb") for co in range(NCO)]
                        for ci in range(NCI):
                            if prefetch and iph == 0 and s == 0 and ci + 2 < NCI:
                                emit_xp(ci + 2)
                                emit_uv(ci + 2)
                            wt = wts.tile([128, 3, 512], bf, name=f"{wtag}w", tag="w5")
                            nc.sync.dma_start(
                                wt[:, :nblk, :],
                                wg_d[gofs:gofs + nblk].rearrange("l c m -> c l m"))
                            gofs += nblk
                            for ai, aw in enumerate(aws):
                                rhs = rhs_fn(ci, s, slice(1 + aw, 17 + aw))
                                first = ci == 0 and ai == 0
                                last = ci == NCI - 1 and ai == nblk - 1
                                for co in range(NCO):
                                    nc.tensor.matmul(
                                        pps[co][:], wt[:, ai, co * 128:(co + 1) * 128],
                                        rhs, start=first, stop=last)
                        # ---- drain position s into y0/y1 (in-place partials) --
                        for co in range(NCO):
                            c0 = ((co * 4 + iph) * 2) * 2
                            for b in range(B_LOC):
                                m = pps[co][:, b * 128:(b + 1) * 128].rearrange(
                                    "c (t w) -> c t w", t=8)
                                y0 = dst_fn(co, p, q, 0)[:, b]
                                y1 = dst_fn(co, p, q, 1)[:, b]
                                s0 = sums[:, c0 + b:c0 + b + 1]
                                s1 = sums[:, c0 + 2 + b:c0 + 3 + b]
                                sg = 1.0
                                y0_ops, y1_ops = [], []
                                if s == 0:
                                    y0_ops = ["init"]
                                elif s == 1:
                                    y0_ops = ["acc"]
                                    y1_ops = ["init"]
                                elif s == 2:
                                    if p == 0:
                                        y0_ops = ["acc_fin"]
                                        y1_ops = ["neg"]
                                    else:
                                        y1_ops = ["acc_fin"]
                                else:
                                    y1_ops = ["neg_fin"]
                                if p == 1 and s == 1:
                                    y0_ops = ["acc_fin"]
                                for tgt, ops_, scol in ((y0, y0_ops, s0),
                                                        (y1, y1_ops, s1)):
                                    for opk in ops_:
                                        if opk == "init":
                                            nc.vector.tensor_scalar(
                                                tgt, m, 0.0, 0.0,
                                                op0=ALU.add, op1=ALU.add)
                                        else:
                                            fin = opk.endswith("fin")
                                            sgn = -1.0 if opk.startswith("neg") else 1.0
                                            nc.vector.scalar_tensor_tensor(
                                                tgt, m, sgn, tgt,
                                                op0=ALU.mult, op1=ALU.add,
                                                accum_out=(scol if fin else None))
                                if s == _ht(p) - 1:
                                    nc.scalar.activation(
                                        m128b(sqscr, 0, b), y0, AFT.Square,
                                        accum_out=sqs[:, c0 + b:c0 + b + 1])
                                    nc.scalar.activation(
                                        m128b(sqscr, 1, b), y1, AFT.Square,
                                        accum_out=sqs[:, c0 + 2 + b:c0 + 3 + b])

            # ================= conv1 =================
            conv5(w1g_d, op1_dst, sums1, sq1, "c1", prefetch=True)
            nc.sync.dma_start(gbv[:], gb_d)

            # ---- c1 stats -> AllReduce #1 (overlaps convsc) ----
            nc.vector.tensor_reduce(
                pack1[:, 0, :], sums1[:].rearrange("c (co x) -> c co x", x=16),
                axis=mybir.AxisListType.X, op=ALU.add)
            nc.vector.tensor_reduce(
                pack1[:, 1, :], sq1[:].rearrange("c (co x) -> c co x", x=16),
                axis=mybir.AxisListType.X, op=ALU.add)
            nc.sync.dma_start(ar1_in[:].rearrange("s (co c) -> c s co", c=128), pack1[:])
            nc.gpsimd.collective_compute(
                "AllReduce", ALU.add,
                replica_groups=[list(range(NCORES))],
                ins=[ar1_in.opt()], outs=[ar1_out.opt()])
            nc.sync.dma_start(st1[:], ar1_out[:].rearrange("s (co c) -> c s co", c=128))

            # ================= convsc =================
            conv5(wscg_d, csc_dst, sumssc, sqsc, "sc")

            # ---- BN1 scale/shift ----
            nc.vector.tensor_scalar_mul(st1[:], st1[:], 1.0 / CNT)
            m1 = st1[:, 0, :]
            nc.vector.tensor_tensor(tmpa[:], m1, m1, op=ALU.mult)
            nc.vector.tensor_tensor(tmpb[:], st1[:, 1, :], tmpa[:], op=ALU.subtract)
            nc.scalar.activation(tmpb[:], tmpb[:], AFT.Sqrt, bias=epsc[:])
            nc.vector.reciprocal(tmpa[:], tmpb[:])
            nc.vector.tensor_tensor(scale1[:], gbv[:, 0, :], tmpa[:], op=ALU.mult)
            nc.vector.tensor_tensor(tmpa[:], m1, scale1[:], op=ALU.mult)
            nc.vector.tensor_tensor(shift1[:], gbv[:, 1, :], tmpa[:], op=ALU.subtract)

            # ---- BN1 apply (in-place relu) + conv2 input transform ----
            UH = scr_pool.tile([128, 4, 2, 16, 34], bf, name="uh", tag="uh")
            for co in range(NCO):
                for par, wsl in ((0, slice(1, 17)), (1, slice(0, 16))):
                    for b in range(B_LOC):
                        v = OP1[co][:, b, 1:33, par, wsl]
                        nc.scalar.activation(v, v, AFT.Relu,
                                             bias=shift1[:, co:co + 1],
                                             scale=scale1[:, co:co + 1])
                # H-stage: window rows 2t..2t+3 of padded OP1 (34 rows)
                flat = OP1[co][:].rearrange("c b h par w -> c b h (par w)")

                def rows(lo, parity, b):
                    return flat[:, b, lo:lo + 32].rearrange(
                        "c (t two) w -> c t two w", two=2)[:, :, parity, :]

                for b in range(B_LOC):
                    d0 = rows(0, 0, b)
                    d1 = rows(1, 0, b)
                    d2 = rows(2, 0, b)
                    d3 = rows(2, 1, b)
                    nc.vector.tensor_tensor(UH[:, 0, b], d0, d2, op=ALU.subtract)
                    nc.vector.tensor_tensor(UH[:, 1, b], d1, d2, op=ALU.add)
                    nc.vector.tensor_tensor(UH[:, 2, b], d2, d1, op=ALU.subtract)
                    nc.vector.tensor_tensor(UH[:, 3, b], d1, d3, op=ALU.subtract)
                # W-stage: (par w) cols: even block wp0..16, odd block 17..33.
                # tile wt: d0 = E[wt], d1 = O[wt], d2 = E[wt+1], d3 = O[wt+1]
                for s in range(4):
                    E0 = UH[:, s, :, :, 0:16]
                    E1 = UH[:, s, :, :, 1:17]
                    O0 = UH[:, s, :, :, 17:33]
                    O1 = UH[:, s, :, :, 18:34]
                    nc.vector.tensor_tensor(U2[co][:, 4 * s + 0], E0, E1, op=ALU.subtract)
                    nc.vector.tensor_tensor(U2[co][:, 4 * s + 1], O0, E1, op=ALU.add)
                    nc.vector.tensor_tensor(U2[co][:, 4 * s + 2], E1, O0, op=ALU.subtract)
                    nc.vector.tensor_tensor(U2[co][:, 4 * s + 3], O0, O1, op=ALU.subtract)

            # ---- csc stats into pack2 rows 2,3 ----
            nc.vector.tensor_reduce(
                pack2[:, 2, :], sumssc[:].rearrange("c (co x) -> c co x", x=16),
                axis=mybir.AxisListType.X, op=ALU.add)
            nc.vector.tensor_reduce(
                pack2[:, 3, :], sqsc[:].rearrange("c (co x) -> c co x", x=16),
                axis=mybir.AxisListType.X, op=ALU.add)

            # ================= conv2 (2D Winograd F(2x2,3x3)) =================
            # co outer; per (co, s): 4 sw-chains accumulate over ci2.
            # w-inverse per (co, s) -> T[s, a]; h-inverse per co -> C2 + stats.
            for co in range(NCO):
                T = scr_pool.tile([128, 4, 2, 512], bf, name="T", tag="T", bufs=1)
                for s in range(4):
                    wt2 = w2p.tile([128, 16, 128], bf, name="c2w", tag="w2")
                    nc.sync.dma_start(
                        wt2[:].rearrange("c l m -> c (l m)"),
                        w2g_d[co * 4 + s])
                    pps = [ps.tile([128, 512], f32, name=f"c2ps{co}{s}_{sw}",
                                   tag="psb") for sw in range(4)]
                    for ci in range(NCO):
                        for sw in range(4):
                            nc.tensor.matmul(
                                pps[sw][:], wt2[:, ci * 4 + sw, :],
                                U2[ci][:, 4 * s + sw].rearrange("c b t w -> c (b t w)"),
                                start=(ci == 0), stop=(ci == NCO - 1))
                    # w-inverse: T0 = m0+m1+m2 ; T1 = m1-m2-m3 (1 psum read/op)
                    a2 = scr_pool.tile([128, 512], f32, name="a2", tag="a2")
                    q0 = scr_pool.tile([128, 512], f32, name="q0", tag="q0")
                    nc.vector.tensor_scalar(a2[:], pps[1][:], 0.0, 0.0,
                                            op0=ALU.add, op1=ALU.add)
                    nc.vector.scalar_tensor_tensor(q0[:], pps[0][:], 1.0, a2[:],
                                                   op0=ALU.mult, op1=ALU.add)
                    nc.vector.scalar_tensor_tensor(T[:, s, 0], pps[2][:], 1.0,
                                                   q0[:], op0=ALU.mult, op1=ALU.add)
                    nc.vector.scalar_tensor_tensor(q0[:], pps[2][:], -1.0, a2[:],
                                                   op0=ALU.mult, op1=ALU.add)
                    nc.vector.scalar_tensor_tensor(T[:, s, 1], pps[3][:], -1.0,
                                                   q0[:], op0=ALU.mult, op1=ALU.add)
                # h-inverse: y(jh,a) -> C2 phase (2*jh + a); + stats
                hscr = scr_pool.tile([128, 512], f32, name="hscr", tag="a2")
                for a in range(2):
                    for jh in range(2):
                        ph = 2 * jh + a
                        dst = C2[co][:, ph].rearrange("c b t w -> c (b t w)")
                        scol = sums2[:, co * 4 + ph:co * 4 + ph + 1]
                        if jh == 0:
                            nc.vector.tensor_tensor(hscr[:], T[:, 0, a],
                                                    T[:, 1, a], op=ALU.add)
                            nc.vector.scalar_tensor_tensor(
                                dst, T[:, 2, a], 1.0, hscr[:],
                                op0=ALU.mult, op1=ALU.add, accum_out=scol)
                        else:
                            nc.vector.tensor_tensor(hscr[:], T[:, 1, a],
                                                    T[:, 2, a], op=ALU.subtract)
                            nc.vector.scalar_tensor_tensor(
                                dst, T[:, 3, a], -1.0, hscr[:],
                                op0=ALU.mult, op1=ALU.add, accum_out=scol)
                        nc.scalar.activation(
                            sqscr[:], dst, AFT.Square,
                            accum_out=sq2[:, co * 4 + ph:co * 4 + ph + 1])

            # ---- stats of c2 -> AllReduce #2 ----
            nc.vector.tensor_reduce(
                pack2[:, 0, :], sums2[:].rearrange("c (co x) -> c co x", x=4),
                axis=mybir.AxisListType.X, op=ALU.add)
            nc.vector.tensor_reduce(
                pack2[:, 1, :], sq2[:].rearrange("c (co x) -> c co x", x=4),
                axis=mybir.AxisListType.X, op=ALU.add)
            nc.sync.dma_start(ar2_in[:].rearrange("s (co c) -> c s co", c=128), pack2[:])
            nc.gpsimd.collective_compute(
                "AllReduce", ALU.add,
                replica_groups=[list(range(NCORES))],
                ins=[ar2_in.opt()], outs=[ar2_out.opt()])
            nc.sync.dma_start(
                st2[:].rearrange("c g s co -> c (g s) co"),
                ar2_out[:].rearrange("s (co c) -> c s co", c=128))

            # ---- BN2 / BNsc scale+shift ----
            # final = relu(s2*(c2 + (ssc/s2)*csc) + (t2 + tsc))
            nc.vector.tensor_scalar_mul(st2[:], st2[:], 1.0 / CNT)
            means = st2[:, :, 0, :]
            e2s = st2[:, :, 1, :]
            gpair = gbv[:, 2:, :].rearrange("c (g s) co -> c g s co", s=2)
            nc.vector.tensor_tensor(m2b[:], means, means, op=ALU.mult)
            nc.vector.tensor_tensor(varb[:], e2s, m2b[:], op=ALU.subtract)
            nc.scalar.activation(varb[:], varb[:], AFT.Sqrt, bias=epsc[:])
            nc.vector.reciprocal(invb[:], varb[:])
            nc.vector.tensor_tensor(scaleb[:], gpair[:, :, 0, :], invb[:], op=ALU.mult)
            nc.vector.tensor_tensor(m2b[:], means, scaleb[:], op=ALU.mult)
            nc.vector.tensor_tensor(shiftb2[:], gpair[:, :, 1, :], m2b[:], op=ALU.subtract)
            nc.vector.tensor_tensor(shiftB[:], shiftb2[:, 0, :], shiftb2[:, 1, :], op=ALU.add)
            nc.vector.reciprocal(tmpa[:], scaleb[:, 0, :])
            nc.vector.tensor_tensor(rmix[:], scaleb[:, 1, :], tmpa[:], op=ALU.mult)

            # ---- final fuse: c2 += rmix*csc ; out = relu(scale2*c2 + shiftB) ----
            for co in range(NCO):
                nc.vector.scalar_tensor_tensor(
                    C2[co][:].rearrange("c p b t w -> c (p b t w)"),
                    CSC[co][:].rearrange("c p b t w -> c (p b t w)"),
                    rmix[:, co:co + 1],
                    C2[co][:].rearrange("c p b t w -> c (p b t w)"),
                    op0=ALU.mult, op1=ALU.add)
                for b in range(B_LOC):
                    fin = scr_pool.tile([128, 1024], f32, name="fin", tag="fin",
                                        bufs=2)
                    for iph, (p, q) in enumerate(PHASES):
                        dst = fin.rearrange(
                            "c (th p2 tw q2) -> c th p2 tw q2",
                            th=16, p2=2, q2=2)[:, :, p, :, q]
                        nc.scalar.activation(dst, C2[co][:, iph, b], AFT.Relu,
                                             bias=shiftB[:, co:co + 1],
                                             scale=scaleb[:, 0, co:co + 1])
                    nc.sync.dma_start(
                        out_d[b, co * 128:(co + 1) * 128].rearrange("c h w -> c (h w)"),
                        fin[:])

            for _f in _frees:
                _f()

    nc.compile()
    return nc


def _get_nc():
    if "nc" not in _CACHE:
        _CACHE["nc"] = _build_nc()
    return _CACHE["nc"]


def _prep_inputs(x, w1, w2, wsc, g1, b1, g2, b2, gsc, bsc):
    xpad = np.zeros((B, CIN, 18, 18), dtype=np.float32)
    xpad[:, :, 1:17, 1:17] = x
    xpad = xpad.astype(BF16)
    w1g = _w5_blocks(w1).astype(BF16)
    wscg = _w5_blocks(wsc).astype(BF16)
    w2g = _w2_blocks(w2).astype(BF16)
    gb = np.stack([g1, b1, g2, b2, gsc, bsc]).astype(np.float32)   # [6, 512]
    gbt = np.ascontiguousarray(gb.reshape(6, 4, 128).transpose(2, 0, 1))  # [128, 6, 4]
    return xpad, w1g, wscg, w2g, gbt


def kernel(x, w1, g1, b1, w2, g2, b2, wsc, gsc, bsc, _trace=False, **_kw):
    from concourse.bass_utils import run_bass_kernel_spmd

    x = np.asarray(x, dtype=np.float32)
    xpad, w1g, wscg, w2g, gbt = _prep_inputs(
        np.asarray(x), np.asarray(w1), np.asarray(w2), np.asarray(wsc),
        np.asarray(g1), np.asarray(b1), np.asarray(g2), np.asarray(b2),
        np.asarray(gsc), np.asarray(bsc))

    nc = _get_nc()
    in_maps = []
    for core in range(NCORES):
        in_maps.append({
            "xpad": xpad[core * B_LOC:(core + 1) * B_LOC],
            "w1g": w1g, "wscg": wscg, "w2g": w2g, "gb": gbt,
        })
    res = run_bass_kernel_spmd(nc, in_maps, list(range(NCORES)), trace=_trace)
    out = np.concatenate([res.results[i]["out"] for i in range(NCORES)], axis=0)
    if _trace:
        _CACHE["last_result"] = res
    return out


# revision 26
# speedup vs baseline: 1.3365x; 1.0302x over previous
"""Trainium2 Bass kernel for the Gudi UpProj block — Winograd + bf16.

Reference computation (per image, NCHW):
    xu  = zero_stuff_2x(x)                    # [B,1024,32,32]
    c1  = conv5x5(xu, w1, pad=2);  out1 = relu(BN(c1))
    c2  = conv3x3(out1, w2, pad=1)
    csc = conv5x5(xu, wsc, pad=2)
    out = relu(BN(c2) + BN(csc))              # BN: batch stats over (N,H,W)

Strategy:
  * Data-parallel over batch: 16 images -> 2 per core (8 cores).
  * Zero-stuffing: the 5x5 conv decomposes into 4 parity phases with
    3x3 / 3x2 / 2x3 / 2x2 kernels on the 16x16 grid.
  * 1D Winograd along h on each phase: F(2,3) for 3-tap rows, F(2,2) for
    2-tap rows; w taps stay direct -> 25 -> 17.5 effective taps/quad.
  * conv2 (3x3) via full 2D Winograd F(2x2,3x3): 9 -> 4 eff taps.
  * All matmuls in bf16 (full PE rate at any N; PSUM accumulates f32).
  * BN batch stats cross-core via two small AllReduces (c1; c2+csc).
"""

import numpy as np
import ml_dtypes

NCORES = 8
B = 16
B_LOC = B // NCORES
CIN, COUT = 1024, 512
NCI, NCO = CIN // 128, COUT // 128   # 8, 4
EPS = 1e-5
CNT = float(B * 32 * 32)
PHASES = [(0, 0), (0, 1), (1, 0), (1, 1)]
BF16 = ml_dtypes.bfloat16

# F(2,3): 3-tap kernel, 2 outputs, 4 positions; F(2,2): 2-tap, 2 out, 3 pos.
G3 = np.array([[1, 0, 0], [.5, .5, .5], [.5, -.5, .5], [0, 0, 1]], np.float64)
G2 = np.array([[1, 0], [1, 1], [0, 1]], np.float64)
# Output transforms hardcoded in drain code:
#   F(2,3): y0 = m0+m1+m2 ; y1 = m1-m2-m3
#   F(2,2): y0 = m0+m1    ; y1 = m1+m2

_CACHE = {}


def _aws(q):
    return (-1, 0, 1) if q == 0 else (0, 1)


def _ht(p):
    return 4 if p == 0 else 3


def _w5_order():
    """(phase, s, ci, aw) emission order for conv5 weight blocks."""
    order = []
    for (p, q) in PHASES:
        for s in range(_ht(p)):
            for ci in range(NCI):
                for aw in _aws(q):
                    order.append((p, q, ci, s, aw))
    return order


N_W5 = len(_w5_order())  # 280


def _w5_blocks(w):
    """w [Cout, Cin, 5, 5] f32 -> [280, 128, COUT] transformed blocks (f32)."""
    w = np.asarray(w, np.float64)
    out = np.empty((N_W5, 128, COUT), np.float32)
    for g, (p, q, ci, s, aw) in enumerate(_w5_order()):
        Gm = G3 if p == 0 else G2
        nk = 3 - p
        acc = np.zeros((COUT, 128), np.float64)
        for k in range(nk):
            kh = 2 * k + p
            kw = 2 * (aw + 1) if q == 0 else 2 * aw + 1
            acc += Gm[s, k] * w[:, ci * 128:(ci + 1) * 128, kh, kw]
        out[g] = acc.T
    return out


def _w2_blocks(w2):
    """w2 [Cout, Cin, 3, 3] -> [16, 128, 16*128]: per (co, s): [c, (ci sw), m].

    DMA'd per (co, s) with per-partition-contiguous 4KB lines.
    """
    W2p = np.einsum('sr,ocrv,wv->swoc', G3, np.asarray(w2, np.float64), G3)
    out = np.empty((16, 128, 16, 128), np.float32)
    for co in range(NCO):
        for s in range(4):
            for ci in range(4):
                for sw in range(4):
                    blk = W2p[s, sw, co * 128:(co + 1) * 128,
                              ci * 128:(ci + 1) * 128]   # [m, c]
                    out[co * 4 + s, :, ci * 4 + sw, :] = blk.T
    return out.reshape(16, 128, 16 * 128)


def _build_nc():
    import concourse.mybir as mybir
    import concourse.tile as tile
    from concourse import bacc

    f32 = mybir.dt.float32
    bf = mybir.dt.bfloat16
    ALU = mybir.AluOpType
    AFT = mybir.ActivationFunctionType

    nc = bacc.Bacc("TRN2", target_bir_lowering=False, debug=False)

    xpad_d = nc.dram_tensor("xpad", [B_LOC, CIN, 18, 18], bf, kind="ExternalInput").ap()
    w1g_d = nc.dram_tensor("w1g", [N_W5, 128, COUT], bf, kind="ExternalInput").ap()
    wscg_d = nc.dram_tensor("wscg", [N_W5, 128, COUT], bf, kind="ExternalInput").ap()
    w2g_d = nc.dram_tensor("w2g", [16, 128, 16 * 128], bf, kind="ExternalInput").ap()
    gb_d = nc.dram_tensor("gb", [128, 6, 4], f32, kind="ExternalInput").ap()
    out_d = nc.dram_tensor("out", [B_LOC, COUT, 32, 32], f32, kind="ExternalOutput").ap()

    def r256(ap):
        return ap.rearrange("c (b t w) -> c b t w", b=2, t=8, w=16)

    def m128b(t, half, b):
        return t[:, half * 256 + b * 128:half * 256 + (b + 1) * 128].rearrange(
            "c (t w) -> c t w", t=8)

    with tile.TileContext(nc) as tc:
        _frees = []
        ar1_in, _f = tc.tile([2, COUT], f32, space="DRAM", name="ar1_in"); _frees.append(_f)
        ar1_out, _f = tc.tile([2, COUT], f32, space="DRAM", addr_space="Shared", name="ar1_out"); _frees.append(_f)
        ar2_in, _f = tc.tile([4, COUT], f32, space="DRAM", name="ar2_in"); _frees.append(_f)
        ar2_out, _f = tc.tile([4, COUT], f32, space="DRAM", addr_space="Shared", name="ar2_out"); _frees.append(_f)

        with tc.tile_pool(name="xp", bufs=1) as xp_pool, \
             tc.tile_pool(name="uv", bufs=1) as uv_pool, \
             tc.tile_pool(name="acts", bufs=1) as acts, \
             tc.tile_pool(name="op1", bufs=1) as op1_pool, \
             tc.tile_pool(name="u2", bufs=1) as u2_pool, \
             tc.tile_pool(name="wts", bufs=4) as wts, \
             tc.tile_pool(name="w2p", bufs=2) as w2p, \
             tc.tile_pool(name="scr", bufs=1) as scr_pool, \
             tc.tile_pool(name="small", bufs=1) as small, \
             tc.tile_pool(name="ps", bufs=8, space="PSUM") as ps:

            # ---- persistent SBUF tensors ----
            XP = [xp_pool.tile([128, 2, 18, 18], bf, name=f"xp{i}", tag=f"xp{i}")
                  for i in range(NCI)]
            # U: h-Winograd F(2,3) positions for p=0 phases; V: F(2,2) (v0,v2)
            U = [uv_pool.tile([128, 4, 2, 8, 18], bf, name=f"u{i}", tag=f"u{i}")
                 for i in range(NCI)]
            V = [uv_pool.tile([128, 2, 2, 8, 18], bf, name=f"v{i}", tag=f"v{i}")
                 for i in range(NCI)]
            # OP1: BN1(relu) output, padded, parity-split cols:
            # [c, b, h(0..33), par, wp]; par0 = even cols 0..32, par1 = odd 1..33
            OP1 = [op1_pool.tile([128, 2, 34, 2, 17], bf, name=f"op1_{i}", tag=f"op1_{i}")
                   for i in range(NCO)]
            # CSC / C2: phase-major [c, ph(2p+q), b, i, j]
            CSC = [acts.tile([128, 4, 2, 16, 16], bf, name=f"csc{i}", tag=f"csc{i}")
                   for i in range(NCO)]
            C2 = [acts.tile([128, 4, 2, 16, 16], bf, name=f"c2_{i}", tag=f"c2_{i}")
                  for i in range(NCO)]
            # U2: conv2 2D-Winograd input transform, per OP1 tile (=ci2)
            U2 = [u2_pool.tile([128, 16, 2, 16, 16], bf, name=f"u2_{i}", tag=f"u2_{i}")
                  for i in range(NCO)]

            # stats: conv5 sums col = ((co*4 + iph)*2 + j)*2 + b (per-image,
            # since the split drain ops each carry their own accum); sq col =
            # (co*4 + iph)*2 + j; conv2 col = co*4 + ph
            sums1 = small.tile([128, 64], f32, name="sums1")
            sq1 = small.tile([128, 64], f32, name="sq1")
            sumssc = small.tile([128, 64], f32, name="sumssc")
            sqsc = small.tile([128, 64], f32, name="sqsc")
            sums2 = small.tile([128, 16], f32, name="sums2")
            sq2 = small.tile([128, 16], f32, name="sq2")
            pack1 = small.tile([128, 2, 4], f32, name="pack1")
            pack2 = small.tile([128, 4, 4], f32, name="pack2")
            st1 = small.tile([128, 2, 4], f32, name="st1")
            st2 = small.tile([128, 2, 2, 4], f32, name="st2")
            gbv = small.tile([128, 6, 4], f32, name="gbv")
            scale1 = small.tile([128, 4], f32, name="scale1")
            shift1 = small.tile([128, 4], f32, name="shift1")
            m2b = small.tile([128, 2, 4], f32, name="m2b")
            varb = small.tile([128, 2, 4], f32, name="varb")
            invb = small.tile([128, 2, 4], f32, name="invb")
            scaleb = small.tile([128, 2, 4], f32, name="scaleb")
            shiftb2 = small.tile([128, 2, 4], f32, name="shiftb2")
            shiftB = small.tile([128, 4], f32, name="shiftB")
            rmix = small.tile([128, 4], f32, name="rmix")
            tmpa = small.tile([128, 4], f32, name="tmpa")
            tmpb = small.tile([128, 4], f32, name="tmpb")
            epsc = small.tile([128, 1], f32, name="epsc")
            sqscr = small.tile([128, 512], f32, name="sqscr")

            # ---- helpers ----
            def rowpair(xp, lo, parity):
                """rows lo..lo+15 of an 18-row dim, split in pairs, pick one."""
                return xp[:, :, lo:lo + 16].rearrange(
                    "c b (t two) w -> c b t two w", two=2)[:, :, :, parity, :]

            def emit_xp(ci):
                nc.sync.dma_start(
                    XP[ci][:].rearrange("c b h w -> c b (h w)"),
                    xpad_d[:, ci * 128:(ci + 1) * 128].rearrange("b c h w -> c b (h w)"))

            def emit_uv(ci):
                # per-image ops: walrus caps DVE APs at 3 canonical dims
                xp = XP[ci]
                for b in range(B_LOC):
                    d0 = rowpair(xp, 0, 0)[:, b]   # xpad rows 0,2..14
                    d1 = rowpair(xp, 1, 0)[:, b]   # 1,3..15
                    d2 = rowpair(xp, 2, 0)[:, b]   # 2,4..16
                    d3 = rowpair(xp, 2, 1)[:, b]   # 3,5..17
                    nc.vector.tensor_tensor(U[ci][:, 0, b], d0, d2, op=ALU.subtract)
                    nc.vector.tensor_tensor(U[ci][:, 1, b], d1, d2, op=ALU.add)
                    nc.vector.tensor_tensor(U[ci][:, 2, b], d2, d1, op=ALU.subtract)
                    nc.vector.tensor_tensor(U[ci][:, 3, b], d1, d3, op=ALU.subtract)
                    # p=1 window rows: d0'=d1, d1'=d2, d2'=d3:
                    # v0 = d0'-d1', v1 = d1' (XP view), v2 = d2'-d1'
                    nc.vector.tensor_tensor(V[ci][:, 0, b], d1, d2, op=ALU.subtract)
                    nc.vector.tensor_tensor(V[ci][:, 1, b], d3, d2, op=ALU.subtract)

            def v_rhs(ci, s, wsl):
                if s == 0:
                    return V[ci][:, 0, :, :, wsl]
                if s == 1:
                    return rowpair(XP[ci], 2, 0)[:, :, :, wsl]
                return V[ci][:, 1, :, :, wsl]

            def u_rhs(ci, s, wsl):
                return U[ci][:, s, :, :, wsl]

            def op1_dst(co, p, q, j):
                """conv1 drain dest: OP1 interior phase view [c, b, t, 16]."""
                par = 1 - q          # q=0 -> odd cols (1..31), q=1 -> even (2..32)
                wsl = slice(0, 16) if q == 0 else slice(1, 17)
                return OP1[co][:, :, 1:33].rearrange(
                    "c b (t four) par w -> c b t four par w", four=4)[
                        :, :, :, 2 * j + p, par, wsl]

            def csc_dst(co, p, q, j):
                iph = 2 * p + q
                return CSC[co][:, iph].rearrange(
                    "c b (t two) w -> c b t two w", two=2)[:, :, :, j, :]

            # ---- startup DMAs / init ----
            emit_xp(0)
            emit_xp(1)
            nc.vector.memset(epsc[:], EPS)
            for co in range(NCO):
                nc.gpsimd.memset(OP1[co][:].rearrange("c b h p w -> c b (h p w)"), 0.0)
            emit_uv(0)
            emit_uv(1)

            # ---- conv5: phase / h-position / ci loops ----
            # Per s-position: 4 PSUM chains (one per co), accumulated over
            # (ci, aw). Drains fold the h-inverse incrementally, in place in
            # the bf16 destination:
            #   p=0: y0 = m0+m1+m2 ; y1 = m1-m2-m3
            #   p=1: y0 = m0+m1    ; y1 = m1+m2
            def conv5(wg_d, dst_fn, sums, sqs, wtag, prefetch=False):
                gofs = 0
                for iph, (p, q) in enumerate(PHASES):
                    aws = _aws(q)
                    nblk = len(aws)
                    rhs_fn = u_rhs if p == 0 else v_rhs
                    for s in range(_ht(p)):
                        pps = [ps.tile([128, 256], f32, name=f"{wtag}ps{iph}{s}_{co}",
                                       tag="psb") for co in range(NCO)]
                        for ci in range(NCI):
                            if prefetch and iph == 0 and s == 0 and ci + 2 < NCI:
                                emit_xp(ci + 2)
                                emit_uv(ci + 2)
                            wt = wts.tile([128, 3, 512], bf, name=f"{wtag}w", tag="w5")
                            nc.sync.dma_start(
                                wt[:, :nblk, :],
                                wg_d[gofs:gofs + nblk].rearrange("l c m -> c l m"))
                            gofs += nblk
                            for ai, aw in enumerate(aws):
                                rhs = rhs_fn(ci, s, slice(1 + aw, 17 + aw))
                                first = ci == 0 and ai == 0
                                last = ci == NCI - 1 and ai == nblk - 1
                                for co in range(NCO):
                                    nc.tensor.matmul(
                                        pps[co][:], wt[:, ai, co * 128:(co + 1) * 128],
                                        rhs, start=first, stop=last)
                        # ---- drain position s into y0/y1 (in-place partials) --
                        for co in range(NCO):
                            c0 = ((co * 4 + iph) * 2) * 2
                            for b in range(B_LOC):
                                m = pps[co][:, b * 128:(b + 1) * 128].rearrange(
                                    "c (t w) -> c t w", t=8)
                                y0 = dst_fn(co, p, q, 0)[:, b]
                                y1 = dst_fn(co, p, q, 1)[:, b]
                                s0 = sums[:, c0 + b:c0 + b + 1]
                                s1 = sums[:, c0 + 2 + b:c0 + 3 + b]
                                sg = 1.0
                                y0_ops, y1_ops = [], []
                                if s == 0:
                                    y0_ops = ["init"]
                                elif s == 1:
                                    y0_ops = ["acc"]
                                    y1_ops = ["init"]
                                elif s == 2:
                                    if p == 0:
                                        y0_ops = ["acc_fin"]
                                        y1_ops = ["neg"]
                                    else:
                                        y1_ops = ["acc_fin"]
                                else:
                                    y1_ops = ["neg_fin"]
                                if p == 1 and s == 1:
                                    y0_ops = ["acc_fin"]
                                for tgt, ops_, scol in ((y0, y0_ops, s0),
                                                        (y1, y1_ops, s1)):
                                    for opk in ops_:
                                        if opk == "init":
                                            nc.vector.tensor_scalar(
                                                tgt, m, 0.0, 0.0,
                                                op0=ALU.add, op1=ALU.add)
                                        else:
                                            fin = opk.endswith("fin")
                                            sgn = -1.0 if opk.startswith("neg") else 1.0
                                            nc.vector.scalar_tensor_tensor(
                                                tgt, m, sgn, tgt,
                                                op0=ALU.mult, op1=ALU.add,
                                                accum_out=(scol if fin else None))
                                if s == _ht(p) - 1:
                                    nc.scalar.activation(
                                        m128b(sqscr, 0, b), y0, AFT.Square,
                                        accum_out=sqs[:, c0 + b:c0 + b + 1])
                                    nc.scalar.activation(
                                        m128b(sqscr, 1, b), y1, AFT.Square,
                                        accum_out=sqs[:, c0 + 2 + b:c0 + 3 + b])

            # ================= conv1 =================
            conv5(w1g_d, op1_dst, sums1, sq1, "c1", prefetch=True)
            nc.sync.dma_start(gbv[:], gb_d)

            # ---- c1 stats -> AllReduce #1 (overlaps convsc) ----
            nc.vector.tensor_reduce(
                pack1[:, 0, :], sums1[:].rearrange("c (co x) -> c co x", x=16),
                axis=mybir.AxisListType.X, op=ALU.add)
            nc.vector.tensor_reduce(
                pack1[:, 1, :], sq1[:].rearrange("c (co x) -> c co x", x=16),
                axis=mybir.AxisListType.X, op=ALU.add)
            nc.sync.dma_start(ar1_in[:].rearrange("s (co c) -> c s co", c=128), pack1[:])
            nc.gpsimd.collective_compute(
                "AllReduce", ALU.add,
                replica_groups=[list(range(NCORES))],
                ins=[ar1_in.opt()], outs=[ar1_out.opt()])
            nc.sync.dma_start(st1[:], ar1_out[:].rearrange("s (co c) -> c s co", c=128))

            # ================= convsc =================
            conv5(wscg_d, csc_dst, sumssc, sqsc, "sc")

            # ---- BN1 scale/shift ----
            nc.vector.tensor_scalar_mul(st1[:], st1[:], 1.0 / CNT)
            m1 = st1[:, 0, :]
            nc.vector.tensor_tensor(tmpa[:], m1, m1, op=ALU.mult)
            nc.vector.tensor_tensor(tmpb[:], st1[:, 1, :], tmpa[:], op=ALU.subtract)
            nc.scalar.activation(tmpb[:], tmpb[:], AFT.Sqrt, bias=epsc[:])
            nc.vector.reciprocal(tmpa[:], tmpb[:])
            nc.vector.tensor_tensor(scale1[:], gbv[:, 0, :], tmpa[:], op=ALU.mult)
            nc.vector.tensor_tensor(tmpa[:], m1, scale1[:], op=ALU.mult)
            nc.vector.tensor_tensor(shift1[:], gbv[:, 1, :], tmpa[:], op=ALU.subtract)

            # ---- BN1 apply (in-place relu) + conv2 input transform ----
            UH = scr_pool.tile([128, 4, 2, 16, 34], bf, name="uh", tag="uh")
            for co in range(NCO):
                for par, wsl in ((0, slice(1, 17)), (1, slice(0, 16))):
                    for b in range(B_LOC):
                        v = OP1[co][:, b, 1:33, par, wsl]
                        nc.scalar.activation(v, v, AFT.Relu,
                                             bias=shift1[:, co:co + 1],
                                             scale=scale1[:, co:co + 1])
                # H-stage: window rows 2t..2t+3 of padded OP1 (34 rows)
                flat = OP1[co][:].rearrange("c b h par w -> c b h (par w)")

                def rows(lo, parity, b):
                    return flat[:, b, lo:lo + 32].rearrange(
                        "c (t two) w -> c t two w", two=2)[:, :, parity, :]

                for b in range(B_LOC):
                    d0 = rows(0, 0, b)
                    d1 = rows(1, 0, b)
                    d2 = rows(2, 0, b)
                    d3 = rows(2, 1, b)
                    nc.vector.tensor_tensor(UH[:, 0, b], d0, d2, op=ALU.subtract)
                    nc.vector.tensor_tensor(UH[:, 1, b], d1, d2, op=ALU.add)
                    nc.vector.tensor_tensor(UH[:, 2, b], d2, d1, op=ALU.subtract)
                    nc.vector.tensor_tensor(UH[:, 3, b], d1, d3, op=ALU.subtract)
                # W-stage: (par w) cols: even block wp0..16, odd block 17..33.
                # tile wt: d0 = E[wt], d1 = O[wt], d2 = E[wt+1], d3 = O[wt+1]
                for s in range(4):
                    E0 = UH[:, s, :, :, 0:16]
                    E1 = UH[:, s, :, :, 1:17]
                    O0 = UH[:, s, :, :, 17:33]
                    O1 = UH[:, s, :, :, 18:34]
                    nc.vector.tensor_tensor(U2[co][:, 4 * s + 0], E0, E1, op=ALU.subtract)
                    nc.vector.tensor_tensor(U2[co][:, 4 * s + 1], O0, E1, op=ALU.add)
                    nc.vector.tensor_tensor(U2[co][:, 4 * s + 2], E1, O0, op=ALU.subtract)
                    nc.vector.tensor_tensor(U2[co][:, 4 * s + 3], O0, O1, op=ALU.subtract)

            # ---- csc stats into pack2 rows 2,3 ----
            nc.vector.tensor_reduce(
                pack2[:, 2, :], sumssc[:].rearrange("c (co x) -> c co x", x=16),
                axis=mybir.AxisListType.X, op=ALU.add)
            nc.vector.tensor_reduce(
                pack2[:, 3, :], sqsc[:].rearrange("c (co x) -> c co x", x=16),
                axis=mybir.AxisListType.X, op=ALU.add)

            # ================= conv2 (2D Winograd F(2x2,3x3)) =================
            # co outer; per (co, s): 4 sw-chains accumulate over ci2.
            # w-inverse per (co, s) -> T[s, a]; h-inverse per co -> C2 + stats.
            for co in range(NCO):
                T = scr_pool.tile([128, 4, 2, 512], bf, name="T", tag="T", bufs=1)
                hy0 = scr_pool.tile([128, 2, 512], bf, name="hy0", tag="hy0")
                hy1 = scr_pool.tile([128, 2, 512], bf, name="hy1", tag="hy1")
                for s in range(4):
                    wt2 = w2p.tile([128, 16, 128], bf, name="c2w", tag="w2")
                    nc.sync.dma_start(
                        wt2[:].rearrange("c l m -> c (l m)"),
                        w2g_d[co * 4 + s])
                    pps = [ps.tile([128, 512], f32, name=f"c2ps{co}{s}_{sw}",
                                   tag="psb") for sw in range(4)]
                    for ci in range(NCO):
                        for sw in range(4):
                            nc.tensor.matmul(
                                pps[sw][:], wt2[:, ci * 4 + sw, :],
                                U2[ci][:, 4 * s + sw].rearrange("c b t w -> c (b t w)"),
                                start=(ci == 0), stop=(ci == NCO - 1))
                    # w-inverse: T0 = m0+m1+m2 ; T1 = m1-m2-m3 (1 psum read/op)
                    a2 = scr_pool.tile([128, 512], f32, name="a2", tag="a2")
                    q0 = scr_pool.tile([128, 512], f32, name="q0", tag="q0")
                    nc.vector.tensor_scalar(a2[:], pps[1][:], 0.0, 0.0,
                                            op0=ALU.add, op1=ALU.add)
                    nc.vector.scalar_tensor_tensor(q0[:], pps[0][:], 1.0, a2[:],
                                                   op0=ALU.mult, op1=ALU.add)
                    nc.vector.scalar_tensor_tensor(T[:, s, 0], pps[2][:], 1.0,
                                                   q0[:], op0=ALU.mult, op1=ALU.add)
                    nc.vector.scalar_tensor_tensor(q0[:], pps[2][:], -1.0, a2[:],
                                                   op0=ALU.mult, op1=ALU.add)
                    nc.vector.scalar_tensor_tensor(T[:, s, 1], pps[3][:], -1.0,
                                                   q0[:], op0=ALU.mult, op1=ALU.add)
                    # incremental h-inverse: y(jh,a) -> C2 phase (2*jh + a)
                    if s == 1:
                        for a in range(2):
                            nc.vector.tensor_tensor(hy0[:, a], T[:, 0, a],
                                                    T[:, 1, a], op=ALU.add)
                    elif s == 2:
                        for a in range(2):
                            ph = a
                            dst = C2[co][:, ph].rearrange("c b t w -> c (b t w)")
                            nc.vector.scalar_tensor_tensor(
                                dst, T[:, 2, a], 1.0, hy0[:, a],
                                op0=ALU.mult, op1=ALU.add,
                                accum_out=sums2[:, co * 4 + ph:co * 4 + ph + 1])
                            nc.scalar.activation(
                                sqscr[:], dst, AFT.Square,
                                accum_out=sq2[:, co * 4 + ph:co * 4 + ph + 1])
                            nc.vector.tensor_tensor(hy1[:, a], T[:, 1, a],
                                                    T[:, 2, a], op=ALU.subtract)
                    elif s == 3:
                        for a in range(2):
                            ph = 2 + a
                            dst = C2[co][:, ph].rearrange("c b t w -> c (b t w)")
                            nc.vector.scalar_tensor_tensor(
                                dst, T[:, 3, a], -1.0, hy1[:, a],
                                op0=ALU.mult, op1=ALU.add,
                                accum_out=sums2[:, co * 4 + ph:co * 4 + ph + 1])
                            nc.scalar.activation(
                                sqscr[:], dst, AFT.Square,
                                accum_out=sq2[:, co * 4 + ph:co * 4 + ph + 1])

            # ---- stats of c2 -> AllReduce #2 ----
            nc.vector.tensor_reduce(
                pack2[:, 0, :], sums2[:].rearrange("c (co x) -> c co x", x=4),
                axis=mybir.AxisListType.X, op=ALU.add)
            nc.vector.tensor_reduce(
                pack2[:, 1, :], sq2[:].rearrange("c (co x) -> c co x", x=4),
                axis=mybir.AxisListType.X, op=ALU.add)
            nc.sync.dma_start(ar2_in[:].rearrange("s (co c) -> c s co", c=128), pack2[:])
            nc.gpsimd.collective_compute(
                "AllReduce", ALU.add,
                replica_groups=[list(range(NCORES))],
                ins=[ar2_in.opt()], outs=[ar2_out.opt()])
            nc.sync.dma_start(
                st2[:].rearrange("c g s co -> c (g s) co"),
                ar2_out[:].rearrange("s (co c) -> c s co", c=128))

            # ---- BN2 / BNsc scale+shift ----
            # final = relu(s2*(c2 + (ssc/s2)*csc) + (t2 + tsc))
            nc.vector.tensor_scalar_mul(st2[:], st2[:], 1.0 / CNT)
            means = st2[:, :, 0, :]
            e2s = st2[:, :, 1, :]
            gpair = gbv[:, 2:, :].rearrange("c (g s) co -> c g s co", s=2)
            nc.vector.tensor_tensor(m2b[:], means, means, op=ALU.mult)
            nc.vector.tensor_tensor(varb[:], e2s, m2b[:], op=ALU.subtract)
            nc.scalar.activation(varb[:], varb[:], AFT.Sqrt, bias=epsc[:])
            nc.vector.reciprocal(invb[:], varb[:])
            nc.vector.tensor_tensor(scaleb[:], gpair[:, :, 0, :], invb[:], op=ALU.mult)
            nc.vector.tensor_tensor(m2b[:], means, scaleb[:], op=ALU.mult)
            nc.vector.tensor_tensor(shiftb2[:], gpair[:, :, 1, :], m2b[:], op=ALU.subtract)
            nc.vector.tensor_tensor(shiftB[:], shiftb2[:, 0, :], shiftb2[:, 1, :], op=ALU.add)
            nc.vector.reciprocal(tmpa[:], scaleb[:, 0, :])
            nc.vector.tensor_tensor(rmix[:], scaleb[:, 1, :], tmpa[:], op=ALU.mult)

            # ---- final fuse: c2 += rmix*csc ; out = relu(scale2*c2 + shiftB) ----
            for co in range(NCO):
                for iph in range(4):
                    nc.vector.scalar_tensor_tensor(
                        C2[co][:, iph].rearrange("c b t w -> c (b t w)"),
                        CSC[co][:, iph].rearrange("c b t w -> c (b t w)"),
                        rmix[:, co:co + 1],
                        C2[co][:, iph].rearrange("c b t w -> c (b t w)"),
                        op0=ALU.mult, op1=ALU.add)
                for b in range(B_LOC):
                    fin = scr_pool.tile([128, 1024], f32, name="fin", tag="fin",
                                        bufs=2)
                    for iph, (p, q) in enumerate(PHASES):
                        dst = fin.rearrange(
                            "c (th p2 tw q2) -> c th p2 tw q2",
                            th=16, p2=2, q2=2)[:, :, p, :, q]
                        nc.scalar.activation(dst, C2[co][:, iph, b], AFT.Relu,
                                             bias=shiftB[:, co:co + 1],
                                             scale=scaleb[:, 0, co:co + 1])
                    nc.sync.dma_start(
                        out_d[b, co * 128:(co + 1) * 128].rearrange("c h w -> c (h w)"),
                        fin[:])

            for _f in _frees:
                _f()

    nc.compile()
    return nc


def _get_nc():
    if "nc" not in _CACHE:
        _CACHE["nc"] = _build_nc()
    return _CACHE["nc"]


def _prep_inputs(x, w1, w2, wsc, g1, b1, g2, b2, gsc, bsc):
    xpad = np.zeros((B, CIN, 18, 18), dtype=np.float32)
    xpad[:, :, 1:17, 1:17] = x
    xpad = xpad.astype(BF16)
    w1g = _w5_blocks(w1).astype(BF16)
    wscg = _w5_blocks(wsc).astype(BF16)
    w2g = _w2_blocks(w2).astype(BF16)
    gb = np.stack([g1, b1, g2, b2, gsc, bsc]).astype(np.float32)   # [6, 512]
    gbt = np.ascontiguousarray(gb.reshape(6, 4, 128).transpose(2, 0, 1))  # [128, 6, 4]
    return xpad, w1g, wscg, w2g, gbt


def kernel(x, w1, g1, b1, w2, g2, b2, wsc, gsc, bsc, _trace=False, **_kw):
    from concourse.bass_utils import run_bass_kernel_spmd

    x = np.asarray(x, dtype=np.float32)
    xpad, w1g, wscg, w2g, gbt = _prep_inputs(
        np.asarray(x), np.asarray(w1), np.asarray(w2), np.asarray(wsc),
        np.asarray(g1), np.asarray(b1), np.asarray(g2), np.asarray(b2),
        np.asarray(gsc), np.asarray(bsc))

    nc = _get_nc()
    in_maps = []
    for core in range(NCORES):
        in_maps.append({
            "xpad": xpad[core * B_LOC:(core + 1) * B_LOC],
            "w1g": w1g, "wscg": wscg, "w2g": w2g, "gb": gbt,
        })
    res = run_bass_kernel_spmd(nc, in_maps, list(range(NCORES)), trace=_trace)
    out = np.concatenate([res.results[i]["out"] for i in range(NCORES)], axis=0)
    if _trace:
        _CACHE["last_result"] = res
    return out


# revision 29
# speedup vs baseline: 1.3655x; 1.0217x over previous
"""Trainium2 Bass kernel for the Gudi UpProj block — Winograd + bf16.

Reference computation (per image, NCHW):
    xu  = zero_stuff_2x(x)                    # [B,1024,32,32]
    c1  = conv5x5(xu, w1, pad=2);  out1 = relu(BN(c1))
    c2  = conv3x3(out1, w2, pad=1)
    csc = conv5x5(xu, wsc, pad=2)
    out = relu(BN(c2) + BN(csc))              # BN: batch stats over (N,H,W)

Strategy:
  * Data-parallel over batch: 16 images -> 2 per core (8 cores).
  * Zero-stuffing: the 5x5 conv decomposes into 4 parity phases with
    3x3 / 3x2 / 2x3 / 2x2 kernels on the 16x16 grid.
  * 1D Winograd along h on each phase: F(2,3) for 3-tap rows, F(2,2) for
    2-tap rows; w taps stay direct -> 25 -> 17.5 effective taps/quad.
  * conv2 (3x3) via full 2D Winograd F(2x2,3x3): 9 -> 4 eff taps.
  * All matmuls in bf16 (full PE rate at any N; PSUM accumulates f32).
  * BN batch stats cross-core via two small AllReduces (c1; c2+csc).
"""

import numpy as np
import ml_dtypes

NCORES = 8
B = 16
B_LOC = B // NCORES
CIN, COUT = 1024, 512
NCI, NCO = CIN // 128, COUT // 128   # 8, 4
EPS = 1e-5
CNT = float(B * 32 * 32)
PHASES = [(0, 0), (0, 1), (1, 0), (1, 1)]
BF16 = ml_dtypes.bfloat16

# F(2,3): 3-tap kernel, 2 outputs, 4 positions; F(2,2): 2-tap, 2 out, 3 pos.
G3 = np.array([[1, 0, 0], [.5, .5, .5], [.5, -.5, .5], [0, 0, 1]], np.float64)
G2 = np.array([[1, 0], [1, 1], [0, 1]], np.float64)
# Output transforms hardcoded in drain code:
#   F(2,3): y0 = m0+m1+m2 ; y1 = m1-m2-m3
#   F(2,2): y0 = m0+m1    ; y1 = m1+m2

_CACHE = {}


def _aws(q):
    return (-1, 0, 1) if q == 0 else (0, 1)


def _ht(p):
    return 4 if p == 0 else 3


def _w5_order():
    """(phase, s, ci, aw) emission order for conv5 weight blocks."""
    order = []
    for (p, q) in PHASES:
        for s in range(_ht(p)):
            for ci in range(NCI):
                for aw in _aws(q):
                    order.append((p, q, ci, s, aw))
    return order


N_W5 = len(_w5_order())  # 280


def _w5_blocks(w):
    """w [Cout, Cin, 5, 5] f32 -> [280, 128, COUT] transformed blocks (f32)."""
    w = np.asarray(w, np.float64)
    out = np.empty((N_W5, 128, COUT), np.float32)
    for g, (p, q, ci, s, aw) in enumerate(_w5_order()):
        Gm = G3 if p == 0 else G2
        nk = 3 - p
        acc = np.zeros((COUT, 128), np.float64)
        for k in range(nk):
            kh = 2 * k + p
            kw = 2 * (aw + 1) if q == 0 else 2 * aw + 1
            acc += Gm[s, k] * w[:, ci * 128:(ci + 1) * 128, kh, kw]
        out[g] = acc.T
    return out


def _w2_blocks(w2):
    """w2 [Cout, Cin, 3, 3] -> [16, 128, 16*128]: per (co, s): [c, (ci sw), m].

    DMA'd per (co, s) with per-partition-contiguous 4KB lines.
    """
    W2p = np.einsum('sr,ocrv,wv->swoc', G3, np.asarray(w2, np.float64), G3)
    out = np.empty((16, 128, 16, 128), np.float32)
    for co in range(NCO):
        for s in range(4):
            for ci in range(4):
                for sw in range(4):
                    blk = W2p[s, sw, co * 128:(co + 1) * 128,
                              ci * 128:(ci + 1) * 128]   # [m, c]
                    out[co * 4 + s, :, ci * 4 + sw, :] = blk.T
    return out.reshape(16, 128, 16 * 128)


def _build_nc():
    import concourse.mybir as mybir
    import concourse.tile as tile
    from concourse import bacc

    f32 = mybir.dt.float32
    bf = mybir.dt.bfloat16
    ALU = mybir.AluOpType
    AFT = mybir.ActivationFunctionType

    nc = bacc.Bacc("TRN2", target_bir_lowering=False, debug=False)

    xpad_d = nc.dram_tensor("xpad", [B_LOC, CIN, 18, 18], bf, kind="ExternalInput").ap()
    w1g_d = nc.dram_tensor("w1g", [N_W5, 128, COUT], bf, kind="ExternalInput").ap()
    wscg_d = nc.dram_tensor("wscg", [N_W5, 128, COUT], bf, kind="ExternalInput").ap()
    w2g_d = nc.dram_tensor("w2g", [16, 128, 16 * 128], bf, kind="ExternalInput").ap()
    gb_d = nc.dram_tensor("gb", [128, 6, 4], f32, kind="ExternalInput").ap()
    out_d = nc.dram_tensor("out", [B_LOC, COUT, 32, 32], f32, kind="ExternalOutput").ap()

    def r256(ap):
        return ap.rearrange("c (b t w) -> c b t w", b=2, t=8, w=16)

    def m128b(t, half, b):
        return t[:, half * 256 + b * 128:half * 256 + (b + 1) * 128].rearrange(
            "c (t w) -> c t w", t=8)

    with tile.TileContext(nc) as tc:
        _frees = []
        ar1_in, _f = tc.tile([2, COUT], f32, space="DRAM", name="ar1_in"); _frees.append(_f)
        ar1_out, _f = tc.tile([2, COUT], f32, space="DRAM", addr_space="Shared", name="ar1_out"); _frees.append(_f)
        ar2_in, _f = tc.tile([4, COUT], f32, space="DRAM", name="ar2_in"); _frees.append(_f)
        ar2_out, _f = tc.tile([4, COUT], f32, space="DRAM", addr_space="Shared", name="ar2_out"); _frees.append(_f)

        with tc.tile_pool(name="xp", bufs=1) as xp_pool, \
             tc.tile_pool(name="uv", bufs=1) as uv_pool, \
             tc.tile_pool(name="acts", bufs=1) as acts, \
             tc.tile_pool(name="op1", bufs=1) as op1_pool, \
             tc.tile_pool(name="u2", bufs=1) as u2_pool, \
             tc.tile_pool(name="wts", bufs=4) as wts, \
             tc.tile_pool(name="w2p", bufs=2) as w2p, \
             tc.tile_pool(name="scr", bufs=1) as scr_pool, \
             tc.tile_pool(name="small", bufs=1) as small, \
             tc.tile_pool(name="ps", bufs=8, space="PSUM") as ps:

            # ---- persistent SBUF tensors ----
            XP = [xp_pool.tile([128, 2, 18, 18], bf, name=f"xp{i}", tag=f"xp{i}")
                  for i in range(NCI)]
            # U: h-Winograd F(2,3) positions for p=0 phases; V: F(2,2) (v0,v2)
            U = [uv_pool.tile([128, 4, 2, 8, 18], bf, name=f"u{i}", tag=f"u{i}")
                 for i in range(NCI)]
            V = [uv_pool.tile([128, 2, 2, 8, 18], bf, name=f"v{i}", tag=f"v{i}")
                 for i in range(NCI)]
            # OP1: BN1(relu) output, padded, parity-split cols:
            # [c, b, h(0..33), par, wp]; par0 = even cols 0..32, par1 = odd 1..33
            OP1 = [op1_pool.tile([128, 2, 34, 2, 17], bf, name=f"op1_{i}", tag=f"op1_{i}")
                   for i in range(NCO)]
            # CSC / C2: phase-major [c, ph(2p+q), b, i, j]
            CSC = [acts.tile([128, 4, 2, 16, 16], bf, name=f"csc{i}", tag=f"csc{i}")
                   for i in range(NCO)]
            C2 = [acts.tile([128, 4, 2, 16, 16], bf, name=f"c2_{i}", tag=f"c2_{i}")
                  for i in range(NCO)]
            # U2: conv2 2D-Winograd input transform, per OP1 tile (=ci2)
            U2 = [u2_pool.tile([128, 16, 2, 16, 16], bf, name=f"u2_{i}", tag=f"u2_{i}")
                  for i in range(NCO)]

            # stats: conv5 sums col = ((co*4 + iph)*2 + j)*2 + b (per-image,
            # since the split drain ops each carry their own accum); sq col =
            # (co*4 + iph)*2 + j; conv2 col = co*4 + ph
            sums1 = small.tile([128, 64], f32, name="sums1")
            sq1 = small.tile([128, 64], f32, name="sq1")
            sumssc = small.tile([128, 64], f32, name="sumssc")
            sqsc = small.tile([128, 64], f32, name="sqsc")
            sums2 = small.tile([128, 16], f32, name="sums2")
            sq2 = small.tile([128, 16], f32, name="sq2")
            pack1 = small.tile([128, 2, 4], f32, name="pack1")
            pack2 = small.tile([128, 4, 4], f32, name="pack2")
            st1 = small.tile([128, 2, 4], f32, name="st1")
            st2 = small.tile([128, 2, 2, 4], f32, name="st2")
            gbv = small.tile([128, 6, 4], f32, name="gbv")
            scale1 = small.tile([128, 4], f32, name="scale1")
            shift1 = small.tile([128, 4], f32, name="shift1")
            m2b = small.tile([128, 2, 4], f32, name="m2b")
            varb = small.tile([128, 2, 4], f32, name="varb")
            invb = small.tile([128, 2, 4], f32, name="invb")
            scaleb = small.tile([128, 2, 4], f32, name="scaleb")
            shiftb2 = small.tile([128, 2, 4], f32, name="shiftb2")
            shiftB = small.tile([128, 4], f32, name="shiftB")
            rmix = small.tile([128, 4], f32, name="rmix")
            tmpa = small.tile([128, 4], f32, name="tmpa")
            tmpb = small.tile([128, 4], f32, name="tmpb")
            epsc = small.tile([128, 1], f32, name="epsc")
            sqscr = small.tile([128, 512], f32, name="sqscr")

            # ---- helpers ----
            def rowpair(xp, lo, parity):
                """rows lo..lo+15 of an 18-row dim, split in pairs, pick one."""
                return xp[:, :, lo:lo + 16].rearrange(
                    "c b (t two) w -> c b t two w", two=2)[:, :, :, parity, :]

            def emit_xp(ci):
                nc.sync.dma_start(
                    XP[ci][:].rearrange("c b h w -> c b (h w)"),
                    xpad_d[:, ci * 128:(ci + 1) * 128].rearrange("b c h w -> c b (h w)"))

            def emit_uv(ci):
                # per-image ops: walrus caps DVE APs at 3 canonical dims
                xp = XP[ci]
                for b in range(B_LOC):
                    d0 = rowpair(xp, 0, 0)[:, b]   # xpad rows 0,2..14
                    d1 = rowpair(xp, 1, 0)[:, b]   # 1,3..15
                    d2 = rowpair(xp, 2, 0)[:, b]   # 2,4..16
                    d3 = rowpair(xp, 2, 1)[:, b]   # 3,5..17
                    nc.vector.tensor_tensor(U[ci][:, 0, b], d0, d2, op=ALU.subtract)
                    nc.vector.tensor_tensor(U[ci][:, 1, b], d1, d2, op=ALU.add)
                    nc.vector.tensor_tensor(U[ci][:, 2, b], d2, d1, op=ALU.subtract)
                    nc.vector.tensor_tensor(U[ci][:, 3, b], d1, d3, op=ALU.subtract)
                    # p=1 window rows: d0'=d1, d1'=d2, d2'=d3:
                    # v0 = d0'-d1', v1 = d1' (XP view), v2 = d2'-d1'
                    nc.vector.tensor_tensor(V[ci][:, 0, b], d1, d2, op=ALU.subtract)
                    nc.vector.tensor_tensor(V[ci][:, 1, b], d3, d2, op=ALU.subtract)

            def v_rhs(ci, s, wsl):
                if s == 0:
                    return V[ci][:, 0, :, :, wsl]
                if s == 1:
                    return rowpair(XP[ci], 2, 0)[:, :, :, wsl]
                return V[ci][:, 1, :, :, wsl]

            def u_rhs(ci, s, wsl):
                return U[ci][:, s, :, :, wsl]

            def op1_dst(co, p, q, j):
                """conv1 drain dest: OP1 interior phase view [c, b, t, 16]."""
                par = 1 - q          # q=0 -> odd cols (1..31), q=1 -> even (2..32)
                wsl = slice(0, 16) if q == 0 else slice(1, 17)
                return OP1[co][:, :, 1:33].rearrange(
                    "c b (t four) par w -> c b t four par w", four=4)[
                        :, :, :, 2 * j + p, par, wsl]

            def csc_dst(co, p, q, j):
                iph = 2 * p + q
                return CSC[co][:, iph].rearrange(
                    "c b (t two) w -> c b t two w", two=2)[:, :, :, j, :]

            # ---- startup DMAs / init ----
            emit_xp(0)
            emit_xp(1)
            nc.vector.memset(epsc[:], EPS)
            for co in range(NCO):
                nc.gpsimd.memset(OP1[co][:].rearrange("c b h p w -> c b (h p w)"), 0.0)
            emit_uv(0)
            emit_uv(1)

            # ---- conv5: phase / h-position / ci loops ----
            # Per s-position: 4 PSUM chains (one per co), accumulated over
            # (ci, aw). Drains fold the h-inverse incrementally, in place in
            # the bf16 destination:
            #   p=0: y0 = m0+m1+m2 ; y1 = m1-m2-m3
            #   p=1: y0 = m0+m1    ; y1 = m1+m2
            def conv5(wg_d, dst_fn, sums, sqs, wtag, prefetch=False):
                gofs = 0
                for iph, (p, q) in enumerate(PHASES):
                    aws = _aws(q)
                    nblk = len(aws)
                    rhs_fn = u_rhs if p == 0 else v_rhs
                    for s in range(_ht(p)):
                        pps = [ps.tile([128, 256], f32, name=f"{wtag}ps{iph}{s}_{co}",
                                       tag="psb") for co in range(NCO)]
                        for ci in range(NCI):
                            if prefetch and iph == 0 and s == 0 and ci + 2 < NCI:
                                emit_xp(ci + 2)
                                emit_uv(ci + 2)
                            wt = wts.tile([128, 3, 512], bf, name=f"{wtag}w", tag="w5")
                            nc.sync.dma_start(
                                wt[:, :nblk, :],
                                wg_d[gofs:gofs + nblk].rearrange("l c m -> c l m"))
                            gofs += nblk
                            for ai, aw in enumerate(aws):
                                rhs = rhs_fn(ci, s, slice(1 + aw, 17 + aw))
                                first = ci == 0 and ai == 0
                                last = ci == NCI - 1 and ai == nblk - 1
                                for co in range(NCO):
                                    nc.tensor.matmul(
                                        pps[co][:], wt[:, ai, co * 128:(co + 1) * 128],
                                        rhs, start=first, stop=last)
                        # ---- drain position s into y0/y1 (in-place partials) --
                        for co in range(NCO):
                            c0 = ((co * 4 + iph) * 2) * 2
                            for b in range(B_LOC):
                                m = pps[co][:, b * 128:(b + 1) * 128].rearrange(
                                    "c (t w) -> c t w", t=8)
                                y0 = dst_fn(co, p, q, 0)[:, b]
                                y1 = dst_fn(co, p, q, 1)[:, b]
                                s0 = sums[:, c0 + b:c0 + b + 1]
                                s1 = sums[:, c0 + 2 + b:c0 + 3 + b]
                                sg = 1.0
                                y0_ops, y1_ops = [], []
                                if s == 0:
                                    y0_ops = ["init"]
                                elif s == 1:
                                    y0_ops = ["acc"]
                                    y1_ops = ["init"]
                                elif s == 2:
                                    if p == 0:
                                        y0_ops = ["acc_fin"]
                                        y1_ops = ["neg"]
                                    else:
                                        y1_ops = ["acc_fin"]
                                else:
                                    y1_ops = ["neg_fin"]
                                if p == 1 and s == 1:
                                    y0_ops = ["acc_fin"]
                                for tgt, ops_, scol in ((y0, y0_ops, s0),
                                                        (y1, y1_ops, s1)):
                                    for opk in ops_:
                                        if opk == "init":
                                            nc.vector.tensor_scalar(
                                                tgt, m, 0.0, 0.0,
                                                op0=ALU.add, op1=ALU.add)
                                        else:
                                            fin = opk.endswith("fin")
                                            sgn = -1.0 if opk.startswith("neg") else 1.0
                                            nc.vector.scalar_tensor_tensor(
                                                tgt, m, sgn, tgt,
                                                op0=ALU.mult, op1=ALU.add,
                                                accum_out=(scol if fin else None))
                                if s == _ht(p) - 1:
                                    nc.scalar.activation(
                                        m128b(sqscr, 0, b), y0, AFT.Square,
                                        accum_out=sqs[:, c0 + b:c0 + b + 1])
                                    nc.scalar.activation(
                                        m128b(sqscr, 1, b), y1, AFT.Square,
                                        accum_out=sqs[:, c0 + 2 + b:c0 + 3 + b])

            # ================= conv1 =================
            conv5(w1g_d, op1_dst, sums1, sq1, "c1", prefetch=True)
            nc.sync.dma_start(gbv[:], gb_d)

            # ---- c1 stats -> AllReduce #1 (overlaps convsc) ----
            nc.vector.tensor_reduce(
                pack1[:, 0, :], sums1[:].rearrange("c (co x) -> c co x", x=16),
                axis=mybir.AxisListType.X, op=ALU.add)
            nc.vector.tensor_reduce(
                pack1[:, 1, :], sq1[:].rearrange("c (co x) -> c co x", x=16),
                axis=mybir.AxisListType.X, op=ALU.add)
            nc.sync.dma_start(ar1_in[:].rearrange("s (co c) -> c s co", c=128), pack1[:])
            nc.gpsimd.collective_compute(
                "AllReduce", ALU.add,
                replica_groups=[list(range(NCORES))],
                ins=[ar1_in.opt()], outs=[ar1_out.opt()])
            nc.sync.dma_start(st1[:], ar1_out[:].rearrange("s (co c) -> c s co", c=128))

            # ================= convsc =================
            conv5(wscg_d, csc_dst, sumssc, sqsc, "sc")

            # ---- BN1 scale/shift ----
            nc.vector.tensor_scalar_mul(st1[:], st1[:], 1.0 / CNT)
            m1 = st1[:, 0, :]
            nc.vector.tensor_tensor(tmpa[:], m1, m1, op=ALU.mult)
            nc.vector.tensor_tensor(tmpb[:], st1[:, 1, :], tmpa[:], op=ALU.subtract)
            nc.scalar.activation(tmpb[:], tmpb[:], AFT.Sqrt, bias=epsc[:])
            nc.vector.reciprocal(tmpa[:], tmpb[:])
            nc.vector.tensor_tensor(scale1[:], gbv[:, 0, :], tmpa[:], op=ALU.mult)
            nc.vector.tensor_tensor(tmpa[:], m1, scale1[:], op=ALU.mult)
            nc.vector.tensor_tensor(shift1[:], gbv[:, 1, :], tmpa[:], op=ALU.subtract)

            # ---- BN1 apply (in-place relu) + conv2 input transform ----
            UH = scr_pool.tile([128, 4, 2, 16, 34], bf, name="uh", tag="uh")
            for co in range(NCO):
                for par, wsl in ((0, slice(1, 17)), (1, slice(0, 16))):
                    for b in range(B_LOC):
                        v = OP1[co][:, b, 1:33, par, wsl]
                        nc.scalar.activation(v, v, AFT.Relu,
                                             bias=shift1[:, co:co + 1],
                                             scale=scale1[:, co:co + 1])
                # H-stage: window rows 2t..2t+3 of padded OP1 (34 rows)
                flat = OP1[co][:].rearrange("c b h par w -> c b h (par w)")

                def rows(lo, parity, b):
                    return flat[:, b, lo:lo + 32].rearrange(
                        "c (t two) w -> c t two w", two=2)[:, :, parity, :]

                for b in range(B_LOC):
                    d0 = rows(0, 0, b)
                    d1 = rows(1, 0, b)
                    d2 = rows(2, 0, b)
                    d3 = rows(2, 1, b)
                    nc.vector.tensor_tensor(UH[:, 0, b], d0, d2, op=ALU.subtract)
                    nc.vector.tensor_tensor(UH[:, 1, b], d1, d2, op=ALU.add)
                    nc.vector.tensor_tensor(UH[:, 2, b], d2, d1, op=ALU.subtract)
                    nc.vector.tensor_tensor(UH[:, 3, b], d1, d3, op=ALU.subtract)
                # W-stage: (par w) cols: even block wp0..16, odd block 17..33.
                # tile wt: d0 = E[wt], d1 = O[wt], d2 = E[wt+1], d3 = O[wt+1]
                for s in range(4):
                    E0 = UH[:, s, :, :, 0:16]
                    E1 = UH[:, s, :, :, 1:17]
                    O0 = UH[:, s, :, :, 17:33]
                    O1 = UH[:, s, :, :, 18:34]
                    nc.vector.tensor_tensor(U2[co][:, 4 * s + 0], E0, E1, op=ALU.subtract)
                    nc.vector.tensor_tensor(U2[co][:, 4 * s + 1], O0, E1, op=ALU.add)
                    nc.vector.tensor_tensor(U2[co][:, 4 * s + 2], E1, O0, op=ALU.subtract)
                    nc.vector.tensor_tensor(U2[co][:, 4 * s + 3], O0, O1, op=ALU.subtract)

            # ---- csc stats into pack2 rows 2,3 ----
            nc.vector.tensor_reduce(
                pack2[:, 2, :], sumssc[:].rearrange("c (co x) -> c co x", x=16),
                axis=mybir.AxisListType.X, op=ALU.add)
            nc.vector.tensor_reduce(
                pack2[:, 3, :], sqsc[:].rearrange("c (co x) -> c co x", x=16),
                axis=mybir.AxisListType.X, op=ALU.add)

            # ================= conv2 (2D Winograd F(2x2,3x3)) =================
            # co outer; per (co, s): 4 sw-chains accumulate over ci2.
            # w-inverse per (co, s) -> T[s, a]; h-inverse per co -> C2 + stats.
            for co in range(NCO):
                T = scr_pool.tile([128, 4, 2, 512], bf, name="T", tag="T", bufs=1)
                hy0 = scr_pool.tile([128, 2, 512], bf, name="hy0", tag="hy0")
                hy1 = scr_pool.tile([128, 2, 512], bf, name="hy1", tag="hy1")
                for s in range(4):
                    wt2 = w2p.tile([128, 16, 128], bf, name="c2w", tag="w2")
                    nc.sync.dma_start(
                        wt2[:].rearrange("c l m -> c (l m)"),
                        w2g_d[co * 4 + s])
                    pps = [ps.tile([128, 512], f32, name=f"c2ps{co}{s}_{sw}",
                                   tag="psb") for sw in range(4)]
                    for ci in range(NCO):
                        for sw in range(4):
                            nc.tensor.matmul(
                                pps[sw][:], wt2[:, ci * 4 + sw, :],
                                U2[ci][:, 4 * s + sw].rearrange("c b t w -> c (b t w)"),
                                start=(ci == 0), stop=(ci == NCO - 1))
                    # w-inverse: T0 = m0+m1+m2 ; T1 = m1-m2-m3 (1 psum read/op)
                    a2 = scr_pool.tile([128, 512], f32, name="a2", tag="a2")
                    q0 = scr_pool.tile([128, 512], f32, name="q0", tag="q0")
                    nc.scalar.copy(a2[:], pps[1][:])
                    nc.vector.scalar_tensor_tensor(q0[:], pps[0][:], 1.0, a2[:],
                                                   op0=ALU.mult, op1=ALU.add)
                    nc.vector.scalar_tensor_tensor(T[:, s, 0], pps[2][:], 1.0,
                                                   q0[:], op0=ALU.mult, op1=ALU.add)
                    nc.vector.scalar_tensor_tensor(q0[:], pps[2][:], -1.0, a2[:],
                                                   op0=ALU.mult, op1=ALU.add)
                    nc.vector.scalar_tensor_tensor(T[:, s, 1], pps[3][:], -1.0,
                                                   q0[:], op0=ALU.mult, op1=ALU.add)
                    # incremental h-inverse: y(jh,a) -> C2 phase (2*jh + a)
                    if s == 1:
                        for a in range(2):
                            nc.vector.tensor_tensor(hy0[:, a], T[:, 0, a],
                                                    T[:, 1, a], op=ALU.add)
                    elif s == 2:
                        for a in range(2):
                            ph = a
                            dst = C2[co][:, ph].rearrange("c b t w -> c (b t w)")
                            nc.vector.scalar_tensor_tensor(
                                dst, T[:, 2, a], 1.0, hy0[:, a],
                                op0=ALU.mult, op1=ALU.add,
                                accum_out=sums2[:, co * 4 + ph:co * 4 + ph + 1])
                            nc.scalar.activation(
                                sqscr[:], dst, AFT.Square,
                                accum_out=sq2[:, co * 4 + ph:co * 4 + ph + 1])
                            nc.vector.tensor_tensor(hy1[:, a], T[:, 1, a],
                                                    T[:, 2, a], op=ALU.subtract)
                    elif s == 3:
                        for a in range(2):
                            ph = 2 + a
                            dst = C2[co][:, ph].rearrange("c b t w -> c (b t w)")
                            nc.vector.scalar_tensor_tensor(
                                dst, T[:, 3, a], -1.0, hy1[:, a],
                                op0=ALU.mult, op1=ALU.add,
                                accum_out=sums2[:, co * 4 + ph:co * 4 + ph + 1])
                            nc.scalar.activation(
                                sqscr[:], dst, AFT.Square,
                                accum_out=sq2[:, co * 4 + ph:co * 4 + ph + 1])

            # ---- stats of c2 -> AllReduce #2 ----
            nc.vector.tensor_reduce(
                pack2[:, 0, :], sums2[:].rearrange("c (co x) -> c co x", x=4),
                axis=mybir.AxisListType.X, op=ALU.add)
            nc.vector.tensor_reduce(
                pack2[:, 1, :], sq2[:].rearrange("c (co x) -> c co x", x=4),
                axis=mybir.AxisListType.X, op=ALU.add)
            nc.sync.dma_start(ar2_in[:].rearrange("s (co c) -> c s co", c=128), pack2[:])
            nc.gpsimd.collective_compute(
                "AllReduce", ALU.add,
                replica_groups=[list(range(NCORES))],
                ins=[ar2_in.opt()], outs=[ar2_out.opt()])
            nc.sync.dma_start(
                st2[:].rearrange("c g s co -> c (g s) co"),
                ar2_out[:].rearrange("s (co c) -> c s co", c=128))

            # ---- BN2 / BNsc scale+shift ----
            # final = relu(s2*(c2 + (ssc/s2)*csc) + (t2 + tsc))
            nc.vector.tensor_scalar_mul(st2[:], st2[:], 1.0 / CNT)
            means = st2[:, :, 0, :]
            e2s = st2[:, :, 1, :]
            gpair = gbv[:, 2:, :].rearrange("c (g s) co -> c g s co", s=2)
            nc.vector.tensor_tensor(m2b[:], means, means, op=ALU.mult)
            nc.vector.tensor_tensor(varb[:], e2s, m2b[:], op=ALU.subtract)
            nc.scalar.activation(varb[:], varb[:], AFT.Sqrt, bias=epsc[:])
            nc.vector.reciprocal(invb[:], varb[:])
            nc.vector.tensor_tensor(scaleb[:], gpair[:, :, 0, :], invb[:], op=ALU.mult)
            nc.vector.tensor_tensor(m2b[:], means, scaleb[:], op=ALU.mult)
            nc.vector.tensor_tensor(shiftb2[:], gpair[:, :, 1, :], m2b[:], op=ALU.subtract)
            nc.vector.tensor_tensor(shiftB[:], shiftb2[:, 0, :], shiftb2[:, 1, :], op=ALU.add)
            nc.vector.reciprocal(tmpa[:], scaleb[:, 0, :])
            nc.vector.tensor_tensor(rmix[:], scaleb[:, 1, :], tmpa[:], op=ALU.mult)

            # ---- final fuse: c2 += rmix*csc ; out = relu(scale2*c2 + shiftB) ----
            for co in range(NCO):
                for iph in range(4):
                    nc.vector.scalar_tensor_tensor(
                        C2[co][:, iph].rearrange("c b t w -> c (b t w)"),
                        CSC[co][:, iph].rearrange("c b t w -> c (b t w)"),
                        rmix[:, co:co + 1],
                        C2[co][:, iph].rearrange("c b t w -> c (b t w)"),
                        op0=ALU.mult, op1=ALU.add)
                for b in range(B_LOC):
                    fin = scr_pool.tile([128, 1024], f32, name="fin", tag="fin",
                                        bufs=2)
                    for p in range(2):
                        # phases (2p, 2p+1) together: src dims (q, th, tw)
                        dst = fin.rearrange(
                            "c (th p2 tw q2) -> c p2 q2 th tw",
                            th=16, p2=2, q2=2)[:, p]
                        nc.scalar.activation(dst, C2[co][:, 2 * p:2 * p + 2, b],
                                             AFT.Relu,
                                             bias=shiftB[:, co:co + 1],
                                             scale=scaleb[:, 0, co:co + 1])
                    nc.sync.dma_start(
                        out_d[b, co * 128:(co + 1) * 128].rearrange("c h w -> c (h w)"),
                        fin[:])

            for _f in _frees:
                _f()

    nc.compile()
    return nc


def _get_nc():
    if "nc" not in _CACHE:
        _CACHE["nc"] = _build_nc()
    return _CACHE["nc"]


def _prep_inputs(x, w1, w2, wsc, g1, b1, g2, b2, gsc, bsc):
    xpad = np.zeros((B, CIN, 18, 18), dtype=np.float32)
    xpad[:, :, 1:17, 1:17] = x
    xpad = xpad.astype(BF16)
    w1g = _w5_blocks(w1).astype(BF16)
    wscg = _w5_blocks(wsc).astype(BF16)
    w2g = _w2_blocks(w2).astype(BF16)
    gb = np.stack([g1, b1, g2, b2, gsc, bsc]).astype(np.float32)   # [6, 512]
    gbt = np.ascontiguousarray(gb.reshape(6, 4, 128).transpose(2, 0, 1))  # [128, 6, 4]
    return xpad, w1g, wscg, w2g, gbt


def kernel(x, w1, g1, b1, w2, g2, b2, wsc, gsc, bsc, _trace=False, **_kw):
    from concourse.bass_utils import run_bass_kernel_spmd

    x = np.asarray(x, dtype=np.float32)
    xpad, w1g, wscg, w2g, gbt = _prep_inputs(
        np.asarray(x), np.asarray(w1), np.asarray(w2), np.asarray(wsc),
        np.asarray(g1), np.asarray(b1), np.asarray(g2), np.asarray(b2),
        np.asarray(gsc), np.asarray(bsc))

    nc = _get_nc()
    in_maps = []
    for core in range(NCORES):
        in_maps.append({
            "xpad": xpad[core * B_LOC:(core + 1) * B_LOC],
            "w1g": w1g, "wscg": wscg, "w2g": w2g, "gb": gbt,
        })
    res = run_bass_kernel_spmd(nc, in_maps, list(range(NCORES)), trace=_trace)
    out = np.concatenate([res.results[i]["out"] for i in range(NCORES)], axis=0)
    if _trace:
        _CACHE["last_result"] = res
    return out


# revision 36
# speedup vs baseline: 1.3759x; 1.0076x over previous
"""Trainium2 Bass kernel for the Gudi UpProj block — Winograd + bf16.

Reference computation (per image, NCHW):
    xu  = zero_stuff_2x(x)                    # [B,1024,32,32]
    c1  = conv5x5(xu, w1, pad=2);  out1 = relu(BN(c1))
    c2  = conv3x3(out1, w2, pad=1)
    csc = conv5x5(xu, wsc, pad=2)
    out = relu(BN(c2) + BN(csc))              # BN: batch stats over (N,H,W)

Strategy:
  * Data-parallel over batch: 16 images -> 2 per core (8 cores).
  * Zero-stuffing: the 5x5 conv decomposes into 4 parity phases with
    3x3 / 3x2 / 2x3 / 2x2 kernels on the 16x16 grid.
  * 1D Winograd along h on each phase: F(2,3) for 3-tap rows, F(2,2) for
    2-tap rows; w taps stay direct -> 25 -> 17.5 effective taps/quad.
  * conv2 (3x3) via full 2D Winograd F(2x2,3x3): 9 -> 4 eff taps.
  * All matmuls in bf16 (full PE rate at any N; PSUM accumulates f32).
  * BN batch stats cross-core via two small AllReduces (c1; c2+csc).
"""

import numpy as np
import ml_dtypes

NCORES = 8
B = 16
B_LOC = B // NCORES
CIN, COUT = 1024, 512
NCI, NCO = CIN // 128, COUT // 128   # 8, 4
EPS = 1e-5
CNT = float(B * 32 * 32)
PHASES = [(0, 0), (0, 1), (1, 0), (1, 1)]
BF16 = ml_dtypes.bfloat16

# F(2,3): 3-tap kernel, 2 outputs, 4 positions; F(2,2): 2-tap, 2 out, 3 pos.
G3 = np.array([[1, 0, 0], [.5, .5, .5], [.5, -.5, .5], [0, 0, 1]], np.float64)
G2 = np.array([[1, 0], [1, 1], [0, 1]], np.float64)
# Output transforms hardcoded in drain code:
#   F(2,3): y0 = m0+m1+m2 ; y1 = m1-m2-m3
#   F(2,2): y0 = m0+m1    ; y1 = m1+m2

_CACHE = {}


def _aws(q):
    return (-1, 0, 1) if q == 0 else (0, 1)


def _ht(p):
    return 4 if p == 0 else 3


def _w5_order():
    """(phase, s, ci, aw) emission order for conv5 weight blocks."""
    order = []
    for (p, q) in PHASES:
        for s in range(_ht(p)):
            for ci in range(NCI):
                for aw in _aws(q):
                    order.append((p, q, ci, s, aw))
    return order


N_W5 = len(_w5_order())  # 280


def _w5_blocks(w):
    """w [Cout, Cin, 5, 5] f32 -> [280, 128, COUT] transformed blocks (f32)."""
    w = np.asarray(w, np.float64)
    out = np.empty((N_W5, 128, COUT), np.float32)
    for g, (p, q, ci, s, aw) in enumerate(_w5_order()):
        Gm = G3 if p == 0 else G2
        nk = 3 - p
        acc = np.zeros((COUT, 128), np.float64)
        for k in range(nk):
            kh = 2 * k + p
            kw = 2 * (aw + 1) if q == 0 else 2 * aw + 1
            acc += Gm[s, k] * w[:, ci * 128:(ci + 1) * 128, kh, kw]
        out[g] = acc.T
    return out


def _w2_blocks(w2):
    """w2 [Cout, Cin, 3, 3] -> [16, 128, 16*128]: per (co, s): [c, (ci sw), m].

    DMA'd per (co, s) with per-partition-contiguous 4KB lines.
    """
    W2p = np.einsum('sr,ocrv,wv->swoc', G3, np.asarray(w2, np.float64), G3)
    out = np.empty((16, 128, 16, 128), np.float32)
    for co in range(NCO):
        for s in range(4):
            for ci in range(4):
                for sw in range(4):
                    blk = W2p[s, sw, co * 128:(co + 1) * 128,
                              ci * 128:(ci + 1) * 128]   # [m, c]
                    out[co * 4 + s, :, ci * 4 + sw, :] = blk.T
    return out.reshape(16, 128, 16 * 128)


def _build_nc():
    import concourse.mybir as mybir
    import concourse.tile as tile
    from concourse import bacc

    f32 = mybir.dt.float32
    bf = mybir.dt.bfloat16
    ALU = mybir.AluOpType
    AFT = mybir.ActivationFunctionType

    nc = bacc.Bacc("TRN2", target_bir_lowering=False, debug=False)

    xpad_d = nc.dram_tensor("xpad", [B_LOC, CIN, 18, 18], bf, kind="ExternalInput").ap()
    w1g_d = nc.dram_tensor("w1g", [N_W5, 128, COUT], bf, kind="ExternalInput").ap()
    wscg_d = nc.dram_tensor("wscg", [N_W5, 128, COUT], bf, kind="ExternalInput").ap()
    w2g_d = nc.dram_tensor("w2g", [16, 128, 16 * 128], bf, kind="ExternalInput").ap()
    gb_d = nc.dram_tensor("gb", [128, 6, 4], f32, kind="ExternalInput").ap()
    out_d = nc.dram_tensor("out", [B_LOC, COUT, 32, 32], f32, kind="ExternalOutput").ap()

    def r256(ap):
        return ap.rearrange("c (b t w) -> c b t w", b=2, t=8, w=16)

    def m128b(t, half, b):
        return t[:, half * 256 + b * 128:half * 256 + (b + 1) * 128].rearrange(
            "c (t w) -> c t w", t=8)

    with tile.TileContext(nc) as tc:
        _frees = []
        ar1_in, _f = tc.tile([2, COUT], f32, space="DRAM", name="ar1_in"); _frees.append(_f)
        ar1_out, _f = tc.tile([2, COUT], f32, space="DRAM", addr_space="Shared", name="ar1_out"); _frees.append(_f)
        ar2_in, _f = tc.tile([4, COUT], f32, space="DRAM", name="ar2_in"); _frees.append(_f)
        ar2_out, _f = tc.tile([4, COUT], f32, space="DRAM", addr_space="Shared", name="ar2_out"); _frees.append(_f)

        with tc.tile_pool(name="xp", bufs=1) as xp_pool, \
             tc.tile_pool(name="uv", bufs=1) as uv_pool, \
             tc.tile_pool(name="acts", bufs=1) as acts, \
             tc.tile_pool(name="op1", bufs=1) as op1_pool, \
             tc.tile_pool(name="u2", bufs=1) as u2_pool, \
             tc.tile_pool(name="wts", bufs=4) as wts, \
             tc.tile_pool(name="w2p", bufs=2) as w2p, \
             tc.tile_pool(name="scr", bufs=1) as scr_pool, \
             tc.tile_pool(name="small", bufs=1) as small, \
             tc.tile_pool(name="ps", bufs=8, space="PSUM") as ps:

            # ---- persistent SBUF tensors ----
            XP = [xp_pool.tile([128, 2, 18, 18], bf, name=f"xp{i}", tag=f"xp{i}")
                  for i in range(NCI)]
            # U: h-Winograd F(2,3) positions for p=0 phases; V: F(2,2) (v0,v2)
            U = [uv_pool.tile([128, 4, 2, 8, 18], bf, name=f"u{i}", tag=f"u{i}")
                 for i in range(NCI)]
            V = [uv_pool.tile([128, 2, 2, 8, 18], bf, name=f"v{i}", tag=f"v{i}")
                 for i in range(NCI)]
            # OP1: BN1(relu) output, padded, parity-split cols:
            # [c, b, h(0..33), par, wp]; par0 = even cols 0..32, par1 = odd 1..33
            OP1 = [op1_pool.tile([128, 2, 34, 2, 17], bf, name=f"op1_{i}", tag=f"op1_{i}")
                   for i in range(NCO)]
            # CSC / C2: phase-major [c, ph(2p+q), b, i, j]
            CSC = [acts.tile([128, 4, 2, 16, 16], bf, name=f"csc{i}", tag=f"csc{i}")
                   for i in range(NCO)]
            C2 = [acts.tile([128, 4, 2, 16, 16], bf, name=f"c2_{i}", tag=f"c2_{i}")
                  for i in range(NCO)]
            # U2: conv2 2D-Winograd input transform, per OP1 tile (=ci2)
            U2 = [u2_pool.tile([128, 16, 2, 16, 16], bf, name=f"u2_{i}", tag=f"u2_{i}")
                  for i in range(NCO)]

            # stats: conv5 sums col = ((co*4 + iph)*2 + j)*2 + b (per-image,
            # since the split drain ops each carry their own accum); sq col =
            # (co*4 + iph)*2 + j; conv2 col = co*4 + ph
            sums1 = small.tile([128, 64], f32, name="sums1")
            sq1 = small.tile([128, 64], f32, name="sq1")
            sumssc = small.tile([128, 64], f32, name="sumssc")
            sqsc = small.tile([128, 64], f32, name="sqsc")
            sums2 = small.tile([128, 16], f32, name="sums2")
            sq2 = small.tile([128, 16], f32, name="sq2")
            pack1 = small.tile([128, 2, 4], f32, name="pack1")
            pack2 = small.tile([128, 4, 4], f32, name="pack2")
            st1 = small.tile([128, 2, 4], f32, name="st1")
            st2 = small.tile([128, 2, 2, 4], f32, name="st2")
            gbv = small.tile([128, 6, 4], f32, name="gbv")
            scale1 = small.tile([128, 4], f32, name="scale1")
            shift1 = small.tile([128, 4], f32, name="shift1")
            m2b = small.tile([128, 2, 4], f32, name="m2b")
            varb = small.tile([128, 2, 4], f32, name="varb")
            invb = small.tile([128, 2, 4], f32, name="invb")
            scaleb = small.tile([128, 2, 4], f32, name="scaleb")
            shiftb2 = small.tile([128, 2, 4], f32, name="shiftb2")
            shiftB = small.tile([128, 4], f32, name="shiftB")
            rmix = small.tile([128, 4], f32, name="rmix")
            tmpa = small.tile([128, 4], f32, name="tmpa")
            tmpb = small.tile([128, 4], f32, name="tmpb")
            epsc = small.tile([128, 1], f32, name="epsc")
            sqscr = small.tile([128, 512], f32, name="sqscr")

            # ---- helpers ----
            def rowpair(xp, lo, parity):
                """rows lo..lo+15 of an 18-row dim, split in pairs, pick one."""
                return xp[:, :, lo:lo + 16].rearrange(
                    "c b (t two) w -> c b t two w", two=2)[:, :, :, parity, :]

            def emit_xp(ci):
                nc.sync.dma_start(
                    XP[ci][:].rearrange("c b h w -> c b (h w)"),
                    xpad_d[:, ci * 128:(ci + 1) * 128].rearrange("b c h w -> c b (h w)"))

            def emit_uv(ci):
                # per-image ops: walrus caps DVE APs at 3 canonical dims
                xp = XP[ci]
                for b in range(B_LOC):
                    d0 = rowpair(xp, 0, 0)[:, b]   # xpad rows 0,2..14
                    d1 = rowpair(xp, 1, 0)[:, b]   # 1,3..15
                    d2 = rowpair(xp, 2, 0)[:, b]   # 2,4..16
                    d3 = rowpair(xp, 2, 1)[:, b]   # 3,5..17
                    nc.vector.tensor_tensor(U[ci][:, 0, b], d0, d2, op=ALU.subtract)
                    nc.vector.tensor_tensor(U[ci][:, 1, b], d1, d2, op=ALU.add)
                    nc.vector.tensor_tensor(U[ci][:, 2, b], d2, d1, op=ALU.subtract)
                    nc.vector.tensor_tensor(U[ci][:, 3, b], d1, d3, op=ALU.subtract)
                    # p=1 window rows: d0'=d1, d1'=d2, d2'=d3:
                    # v0 = d0'-d1', v1 = d1' (XP view), v2 = d2'-d1'
                    nc.vector.tensor_tensor(V[ci][:, 0, b], d1, d2, op=ALU.subtract)
                    nc.vector.tensor_tensor(V[ci][:, 1, b], d3, d2, op=ALU.subtract)

            def v_rhs(ci, s, wsl):
                if s == 0:
                    return V[ci][:, 0, :, :, wsl]
                if s == 1:
                    return rowpair(XP[ci], 2, 0)[:, :, :, wsl]
                return V[ci][:, 1, :, :, wsl]

            def u_rhs(ci, s, wsl):
                return U[ci][:, s, :, :, wsl]

            def op1_dst(co, p, q, j):
                """conv1 drain dest: OP1 interior phase view [c, b, t, 16]."""
                par = 1 - q          # q=0 -> odd cols (1..31), q=1 -> even (2..32)
                wsl = slice(0, 16) if q == 0 else slice(1, 17)
                return OP1[co][:, :, 1:33].rearrange(
                    "c b (t four) par w -> c b t four par w", four=4)[
                        :, :, :, 2 * j + p, par, wsl]

            def csc_dst(co, p, q, j):
                iph = 2 * p + q
                return CSC[co][:, iph].rearrange(
                    "c b (t two) w -> c b t two w", two=2)[:, :, :, j, :]

            # ---- startup DMAs / init (first weight chunk ahead of x) ----
            wt_pre = wts.tile([128, 3, 512], bf, name="c1w0", tag="w5")
            nc.sync.dma_start(wt_pre[:], w1g_d[0:3].rearrange("l c m -> c l m"))
            emit_xp(0)
            emit_xp(1)
            nc.vector.memset(epsc[:], EPS)
            for co in range(NCO):
                nc.gpsimd.memset(OP1[co][:].rearrange("c b h p w -> c b (h p w)"), 0.0)
            emit_uv(0)
            emit_uv(1)

            # ---- conv5: phase / h-position / ci loops ----
            # Per s-position: 4 PSUM chains (one per co), accumulated over
            # (ci, aw). Drains fold the h-inverse incrementally, in place in
            # the bf16 destination:
            #   p=0: y0 = m0+m1+m2 ; y1 = m1-m2-m3
            #   p=1: y0 = m0+m1    ; y1 = m1+m2
            def conv5(wg_d, dst_fn, sums, sqs, wtag, prefetch=False, pre_wt=None):
                gofs = 0
                for iph, (p, q) in enumerate(PHASES):
                    aws = _aws(q)
                    nblk = len(aws)
                    rhs_fn = u_rhs if p == 0 else v_rhs
                    for s in range(_ht(p)):
                        pps = [ps.tile([128, 256], f32, name=f"{wtag}ps{iph}{s}_{co}",
                                       tag="psb") for co in range(NCO)]
                        for ci in range(NCI):
                            if prefetch and iph == 0 and s == 0 and ci + 2 < NCI:
                                emit_xp(ci + 2)
                                emit_uv(ci + 2)
                            if pre_wt is not None and iph == 0 and s == 0 and ci == 0:
                                wt = pre_wt
                            else:
                                wt = wts.tile([128, 3, 512], bf, name=f"{wtag}w",
                                              tag="w5")
                                nc.sync.dma_start(
                                    wt[:, :nblk, :],
                                    wg_d[gofs:gofs + nblk].rearrange("l c m -> c l m"))
                            gofs += nblk
                            for ai, aw in enumerate(aws):
                                rhs = rhs_fn(ci, s, slice(1 + aw, 17 + aw))
                                first = ci == 0 and ai == 0
                                last = ci == NCI - 1 and ai == nblk - 1
                                for co in range(NCO):
                                    nc.tensor.matmul(
                                        pps[co][:], wt[:, ai, co * 128:(co + 1) * 128],
                                        rhs, start=first, stop=last)
                        # ---- drain position s into y0/y1 (in-place partials) --
                        for co in range(NCO):
                            c0 = ((co * 4 + iph) * 2) * 2
                            for b in range(B_LOC):
                                m = pps[co][:, b * 128:(b + 1) * 128].rearrange(
                                    "c (t w) -> c t w", t=8)
                                y0 = dst_fn(co, p, q, 0)[:, b]
                                y1 = dst_fn(co, p, q, 1)[:, b]
                                s0 = sums[:, c0 + b:c0 + b + 1]
                                s1 = sums[:, c0 + 2 + b:c0 + 3 + b]
                                sg = 1.0
                                y0_ops, y1_ops = [], []
                                if s == 0:
                                    y0_ops = ["init"]
                                elif s == 1:
                                    y0_ops = ["acc"]
                                    y1_ops = ["init"]
                                elif s == 2:
                                    if p == 0:
                                        y0_ops = ["acc_fin"]
                                        y1_ops = ["neg"]
                                    else:
                                        y1_ops = ["acc_fin"]
                                else:
                                    y1_ops = ["neg_fin"]
                                if p == 1 and s == 1:
                                    y0_ops = ["acc_fin"]
                                for tgt, ops_, scol in ((y0, y0_ops, s0),
                                                        (y1, y1_ops, s1)):
                                    for opk in ops_:
                                        if opk == "init":
                                            nc.vector.tensor_scalar(
                                                tgt, m, 0.0, 0.0,
                                                op0=ALU.add, op1=ALU.add)
                                        else:
                                            fin = opk.endswith("fin")
                                            sgn = -1.0 if opk.startswith("neg") else 1.0
                                            nc.vector.scalar_tensor_tensor(
                                                tgt, m, sgn, tgt,
                                                op0=ALU.mult, op1=ALU.add,
                                                accum_out=(scol if fin else None))
                                if s == _ht(p) - 1:
                                    nc.scalar.activation(
                                        m128b(sqscr, 0, b), y0, AFT.Square,
                                        accum_out=sqs[:, c0 + b:c0 + b + 1])
                                    nc.scalar.activation(
                                        m128b(sqscr, 1, b), y1, AFT.Square,
                                        accum_out=sqs[:, c0 + 2 + b:c0 + 3 + b])

            # ================= conv1 =================
            conv5(w1g_d, op1_dst, sums1, sq1, "c1", prefetch=True, pre_wt=wt_pre)
            nc.sync.dma_start(gbv[:], gb_d)

            # ---- c1 stats -> AllReduce #1 (overlaps convsc) ----
            nc.vector.tensor_reduce(
                pack1[:, 0, :], sums1[:].rearrange("c (co x) -> c co x", x=16),
                axis=mybir.AxisListType.X, op=ALU.add)
            nc.vector.tensor_reduce(
                pack1[:, 1, :], sq1[:].rearrange("c (co x) -> c co x", x=16),
                axis=mybir.AxisListType.X, op=ALU.add)
            nc.sync.dma_start(ar1_in[:].rearrange("s (co c) -> c s co", c=128), pack1[:])
            nc.gpsimd.collective_compute(
                "AllReduce", ALU.add,
                replica_groups=[list(range(NCORES))],
                ins=[ar1_in.opt()], outs=[ar1_out.opt()])
            nc.sync.dma_start(st1[:], ar1_out[:].rearrange("s (co c) -> c s co", c=128))

            # ================= convsc =================
            conv5(wscg_d, csc_dst, sumssc, sqsc, "sc")

            # ---- BN1 scale/shift ----
            nc.vector.tensor_scalar_mul(st1[:], st1[:], 1.0 / CNT)
            m1 = st1[:, 0, :]
            nc.vector.tensor_tensor(tmpa[:], m1, m1, op=ALU.mult)
            nc.vector.tensor_tensor(tmpb[:], st1[:, 1, :], tmpa[:], op=ALU.subtract)
            nc.scalar.activation(tmpb[:], tmpb[:], AFT.Sqrt, bias=epsc[:])
            nc.vector.reciprocal(tmpa[:], tmpb[:])
            nc.vector.tensor_tensor(scale1[:], gbv[:, 0, :], tmpa[:], op=ALU.mult)
            nc.vector.tensor_tensor(tmpa[:], m1, scale1[:], op=ALU.mult)
            nc.vector.tensor_tensor(shift1[:], gbv[:, 1, :], tmpa[:], op=ALU.subtract)

            # ---- BN1 apply (in-place relu) + conv2 input transform ----
            UH = scr_pool.tile([128, 4, 2, 16, 34], bf, name="uh", tag="uh")
            for co in range(NCO):
                for par, wsl in ((0, slice(1, 17)), (1, slice(0, 16))):
                    for b in range(B_LOC):
                        v = OP1[co][:, b, 1:33, par, wsl]
                        nc.scalar.activation(v, v, AFT.Relu,
                                             bias=shift1[:, co:co + 1],
                                             scale=scale1[:, co:co + 1])
                # H-stage: window rows 2t..2t+3 of padded OP1 (34 rows)
                flat = OP1[co][:].rearrange("c b h par w -> c b h (par w)")

                def rows(lo, parity, b):
                    return flat[:, b, lo:lo + 32].rearrange(
                        "c (t two) w -> c t two w", two=2)[:, :, parity, :]

                for b in range(B_LOC):
                    d0 = rows(0, 0, b)
                    d1 = rows(1, 0, b)
                    d2 = rows(2, 0, b)
                    d3 = rows(2, 1, b)
                    nc.vector.tensor_tensor(UH[:, 0, b], d0, d2, op=ALU.subtract)
                    nc.vector.tensor_tensor(UH[:, 1, b], d1, d2, op=ALU.add)
                    nc.vector.tensor_tensor(UH[:, 2, b], d2, d1, op=ALU.subtract)
                    nc.vector.tensor_tensor(UH[:, 3, b], d1, d3, op=ALU.subtract)
                # W-stage: (par w) cols: even block wp0..16, odd block 17..33.
                # tile wt: d0 = E[wt], d1 = O[wt], d2 = E[wt+1], d3 = O[wt+1]
                for s in range(4):
                    E0 = UH[:, s, :, :, 0:16]
                    E1 = UH[:, s, :, :, 1:17]
                    O0 = UH[:, s, :, :, 17:33]
                    O1 = UH[:, s, :, :, 18:34]
                    nc.vector.tensor_tensor(U2[co][:, 4 * s + 0], E0, E1, op=ALU.subtract)
                    nc.vector.tensor_tensor(U2[co][:, 4 * s + 1], O0, E1, op=ALU.add)
                    nc.vector.tensor_tensor(U2[co][:, 4 * s + 2], E1, O0, op=ALU.subtract)
                    nc.vector.tensor_tensor(U2[co][:, 4 * s + 3], O0, O1, op=ALU.subtract)

            # ---- csc stats into pack2 rows 2,3 ----
            nc.vector.tensor_reduce(
                pack2[:, 2, :], sumssc[:].rearrange("c (co x) -> c co x", x=16),
                axis=mybir.AxisListType.X, op=ALU.add)
            nc.vector.tensor_reduce(
                pack2[:, 3, :], sqsc[:].rearrange("c (co x) -> c co x", x=16),
                axis=mybir.AxisListType.X, op=ALU.add)

            # ================= conv2 (2D Winograd F(2x2,3x3)) =================
            # co outer; per (co, s): 4 sw-chains accumulate over ci2.
            # w-inverse per (co, s) -> T[s, a]; h-inverse per co -> C2 + stats.
            for co in range(NCO):
                T = scr_pool.tile([128, 4, 2, 512], bf, name="T", tag="T", bufs=1)
                hy0 = scr_pool.tile([128, 2, 512], bf, name="hy0", tag="hy0")
                hy1 = scr_pool.tile([128, 2, 512], bf, name="hy1", tag="hy1")
                for s in range(4):
                    wt2 = w2p.tile([128, 16, 128], bf, name="c2w", tag="w2")
                    nc.sync.dma_start(
                        wt2[:].rearrange("c l m -> c (l m)"),
                        w2g_d[co * 4 + s])
                    pps = [ps.tile([128, 512], f32, name=f"c2ps{co}{s}_{sw}",
                                   tag="psb") for sw in range(4)]
                    for ci in range(NCO):
                        for sw in range(4):
                            nc.tensor.matmul(
                                pps[sw][:], wt2[:, ci * 4 + sw, :],
                                U2[ci][:, 4 * s + sw].rearrange("c b t w -> c (b t w)"),
                                start=(ci == 0), stop=(ci == NCO - 1))
                    # w-inverse: T0 = m0+m1+m2 ; T1 = m1-m2-m3 (1 psum read/op)
                    a2 = scr_pool.tile([128, 512], f32, name="a2", tag="a2")
                    q0 = scr_pool.tile([128, 512], f32, name="q0", tag="q0")
                    nc.scalar.copy(a2[:], pps[1][:])
                    nc.vector.scalar_tensor_tensor(q0[:], pps[0][:], 1.0, a2[:],
                                                   op0=ALU.mult, op1=ALU.add)
                    nc.vector.scalar_tensor_tensor(T[:, s, 0], pps[2][:], 1.0,
                                                   q0[:], op0=ALU.mult, op1=ALU.add)
                    nc.vector.scalar_tensor_tensor(q0[:], pps[2][:], -1.0, a2[:],
                                                   op0=ALU.mult, op1=ALU.add)
                    nc.vector.scalar_tensor_tensor(T[:, s, 1], pps[3][:], -1.0,
                                                   q0[:], op0=ALU.mult, op1=ALU.add)
                    # incremental h-inverse: y(jh,a) -> C2 phase (2*jh + a)
                    if s == 1:
                        for a in range(2):
                            nc.vector.tensor_tensor(hy0[:, a], T[:, 0, a],
                                                    T[:, 1, a], op=ALU.add)
                    elif s == 2:
                        for a in range(2):
                            ph = a
                            dst = C2[co][:, ph].rearrange("c b t w -> c (b t w)")
                            nc.vector.scalar_tensor_tensor(
                                dst, T[:, 2, a], 1.0, hy0[:, a],
                                op0=ALU.mult, op1=ALU.add,
                                accum_out=sums2[:, co * 4 + ph:co * 4 + ph + 1])
                            nc.scalar.activation(
                                sqscr[:], dst, AFT.Square,
                                accum_out=sq2[:, co * 4 + ph:co * 4 + ph + 1])
                            nc.vector.tensor_tensor(hy1[:, a], T[:, 1, a],
                                                    T[:, 2, a], op=ALU.subtract)
                    elif s == 3:
                        for a in range(2):
                            ph = 2 + a
                            dst = C2[co][:, ph].rearrange("c b t w -> c (b t w)")
                            nc.vector.scalar_tensor_tensor(
                                dst, T[:, 3, a], -1.0, hy1[:, a],
                                op0=ALU.mult, op1=ALU.add,
                                accum_out=sums2[:, co * 4 + ph:co * 4 + ph + 1])
                            nc.scalar.activation(
                                sqscr[:], dst, AFT.Square,
                                accum_out=sq2[:, co * 4 + ph:co * 4 + ph + 1])

            # ---- stats of c2 -> AllReduce #2 ----
            nc.vector.tensor_reduce(
                pack2[:, 0, :], sums2[:].rearrange("c (co x) -> c co x", x=4),
                axis=mybir.AxisListType.X, op=ALU.add)
            nc.vector.tensor_reduce(
                pack2[:, 1, :], sq2[:].rearrange("c (co x) -> c co x", x=4),
                axis=mybir.AxisListType.X, op=ALU.add)
            nc.sync.dma_start(ar2_in[:].rearrange("s (co c) -> c s co", c=128), pack2[:])
            nc.gpsimd.collective_compute(
                "AllReduce", ALU.add,
                replica_groups=[list(range(NCORES))],
                ins=[ar2_in.opt()], outs=[ar2_out.opt()])
            nc.sync.dma_start(
                st2[:].rearrange("c g s co -> c (g s) co"),
                ar2_out[:].rearrange("s (co c) -> c s co", c=128))

            # ---- BN2 / BNsc scale+shift ----
            # final = relu(s2*(c2 + (ssc/s2)*csc) + (t2 + tsc))
            nc.vector.tensor_scalar_mul(st2[:], st2[:], 1.0 / CNT)
            means = st2[:, :, 0, :]
            e2s = st2[:, :, 1, :]
            gpair = gbv[:, 2:, :].rearrange("c (g s) co -> c g s co", s=2)
            nc.vector.tensor_tensor(m2b[:], means, means, op=ALU.mult)
            nc.vector.tensor_tensor(varb[:], e2s, m2b[:], op=ALU.subtract)
            nc.scalar.activation(varb[:], varb[:], AFT.Sqrt, bias=epsc[:])
            nc.vector.reciprocal(invb[:], varb[:])
            nc.vector.tensor_tensor(scaleb[:], gpair[:, :, 0, :], invb[:], op=ALU.mult)
            nc.vector.tensor_tensor(m2b[:], means, scaleb[:], op=ALU.mult)
            nc.vector.tensor_tensor(shiftb2[:], gpair[:, :, 1, :], m2b[:], op=ALU.subtract)
            nc.vector.tensor_tensor(shiftB[:], shiftb2[:, 0, :], shiftb2[:, 1, :], op=ALU.add)
            nc.vector.reciprocal(tmpa[:], scaleb[:, 0, :])
            nc.vector.tensor_tensor(rmix[:], scaleb[:, 1, :], tmpa[:], op=ALU.mult)

            # ---- final fuse: c2 += rmix*csc ; out = relu(scale2*c2 + shiftB) ----
            for co in range(NCO):
                for iph in range(4):
                    nc.vector.scalar_tensor_tensor(
                        C2[co][:, iph].rearrange("c b t w -> c (b t w)"),
                        CSC[co][:, iph].rearrange("c b t w -> c (b t w)"),
                        rmix[:, co:co + 1],
                        C2[co][:, iph].rearrange("c b t w -> c (b t w)"),
                        op0=ALU.mult, op1=ALU.add)
                for b in range(B_LOC):
                    fin = scr_pool.tile([128, 1024], f32, name="fin", tag="fin",
                                        bufs=2)
                    for p in range(2):
                        # phases (2p, 2p+1) together: src dims (q, th, tw)
                        dst = fin.rearrange(
                            "c (th p2 tw q2) -> c p2 q2 th tw",
                            th=16, p2=2, q2=2)[:, p]
                        nc.scalar.activation(dst, C2[co][:, 2 * p:2 * p + 2, b],
                                             AFT.Relu,
                                             bias=shiftB[:, co:co + 1],
                                             scale=scaleb[:, 0, co:co + 1])
                    nc.sync.dma_start(
                        out_d[b, co * 128:(co + 1) * 128].rearrange("c h w -> c (h w)"),
                        fin[:])

            for _f in _frees:
                _f()

    nc.compile()
    return nc


def _get_nc():
    if "nc" not in _CACHE:
        _CACHE["nc"] = _build_nc()
    return _CACHE["nc"]


def _prep_inputs(x, w1, w2, wsc, g1, b1, g2, b2, gsc, bsc):
    xpad = np.zeros((B, CIN, 18, 18), dtype=np.float32)
    xpad[:, :, 1:17, 1:17] = x
    xpad = xpad.astype(BF16)
    w1g = _w5_blocks(w1).astype(BF16)
    wscg = _w5_blocks(wsc).astype(BF16)
    w2g = _w2_blocks(w2).astype(BF16)
    gb = np.stack([g1, b1, g2, b2, gsc, bsc]).astype(np.float32)   # [6, 512]
    gbt = np.ascontiguousarray(gb.reshape(6, 4, 128).transpose(2, 0, 1))  # [128, 6, 4]
    return xpad, w1g, wscg, w2g, gbt


def kernel(x, w1, g1, b1, w2, g2, b2, wsc, gsc, bsc, _trace=False, **_kw):
    from concourse.bass_utils import run_bass_kernel_spmd

    x = np.asarray(x, dtype=np.float32)
    xpad, w1g, wscg, w2g, gbt = _prep_inputs(
        np.asarray(x), np.asarray(w1), np.asarray(w2), np.asarray(wsc),
        np.asarray(g1), np.asarray(b1), np.asarray(g2), np.asarray(b2),
        np.asarray(gsc), np.asarray(bsc))

    nc = _get_nc()
    in_maps = []
    for core in range(NCORES):
        in_maps.append({
            "xpad": xpad[core * B_LOC:(core + 1) * B_LOC],
            "w1g": w1g, "wscg": wscg, "w2g": w2g, "gb": gbt,
        })
    res = run_bass_kernel_spmd(nc, in_maps, list(range(NCORES)), trace=_trace)
    out = np.concatenate([res.results[i]["out"] for i in range(NCORES)], axis=0)
    if _trace:
        _CACHE["last_result"] = res
    return out


# revision 37
# speedup vs baseline: 1.3807x; 1.0035x over previous
"""Trainium2 Bass kernel for the Gudi UpProj block — Winograd + bf16.

Reference computation (per image, NCHW):
    xu  = zero_stuff_2x(x)                    # [B,1024,32,32]
    c1  = conv5x5(xu, w1, pad=2);  out1 = relu(BN(c1))
    c2  = conv3x3(out1, w2, pad=1)
    csc = conv5x5(xu, wsc, pad=2)
    out = relu(BN(c2) + BN(csc))              # BN: batch stats over (N,H,W)

Strategy:
  * Data-parallel over batch: 16 images -> 2 per core (8 cores).
  * Zero-stuffing: the 5x5 conv decomposes into 4 parity phases with
    3x3 / 3x2 / 2x3 / 2x2 kernels on the 16x16 grid.
  * 1D Winograd along h on each phase: F(2,3) for 3-tap rows, F(2,2) for
    2-tap rows; w taps stay direct -> 25 -> 17.5 effective taps/quad.
  * conv2 (3x3) via full 2D Winograd F(2x2,3x3): 9 -> 4 eff taps.
  * All matmuls in bf16 (full PE rate at any N; PSUM accumulates f32).
  * BN batch stats cross-core via two small AllReduces (c1; c2+csc).
"""

import numpy as np
import ml_dtypes

NCORES = 8
B = 16
B_LOC = B // NCORES
CIN, COUT = 1024, 512
NCI, NCO = CIN // 128, COUT // 128   # 8, 4
EPS = 1e-5
CNT = float(B * 32 * 32)
PHASES = [(0, 0), (0, 1), (1, 0), (1, 1)]
BF16 = ml_dtypes.bfloat16

# F(2,3): 3-tap kernel, 2 outputs, 4 positions; F(2,2): 2-tap, 2 out, 3 pos.
G3 = np.array([[1, 0, 0], [.5, .5, .5], [.5, -.5, .5], [0, 0, 1]], np.float64)
G2 = np.array([[1, 0], [1, 1], [0, 1]], np.float64)
# Output transforms hardcoded in drain code:
#   F(2,3): y0 = m0+m1+m2 ; y1 = m1-m2-m3
#   F(2,2): y0 = m0+m1    ; y1 = m1+m2

_CACHE = {}


def _aws(q):
    return (-1, 0, 1) if q == 0 else (0, 1)


def _ht(p):
    return 4 if p == 0 else 3


def _w5_order():
    """(phase, s, ci, aw) emission order for conv5 weight blocks."""
    order = []
    for (p, q) in PHASES:
        for s in range(_ht(p)):
            for ci in range(NCI):
                for aw in _aws(q):
                    order.append((p, q, ci, s, aw))
    return order


N_W5 = len(_w5_order())  # 280


def _w5_blocks(w):
    """w [Cout, Cin, 5, 5] f32 -> [280, 128, COUT] transformed blocks (f32)."""
    w = np.asarray(w, np.float64)
    out = np.empty((N_W5, 128, COUT), np.float32)
    for g, (p, q, ci, s, aw) in enumerate(_w5_order()):
        Gm = G3 if p == 0 else G2
        nk = 3 - p
        acc = np.zeros((COUT, 128), np.float64)
        for k in range(nk):
            kh = 2 * k + p
            kw = 2 * (aw + 1) if q == 0 else 2 * aw + 1
            acc += Gm[s, k] * w[:, ci * 128:(ci + 1) * 128, kh, kw]
        out[g] = acc.T
    return out


def _w2_blocks(w2):
    """w2 [Cout, Cin, 3, 3] -> [16, 128, 16*128]: per (co, s): [c, (ci sw), m].

    DMA'd per (co, s) with per-partition-contiguous 4KB lines.
    """
    W2p = np.einsum('sr,ocrv,wv->swoc', G3, np.asarray(w2, np.float64), G3)
    out = np.empty((16, 128, 16, 128), np.float32)
    for co in range(NCO):
        for s in range(4):
            for ci in range(4):
                for sw in range(4):
                    blk = W2p[s, sw, co * 128:(co + 1) * 128,
                              ci * 128:(ci + 1) * 128]   # [m, c]
                    out[co * 4 + s, :, ci * 4 + sw, :] = blk.T
    return out.reshape(16, 128, 16 * 128)


def _build_nc():
    import concourse.mybir as mybir
    import concourse.tile as tile
    from concourse import bacc

    f32 = mybir.dt.float32
    bf = mybir.dt.bfloat16
    ALU = mybir.AluOpType
    AFT = mybir.ActivationFunctionType

    nc = bacc.Bacc("TRN2", target_bir_lowering=False, debug=False)

    xpad_d = nc.dram_tensor("xpad", [B_LOC, CIN, 18, 18], bf, kind="ExternalInput").ap()
    w1g_d = nc.dram_tensor("w1g", [N_W5, 128, COUT], bf, kind="ExternalInput").ap()
    wscg_d = nc.dram_tensor("wscg", [N_W5, 128, COUT], bf, kind="ExternalInput").ap()
    w2g_d = nc.dram_tensor("w2g", [16, 128, 16 * 128], bf, kind="ExternalInput").ap()
    gb_d = nc.dram_tensor("gb", [128, 6, 4], f32, kind="ExternalInput").ap()
    out_d = nc.dram_tensor("out", [B_LOC, COUT, 32, 32], f32, kind="ExternalOutput").ap()

    def r256(ap):
        return ap.rearrange("c (b t w) -> c b t w", b=2, t=8, w=16)

    def m128b(t, half, b):
        return t[:, half * 256 + b * 128:half * 256 + (b + 1) * 128].rearrange(
            "c (t w) -> c t w", t=8)

    with tile.TileContext(nc) as tc:
        _frees = []
        ar1_in, _f = tc.tile([2, COUT], f32, space="DRAM", name="ar1_in"); _frees.append(_f)
        ar1_out, _f = tc.tile([2, COUT], f32, space="DRAM", addr_space="Shared", name="ar1_out"); _frees.append(_f)
        ar2_in, _f = tc.tile([4, COUT], f32, space="DRAM", name="ar2_in"); _frees.append(_f)
        ar2_out, _f = tc.tile([4, COUT], f32, space="DRAM", addr_space="Shared", name="ar2_out"); _frees.append(_f)

        with tc.tile_pool(name="xp", bufs=1) as xp_pool, \
             tc.tile_pool(name="uv", bufs=1) as uv_pool, \
             tc.tile_pool(name="acts", bufs=1) as acts, \
             tc.tile_pool(name="op1", bufs=1) as op1_pool, \
             tc.tile_pool(name="u2", bufs=1) as u2_pool, \
             tc.tile_pool(name="wts", bufs=4) as wts, \
             tc.tile_pool(name="w2p", bufs=2) as w2p, \
             tc.tile_pool(name="scr", bufs=1) as scr_pool, \
             tc.tile_pool(name="small", bufs=1) as small, \
             tc.tile_pool(name="ps", bufs=8, space="PSUM") as ps:

            # ---- persistent SBUF tensors ----
            XP = [xp_pool.tile([128, 2, 18, 18], bf, name=f"xp{i}", tag=f"xp{i}")
                  for i in range(NCI)]
            # U: h-Winograd F(2,3) positions for p=0 phases; V: F(2,2) (v0,v2)
            U = [uv_pool.tile([128, 4, 2, 8, 18], bf, name=f"u{i}", tag=f"u{i}")
                 for i in range(NCI)]
            V = [uv_pool.tile([128, 2, 2, 8, 18], bf, name=f"v{i}", tag=f"v{i}")
                 for i in range(NCI)]
            # OP1: BN1(relu) output, padded, parity-split cols:
            # [c, b, h(0..33), par, wp]; par0 = even cols 0..32, par1 = odd 1..33
            OP1 = [op1_pool.tile([128, 2, 34, 2, 17], bf, name=f"op1_{i}", tag=f"op1_{i}")
                   for i in range(NCO)]
            # CSC / C2: phase-major [c, ph(2p+q), b, i, j]
            CSC = [acts.tile([128, 4, 2, 16, 16], bf, name=f"csc{i}", tag=f"csc{i}")
                   for i in range(NCO)]
            C2 = [acts.tile([128, 4, 2, 16, 16], bf, name=f"c2_{i}", tag=f"c2_{i}")
                  for i in range(NCO)]
            # U2: conv2 2D-Winograd input transform, per OP1 tile (=ci2)
            U2 = [u2_pool.tile([128, 16, 2, 16, 16], bf, name=f"u2_{i}", tag=f"u2_{i}")
                  for i in range(NCO)]

            # stats: conv5 sums col = ((co*4 + iph)*2 + j)*2 + b (per-image,
            # since the split drain ops each carry their own accum); sq col =
            # (co*4 + iph)*2 + j; conv2 col = co*4 + ph
            sums1 = small.tile([128, 64], f32, name="sums1")
            sq1 = small.tile([128, 64], f32, name="sq1")
            sumssc = small.tile([128, 64], f32, name="sumssc")
            sqsc = small.tile([128, 64], f32, name="sqsc")
            sums2 = small.tile([128, 16], f32, name="sums2")
            sq2 = small.tile([128, 16], f32, name="sq2")
            pack1 = small.tile([128, 2, 4], f32, name="pack1")
            pack2 = small.tile([128, 4, 4], f32, name="pack2")
            st1 = small.tile([128, 2, 4], f32, name="st1")
            st2 = small.tile([128, 2, 2, 4], f32, name="st2")
            gbv = small.tile([128, 6, 4], f32, name="gbv")
            scale1 = small.tile([128, 4], f32, name="scale1")
            shift1 = small.tile([128, 4], f32, name="shift1")
            m2b = small.tile([128, 2, 4], f32, name="m2b")
            varb = small.tile([128, 2, 4], f32, name="varb")
            invb = small.tile([128, 2, 4], f32, name="invb")
            scaleb = small.tile([128, 2, 4], f32, name="scaleb")
            shiftb2 = small.tile([128, 2, 4], f32, name="shiftb2")
            shiftB = small.tile([128, 4], f32, name="shiftB")
            rmix = small.tile([128, 4], f32, name="rmix")
            tmpa = small.tile([128, 4], f32, name="tmpa")
            tmpb = small.tile([128, 4], f32, name="tmpb")
            epsc = small.tile([128, 1], f32, name="epsc")
            sqscr = small.tile([128, 512], f32, name="sqscr")

            # ---- helpers ----
            def rowpair(xp, lo, parity):
                """rows lo..lo+15 of an 18-row dim, split in pairs, pick one."""
                return xp[:, :, lo:lo + 16].rearrange(
                    "c b (t two) w -> c b t two w", two=2)[:, :, :, parity, :]

            def emit_xp(ci):
                nc.sync.dma_start(
                    XP[ci][:].rearrange("c b h w -> c b (h w)"),
                    xpad_d[:, ci * 128:(ci + 1) * 128].rearrange("b c h w -> c b (h w)"))

            def emit_uv(ci):
                # per-image ops: walrus caps DVE APs at 3 canonical dims
                xp = XP[ci]
                for b in range(B_LOC):
                    d0 = rowpair(xp, 0, 0)[:, b]   # xpad rows 0,2..14
                    d1 = rowpair(xp, 1, 0)[:, b]   # 1,3..15
                    d2 = rowpair(xp, 2, 0)[:, b]   # 2,4..16
                    d3 = rowpair(xp, 2, 1)[:, b]   # 3,5..17
                    nc.vector.tensor_tensor(U[ci][:, 0, b], d0, d2, op=ALU.subtract)
                    nc.vector.tensor_tensor(U[ci][:, 1, b], d1, d2, op=ALU.add)
                    nc.vector.tensor_tensor(U[ci][:, 2, b], d2, d1, op=ALU.subtract)
                    nc.vector.tensor_tensor(U[ci][:, 3, b], d1, d3, op=ALU.subtract)
                    # p=1 window rows: d0'=d1, d1'=d2, d2'=d3:
                    # v0 = d0'-d1', v1 = d1' (XP view), v2 = d2'-d1'
                    nc.vector.tensor_tensor(V[ci][:, 0, b], d1, d2, op=ALU.subtract)
                    nc.vector.tensor_tensor(V[ci][:, 1, b], d3, d2, op=ALU.subtract)

            def v_rhs(ci, s, wsl):
                if s == 0:
                    return V[ci][:, 0, :, :, wsl]
                if s == 1:
                    return rowpair(XP[ci], 2, 0)[:, :, :, wsl]
                return V[ci][:, 1, :, :, wsl]

            def u_rhs(ci, s, wsl):
                return U[ci][:, s, :, :, wsl]

            def op1_dst(co, p, q, j):
                """conv1 drain dest: OP1 interior phase view [c, b, t, 16]."""
                par = 1 - q          # q=0 -> odd cols (1..31), q=1 -> even (2..32)
                wsl = slice(0, 16) if q == 0 else slice(1, 17)
                return OP1[co][:, :, 1:33].rearrange(
                    "c b (t four) par w -> c b t four par w", four=4)[
                        :, :, :, 2 * j + p, par, wsl]

            def csc_dst(co, p, q, j):
                iph = 2 * p + q
                return CSC[co][:, iph].rearrange(
                    "c b (t two) w -> c b t two w", two=2)[:, :, :, j, :]

            # ---- startup DMAs / init (first weight chunk ahead of x) ----
            wt_pre = wts.tile([128, 3, 512], bf, name="c1w0", tag="w5")
            nc.sync.dma_start(wt_pre[:], w1g_d[0:3].rearrange("l c m -> c l m"))
            emit_xp(0)
            emit_xp(1)
            nc.vector.memset(epsc[:], EPS)
            for co in range(NCO):
                nc.gpsimd.memset(OP1[co][:].rearrange("c b h p w -> c b (h p w)"), 0.0)
            emit_uv(0)
            emit_uv(1)

            # ---- conv5: phase / h-position / ci loops ----
            # Per s-position: 4 PSUM chains (one per co), accumulated over
            # (ci, aw). Drains fold the h-inverse incrementally, in place in
            # the bf16 destination:
            #   p=0: y0 = m0+m1+m2 ; y1 = m1-m2-m3
            #   p=1: y0 = m0+m1    ; y1 = m1+m2
            def conv5(wg_d, dst_fn, sums, sqs, wtag, prefetch=False, pre_wt=None):
                gofs = 0
                for iph, (p, q) in enumerate(PHASES):
                    aws = _aws(q)
                    nblk = len(aws)
                    rhs_fn = u_rhs if p == 0 else v_rhs
                    for s in range(_ht(p)):
                        pps = [ps.tile([128, 256], f32, name=f"{wtag}ps{iph}{s}_{co}",
                                       tag="psb") for co in range(NCO)]
                        for ci in range(NCI):
                            if prefetch and iph == 0 and s == 0 and ci + 2 < NCI:
                                emit_xp(ci + 2)
                                emit_uv(ci + 2)
                            if pre_wt is not None and iph == 0 and s == 0 and ci == 0:
                                wt = pre_wt
                            else:
                                wt = wts.tile([128, 3, 512], bf, name=f"{wtag}w",
                                              tag="w5")
                                nc.sync.dma_start(
                                    wt[:, :nblk, :],
                                    wg_d[gofs:gofs + nblk].rearrange("l c m -> c l m"))
                            gofs += nblk
                            for ai, aw in enumerate(aws):
                                rhs = rhs_fn(ci, s, slice(1 + aw, 17 + aw))
                                first = ci == 0 and ai == 0
                                last = ci == NCI - 1 and ai == nblk - 1
                                for co in range(NCO):
                                    nc.tensor.matmul(
                                        pps[co][:], wt[:, ai, co * 128:(co + 1) * 128],
                                        rhs, start=first, stop=last)
                        # ---- drain position s into y0/y1 (in-place partials) --
                        for co in range(NCO):
                            c0 = ((co * 4 + iph) * 2) * 2
                            for b in range(B_LOC):
                                m = pps[co][:, b * 128:(b + 1) * 128].rearrange(
                                    "c (t w) -> c t w", t=8)
                                y0 = dst_fn(co, p, q, 0)[:, b]
                                y1 = dst_fn(co, p, q, 1)[:, b]
                                s0 = sums[:, c0 + b:c0 + b + 1]
                                s1 = sums[:, c0 + 2 + b:c0 + 3 + b]
                                sg = 1.0
                                y0_ops, y1_ops = [], []
                                if s == 0:
                                    y0_ops = ["init"]
                                elif s == 1:
                                    y0_ops = ["acc"]
                                    y1_ops = ["init"]
                                elif s == 2:
                                    if p == 0:
                                        y0_ops = ["acc_fin"]
                                        y1_ops = ["neg"]
                                    else:
                                        y1_ops = ["acc_fin"]
                                else:
                                    y1_ops = ["neg_fin"]
                                if p == 1 and s == 1:
                                    y0_ops = ["acc_fin"]
                                for tgt, ops_, scol in ((y0, y0_ops, s0),
                                                        (y1, y1_ops, s1)):
                                    for opk in ops_:
                                        if opk == "init":
                                            nc.vector.tensor_scalar(
                                                tgt, m, 0.0, 0.0,
                                                op0=ALU.add, op1=ALU.add)
                                        else:
                                            fin = opk.endswith("fin")
                                            sgn = -1.0 if opk.startswith("neg") else 1.0
                                            nc.vector.scalar_tensor_tensor(
                                                tgt, m, sgn, tgt,
                                                op0=ALU.mult, op1=ALU.add,
                                                accum_out=(scol if fin else None))
                                if s == _ht(p) - 1:
                                    nc.scalar.activation(
                                        m128b(sqscr, 0, b), y0, AFT.Square,
                                        accum_out=sqs[:, c0 + b:c0 + b + 1])
                                    nc.scalar.activation(
                                        m128b(sqscr, 1, b), y1, AFT.Square,
                                        accum_out=sqs[:, c0 + 2 + b:c0 + 3 + b])

            # ================= conv1 =================
            conv5(w1g_d, op1_dst, sums1, sq1, "c1", prefetch=True, pre_wt=wt_pre)
            nc.sync.dma_start(gbv[:], gb_d)

            # ---- c1 stats -> AllReduce #1 (overlaps convsc) ----
            nc.vector.tensor_reduce(
                pack1[:, 0, :], sums1[:].rearrange("c (co x) -> c co x", x=16),
                axis=mybir.AxisListType.X, op=ALU.add)
            nc.vector.tensor_reduce(
                pack1[:, 1, :], sq1[:].rearrange("c (co x) -> c co x", x=16),
                axis=mybir.AxisListType.X, op=ALU.add)
            nc.sync.dma_start(ar1_in[:].rearrange("s (co c) -> c s co", c=128), pack1[:])
            nc.gpsimd.collective_compute(
                "AllReduce", ALU.add,
                replica_groups=[list(range(NCORES))],
                ins=[ar1_in.opt()], outs=[ar1_out.opt()])
            nc.sync.dma_start(st1[:], ar1_out[:].rearrange("s (co c) -> c s co", c=128))

            # ================= convsc =================
            conv5(wscg_d, csc_dst, sumssc, sqsc, "sc")

            # ---- BN1 scale/shift ----
            nc.vector.tensor_scalar_mul(st1[:], st1[:], 1.0 / CNT)
            m1 = st1[:, 0, :]
            nc.vector.tensor_tensor(tmpa[:], m1, m1, op=ALU.mult)
            nc.vector.tensor_tensor(tmpb[:], st1[:, 1, :], tmpa[:], op=ALU.subtract)
            nc.scalar.activation(tmpb[:], tmpb[:], AFT.Sqrt, bias=epsc[:])
            nc.vector.reciprocal(tmpa[:], tmpb[:])
            nc.vector.tensor_tensor(scale1[:], gbv[:, 0, :], tmpa[:], op=ALU.mult)
            nc.vector.tensor_tensor(tmpa[:], m1, scale1[:], op=ALU.mult)
            nc.vector.tensor_tensor(shift1[:], gbv[:, 1, :], tmpa[:], op=ALU.subtract)

            # ---- BN1 apply (in-place relu) + conv2 input transform ----
            UH = scr_pool.tile([128, 4, 2, 16, 34], bf, name="uh", tag="uh")
            for co in range(NCO):
                for par, wsl in ((0, slice(1, 17)), (1, slice(0, 16))):
                    for b in range(B_LOC):
                        v = OP1[co][:, b, 1:33, par, wsl]
                        nc.scalar.activation(v, v, AFT.Relu,
                                             bias=shift1[:, co:co + 1],
                                             scale=scale1[:, co:co + 1])
                # H-stage: window rows 2t..2t+3 of padded OP1 (34 rows)
                flat = OP1[co][:].rearrange("c b h par w -> c b h (par w)")

                def rows(lo, parity, b):
                    return flat[:, b, lo:lo + 32].rearrange(
                        "c (t two) w -> c t two w", two=2)[:, :, parity, :]

                for b in range(B_LOC):
                    d0 = rows(0, 0, b)
                    d1 = rows(1, 0, b)
                    d2 = rows(2, 0, b)
                    d3 = rows(2, 1, b)
                    nc.vector.tensor_tensor(UH[:, 0, b], d0, d2, op=ALU.subtract)
                    nc.vector.tensor_tensor(UH[:, 1, b], d1, d2, op=ALU.add)
                    nc.vector.tensor_tensor(UH[:, 2, b], d2, d1, op=ALU.subtract)
                    nc.vector.tensor_tensor(UH[:, 3, b], d1, d3, op=ALU.subtract)
                # W-stage: (par w) cols: even block wp0..16, odd block 17..33.
                # tile wt: d0 = E[wt], d1 = O[wt], d2 = E[wt+1], d3 = O[wt+1]
                for s in range(4):
                    E0 = UH[:, s, :, :, 0:16]
                    E1 = UH[:, s, :, :, 1:17]
                    O0 = UH[:, s, :, :, 17:33]
                    O1 = UH[:, s, :, :, 18:34]
                    nc.vector.tensor_tensor(U2[co][:, 4 * s + 0], E0, E1, op=ALU.subtract)
                    nc.vector.tensor_tensor(U2[co][:, 4 * s + 1], O0, E1, op=ALU.add)
                    nc.vector.tensor_tensor(U2[co][:, 4 * s + 2], E1, O0, op=ALU.subtract)
                    nc.vector.tensor_tensor(U2[co][:, 4 * s + 3], O0, O1, op=ALU.subtract)

            # ---- csc stats into pack2 rows 2,3 ----
            nc.vector.tensor_reduce(
                pack2[:, 2, :], sumssc[:].rearrange("c (co x) -> c co x", x=16),
                axis=mybir.AxisListType.X, op=ALU.add)
            nc.vector.tensor_reduce(
                pack2[:, 3, :], sqsc[:].rearrange("c (co x) -> c co x", x=16),
                axis=mybir.AxisListType.X, op=ALU.add)

            # ================= conv2 (2D Winograd F(2x2,3x3)) =================
            # co outer; per (co, s): 4 sw-chains accumulate over ci2.
            # w-inverse per (co, s) -> T[s, a]; h-inverse per co -> C2 + stats.
            for co in range(NCO):
                T = scr_pool.tile([128, 4, 2, 512], bf, name="T", tag="T", bufs=1)
                hy0 = scr_pool.tile([128, 2, 512], bf, name="hy0", tag="hy0")
                hy1 = scr_pool.tile([128, 2, 512], bf, name="hy1", tag="hy1")
                for s in range(4):
                    # weights in 2 half-chunks (deeper DMA pipeline)
                    wt2p = []
                    for k in range(2):
                        wt2 = w2p.tile([128, 8, 128], bf, name="c2w", tag="w2",
                                       bufs=4)
                        nc.sync.dma_start(
                            wt2[:].rearrange("c l m -> c (l m)"),
                            w2g_d[co * 4 + s][:, k * 1024:(k + 1) * 1024])
                        wt2p.append(wt2)
                    pps = [ps.tile([128, 512], f32, name=f"c2ps{co}{s}_{sw}",
                                   tag="psb") for sw in range(4)]
                    for ci in range(NCO):
                        for sw in range(4):
                            blk = ci * 4 + sw
                            nc.tensor.matmul(
                                pps[sw][:], wt2p[blk // 8][:, blk % 8, :],
                                U2[ci][:, 4 * s + sw].rearrange("c b t w -> c (b t w)"),
                                start=(ci == 0), stop=(ci == NCO - 1))
                    # w-inverse: T0 = m0+m1+m2 ; T1 = m1-m2-m3 (1 psum read/op)
                    a2 = scr_pool.tile([128, 512], f32, name="a2", tag="a2")
                    q0 = scr_pool.tile([128, 512], f32, name="q0", tag="q0")
                    nc.scalar.copy(a2[:], pps[1][:])
                    nc.vector.scalar_tensor_tensor(q0[:], pps[0][:], 1.0, a2[:],
                                                   op0=ALU.mult, op1=ALU.add)
                    nc.vector.scalar_tensor_tensor(T[:, s, 0], pps[2][:], 1.0,
                                                   q0[:], op0=ALU.mult, op1=ALU.add)
                    nc.vector.scalar_tensor_tensor(q0[:], pps[2][:], -1.0, a2[:],
                                                   op0=ALU.mult, op1=ALU.add)
                    nc.vector.scalar_tensor_tensor(T[:, s, 1], pps[3][:], -1.0,
                                                   q0[:], op0=ALU.mult, op1=ALU.add)
                    # incremental h-inverse: y(jh,a) -> C2 phase (2*jh + a)
                    if s == 1:
                        for a in range(2):
                            nc.vector.tensor_tensor(hy0[:, a], T[:, 0, a],
                                                    T[:, 1, a], op=ALU.add)
                    elif s == 2:
                        for a in range(2):
                            ph = a
                            dst = C2[co][:, ph].rearrange("c b t w -> c (b t w)")
                            nc.vector.scalar_tensor_tensor(
                                dst, T[:, 2, a], 1.0, hy0[:, a],
                                op0=ALU.mult, op1=ALU.add,
                                accum_out=sums2[:, co * 4 + ph:co * 4 + ph + 1])
                            nc.scalar.activation(
                                sqscr[:], dst, AFT.Square,
                                accum_out=sq2[:, co * 4 + ph:co * 4 + ph + 1])
                            nc.vector.tensor_tensor(hy1[:, a], T[:, 1, a],
                                                    T[:, 2, a], op=ALU.subtract)
                    elif s == 3:
                        for a in range(2):
                            ph = 2 + a
                            dst = C2[co][:, ph].rearrange("c b t w -> c (b t w)")
                            nc.vector.scalar_tensor_tensor(
                                dst, T[:, 3, a], -1.0, hy1[:, a],
                                op0=ALU.mult, op1=ALU.add,
                                accum_out=sums2[:, co * 4 + ph:co * 4 + ph + 1])
                            nc.scalar.activation(
                                sqscr[:], dst, AFT.Square,
                                accum_out=sq2[:, co * 4 + ph:co * 4 + ph + 1])

            # ---- stats of c2 -> AllReduce #2 ----
            nc.vector.tensor_reduce(
                pack2[:, 0, :], sums2[:].rearrange("c (co x) -> c co x", x=4),
                axis=mybir.AxisListType.X, op=ALU.add)
            nc.vector.tensor_reduce(
                pack2[:, 1, :], sq2[:].rearrange("c (co x) -> c co x", x=4),
                axis=mybir.AxisListType.X, op=ALU.add)
            nc.sync.dma_start(ar2_in[:].rearrange("s (co c) -> c s co", c=128), pack2[:])
            nc.gpsimd.collective_compute(
                "AllReduce", ALU.add,
                replica_groups=[list(range(NCORES))],
                ins=[ar2_in.opt()], outs=[ar2_out.opt()])
            nc.sync.dma_start(
                st2[:].rearrange("c g s co -> c (g s) co"),
                ar2_out[:].rearrange("s (co c) -> c s co", c=128))

            # ---- BN2 / BNsc scale+shift ----
            # final = relu(s2*(c2 + (ssc/s2)*csc) + (t2 + tsc))
            nc.vector.tensor_scalar_mul(st2[:], st2[:], 1.0 / CNT)
            means = st2[:, :, 0, :]
            e2s = st2[:, :, 1, :]
            gpair = gbv[:, 2:, :].rearrange("c (g s) co -> c g s co", s=2)
            nc.vector.tensor_tensor(m2b[:], means, means, op=ALU.mult)
            nc.vector.tensor_tensor(varb[:], e2s, m2b[:], op=ALU.subtract)
            nc.scalar.activation(varb[:], varb[:], AFT.Sqrt, bias=epsc[:])
            nc.vector.reciprocal(invb[:], varb[:])
            nc.vector.tensor_tensor(scaleb[:], gpair[:, :, 0, :], invb[:], op=ALU.mult)
            nc.vector.tensor_tensor(m2b[:], means, scaleb[:], op=ALU.mult)
            nc.vector.tensor_tensor(shiftb2[:], gpair[:, :, 1, :], m2b[:], op=ALU.subtract)
            nc.vector.tensor_tensor(shiftB[:], shiftb2[:, 0, :], shiftb2[:, 1, :], op=ALU.add)
            nc.vector.reciprocal(tmpa[:], scaleb[:, 0, :])
            nc.vector.tensor_tensor(rmix[:], scaleb[:, 1, :], tmpa[:], op=ALU.mult)

            # ---- final fuse: c2 += rmix*csc ; out = relu(scale2*c2 + shiftB) ----
            for co in range(NCO):
                for iph in range(4):
                    nc.vector.scalar_tensor_tensor(
                        C2[co][:, iph].rearrange("c b t w -> c (b t w)"),
                        CSC[co][:, iph].rearrange("c b t w -> c (b t w)"),
                        rmix[:, co:co + 1],
                        C2[co][:, iph].rearrange("c b t w -> c (b t w)"),
                        op0=ALU.mult, op1=ALU.add)
                for b in range(B_LOC):
                    fin = scr_pool.tile([128, 1024], f32, name="fin", tag="fin",
                                        bufs=2)
                    for p in range(2):
                        # phases (2p, 2p+1) together: src dims (q, th, tw)
                        dst = fin.rearrange(
                            "c (th p2 tw q2) -> c p2 q2 th tw",
                            th=16, p2=2, q2=2)[:, p]
                        nc.scalar.activation(dst, C2[co][:, 2 * p:2 * p + 2, b],
                                             AFT.Relu,
                                             bias=shiftB[:, co:co + 1],
                                             scale=scaleb[:, 0, co:co + 1])
                    nc.sync.dma_start(
                        out_d[b, co * 128:(co + 1) * 128].rearrange("c h w -> c (h w)"),
                        fin[:])

            for _f in _frees:
                _f()

    nc.compile()
    return nc


def _get_nc():
    if "nc" not in _CACHE:
        _CACHE["nc"] = _build_nc()
    return _CACHE["nc"]


def _prep_inputs(x, w1, w2, wsc, g1, b1, g2, b2, gsc, bsc):
    xpad = np.zeros((B, CIN, 18, 18), dtype=np.float32)
    xpad[:, :, 1:17, 1:17] = x
    xpad = xpad.astype(BF16)
    w1g = _w5_blocks(w1).astype(BF16)
    wscg = _w5_blocks(wsc).astype(BF16)
    w2g = _w2_blocks(w2).astype(BF16)
    gb = np.stack([g1, b1, g2, b2, gsc, bsc]).astype(np.float32)   # [6, 512]
    gbt = np.ascontiguousarray(gb.reshape(6, 4, 128).transpose(2, 0, 1))  # [128, 6, 4]
    return xpad, w1g, wscg, w2g, gbt


def kernel(x, w1, g1, b1, w2, g2, b2, wsc, gsc, bsc, _trace=False, **_kw):
    from concourse.bass_utils import run_bass_kernel_spmd

    x = np.asarray(x, dtype=np.float32)
    xpad, w1g, wscg, w2g, gbt = _prep_inputs(
        np.asarray(x), np.asarray(w1), np.asarray(w2), np.asarray(wsc),
        np.asarray(g1), np.asarray(b1), np.asarray(g2), np.asarray(b2),
        np.asarray(gsc), np.asarray(bsc))

    nc = _get_nc()
    in_maps = []
    for core in range(NCORES):
        in_maps.append({
            "xpad": xpad[core * B_LOC:(core + 1) * B_LOC],
            "w1g": w1g, "wscg": wscg, "w2g": w2g, "gb": gbt,
        })
    res = run_bass_kernel_spmd(nc, in_maps, list(range(NCORES)), trace=_trace)
    out = np.concatenate([res.results[i]["out"] for i in range(NCORES)], axis=0)
    if _trace:
        _CACHE["last_result"] = res
    return out


# revision 38
# speedup vs baseline: 1.3897x; 1.0065x over previous
"""Trainium2 Bass kernel for the Gudi UpProj block — Winograd + bf16.

Reference computation (per image, NCHW):
    xu  = zero_stuff_2x(x)                    # [B,1024,32,32]
    c1  = conv5x5(xu, w1, pad=2);  out1 = relu(BN(c1))
    c2  = conv3x3(out1, w2, pad=1)
    csc = conv5x5(xu, wsc, pad=2)
    out = relu(BN(c2) + BN(csc))              # BN: batch stats over (N,H,W)

Strategy:
  * Data-parallel over batch: 16 images -> 2 per core (8 cores).
  * Zero-stuffing: the 5x5 conv decomposes into 4 parity phases with
    3x3 / 3x2 / 2x3 / 2x2 kernels on the 16x16 grid.
  * 1D Winograd along h on each phase: F(2,3) for 3-tap rows, F(2,2) for
    2-tap rows; w taps stay direct -> 25 -> 17.5 effective taps/quad.
  * conv2 (3x3) via full 2D Winograd F(2x2,3x3): 9 -> 4 eff taps.
  * All matmuls in bf16 (full PE rate at any N; PSUM accumulates f32).
  * BN batch stats cross-core via two small AllReduces (c1; c2+csc).
"""

import numpy as np
import ml_dtypes

NCORES = 8
B = 16
B_LOC = B // NCORES
CIN, COUT = 1024, 512
NCI, NCO = CIN // 128, COUT // 128   # 8, 4
EPS = 1e-5
CNT = float(B * 32 * 32)
PHASES = [(0, 0), (0, 1), (1, 0), (1, 1)]
BF16 = ml_dtypes.bfloat16

# F(2,3): 3-tap kernel, 2 outputs, 4 positions; F(2,2): 2-tap, 2 out, 3 pos.
G3 = np.array([[1, 0, 0], [.5, .5, .5], [.5, -.5, .5], [0, 0, 1]], np.float64)
G2 = np.array([[1, 0], [1, 1], [0, 1]], np.float64)
# Output transforms hardcoded in drain code:
#   F(2,3): y0 = m0+m1+m2 ; y1 = m1-m2-m3
#   F(2,2): y0 = m0+m1    ; y1 = m1+m2

_CACHE = {}


def _aws(q):
    return (-1, 0, 1) if q == 0 else (0, 1)


def _ht(p):
    return 4 if p == 0 else 3


def _w5_order():
    """(phase, s, ci, aw) emission order for conv5 weight blocks."""
    order = []
    for (p, q) in PHASES:
        for s in range(_ht(p)):
            for ci in range(NCI):
                for aw in _aws(q):
                    order.append((p, q, ci, s, aw))
    return order


N_W5 = len(_w5_order())  # 280


def _w5_blocks(w):
    """w [Cout, Cin, 5, 5] f32 -> [280, 128, COUT] transformed blocks (f32)."""
    w = np.asarray(w, np.float64)
    out = np.empty((N_W5, 128, COUT), np.float32)
    for g, (p, q, ci, s, aw) in enumerate(_w5_order()):
        Gm = G3 if p == 0 else G2
        nk = 3 - p
        acc = np.zeros((COUT, 128), np.float64)
        for k in range(nk):
            kh = 2 * k + p
            kw = 2 * (aw + 1) if q == 0 else 2 * aw + 1
            acc += Gm[s, k] * w[:, ci * 128:(ci + 1) * 128, kh, kw]
        out[g] = acc.T
    return out


def _w2_blocks(w2):
    """w2 [Cout, Cin, 3, 3] -> [16, 128, 16*128]: per (co, s): [c, (ci sw), m].

    DMA'd per (co, s) with per-partition-contiguous 4KB lines.
    """
    W2p = np.einsum('sr,ocrv,wv->swoc', G3, np.asarray(w2, np.float64), G3)
    out = np.empty((16, 128, 16, 128), np.float32)
    for co in range(NCO):
        for s in range(4):
            for ci in range(4):
                for sw in range(4):
                    blk = W2p[s, sw, co * 128:(co + 1) * 128,
                              ci * 128:(ci + 1) * 128]   # [m, c]
                    out[co * 4 + s, :, ci * 4 + sw, :] = blk.T
    return out.reshape(16, 128, 16 * 128)


def _build_nc():
    import concourse.mybir as mybir
    import concourse.tile as tile
    from concourse import bacc

    f32 = mybir.dt.float32
    bf = mybir.dt.bfloat16
    ALU = mybir.AluOpType
    AFT = mybir.ActivationFunctionType

    nc = bacc.Bacc("TRN2", target_bir_lowering=False, debug=False)

    xpad_d = nc.dram_tensor("xpad", [B_LOC, CIN, 18, 18], bf, kind="ExternalInput").ap()
    w1g_d = nc.dram_tensor("w1g", [N_W5, 128, COUT], bf, kind="ExternalInput").ap()
    wscg_d = nc.dram_tensor("wscg", [N_W5, 128, COUT], bf, kind="ExternalInput").ap()
    w2g_d = nc.dram_tensor("w2g", [16, 128, 16 * 128], bf, kind="ExternalInput").ap()
    gb_d = nc.dram_tensor("gb", [128, 6, 4], f32, kind="ExternalInput").ap()
    out_d = nc.dram_tensor("out", [B_LOC, COUT, 32, 32], f32, kind="ExternalOutput").ap()

    def r256(ap):
        return ap.rearrange("c (b t w) -> c b t w", b=2, t=8, w=16)

    def m128b(t, half, b):
        return t[:, half * 256 + b * 128:half * 256 + (b + 1) * 128].rearrange(
            "c (t w) -> c t w", t=8)

    with tile.TileContext(nc) as tc:
        _frees = []
        ar1_in, _f = tc.tile([2, COUT], f32, space="DRAM", name="ar1_in"); _frees.append(_f)
        ar1_out, _f = tc.tile([2, COUT], f32, space="DRAM", addr_space="Shared", name="ar1_out"); _frees.append(_f)
        ar2_in, _f = tc.tile([4, COUT], f32, space="DRAM", name="ar2_in"); _frees.append(_f)
        ar2_out, _f = tc.tile([4, COUT], f32, space="DRAM", addr_space="Shared", name="ar2_out"); _frees.append(_f)

        with tc.tile_pool(name="xp", bufs=1) as xp_pool, \
             tc.tile_pool(name="uv", bufs=1) as uv_pool, \
             tc.tile_pool(name="acts", bufs=1) as acts, \
             tc.tile_pool(name="op1", bufs=1) as op1_pool, \
             tc.tile_pool(name="u2", bufs=1) as u2_pool, \
             tc.tile_pool(name="wts", bufs=4) as wts, \
             tc.tile_pool(name="w2p", bufs=2) as w2p, \
             tc.tile_pool(name="scr", bufs=1) as scr_pool, \
             tc.tile_pool(name="small", bufs=1) as small, \
             tc.tile_pool(name="ps", bufs=8, space="PSUM") as ps:

            # ---- persistent SBUF tensors ----
            XP = [xp_pool.tile([128, 2, 18, 18], bf, name=f"xp{i}", tag=f"xp{i}")
                  for i in range(NCI)]
            # U: h-Winograd F(2,3) positions for p=0 phases; V: F(2,2) (v0,v2)
            U = [uv_pool.tile([128, 4, 2, 8, 18], bf, name=f"u{i}", tag=f"u{i}")
                 for i in range(NCI)]
            V = [uv_pool.tile([128, 2, 2, 8, 18], bf, name=f"v{i}", tag=f"v{i}")
                 for i in range(NCI)]
            # OP1: BN1(relu) output, padded, parity-split cols:
            # [c, b, h(0..33), par, wp]; par0 = even cols 0..32, par1 = odd 1..33
            OP1 = [op1_pool.tile([128, 2, 34, 2, 17], bf, name=f"op1_{i}", tag=f"op1_{i}")
                   for i in range(NCO)]
            # CSC / C2: phase-major [c, ph(2p+q), b, i, j]
            CSC = [acts.tile([128, 4, 2, 16, 16], bf, name=f"csc{i}", tag=f"csc{i}")
                   for i in range(NCO)]
            C2 = [acts.tile([128, 4, 2, 16, 16], bf, name=f"c2_{i}", tag=f"c2_{i}")
                  for i in range(NCO)]
            # U2: conv2 2D-Winograd input transform, per OP1 tile (=ci2)
            U2 = [u2_pool.tile([128, 16, 2, 16, 16], bf, name=f"u2_{i}", tag=f"u2_{i}")
                  for i in range(NCO)]

            # stats: conv5 sums col = ((co*4 + iph)*2 + j)*2 + b (per-image,
            # since the split drain ops each carry their own accum); sq col =
            # (co*4 + iph)*2 + j; conv2 col = co*4 + ph
            sums1 = small.tile([128, 64], f32, name="sums1")
            sq1 = small.tile([128, 64], f32, name="sq1")
            sumssc = small.tile([128, 64], f32, name="sumssc")
            sqsc = small.tile([128, 64], f32, name="sqsc")
            sums2 = small.tile([128, 16], f32, name="sums2")
            sq2 = small.tile([128, 16], f32, name="sq2")
            pack1 = small.tile([128, 2, 4], f32, name="pack1")
            pack2 = small.tile([128, 4, 4], f32, name="pack2")
            st1 = small.tile([128, 2, 4], f32, name="st1")
            st2 = small.tile([128, 2, 2, 4], f32, name="st2")
            gbv = small.tile([128, 6, 4], f32, name="gbv")
            scale1 = small.tile([128, 4], f32, name="scale1")
            shift1 = small.tile([128, 4], f32, name="shift1")
            m2b = small.tile([128, 2, 4], f32, name="m2b")
            varb = small.tile([128, 2, 4], f32, name="varb")
            invb = small.tile([128, 2, 4], f32, name="invb")
            scaleb = small.tile([128, 2, 4], f32, name="scaleb")
            shiftb2 = small.tile([128, 2, 4], f32, name="shiftb2")
            shiftB = small.tile([128, 4], f32, name="shiftB")
            rmix = small.tile([128, 4], f32, name="rmix")
            tmpa = small.tile([128, 4], f32, name="tmpa")
            tmpb = small.tile([128, 4], f32, name="tmpb")
            epsc = small.tile([128, 1], f32, name="epsc")
            sqscr = small.tile([128, 512], f32, name="sqscr")

            # ---- helpers ----
            def rowpair(xp, lo, parity):
                """rows lo..lo+15 of an 18-row dim, split in pairs, pick one."""
                return xp[:, :, lo:lo + 16].rearrange(
                    "c b (t two) w -> c b t two w", two=2)[:, :, :, parity, :]

            def emit_xp(ci):
                nc.sync.dma_start(
                    XP[ci][:].rearrange("c b h w -> c b (h w)"),
                    xpad_d[:, ci * 128:(ci + 1) * 128].rearrange("b c h w -> c b (h w)"))

            def emit_uv(ci):
                # per-image ops: walrus caps DVE APs at 3 canonical dims
                xp = XP[ci]
                for b in range(B_LOC):
                    d0 = rowpair(xp, 0, 0)[:, b]   # xpad rows 0,2..14
                    d1 = rowpair(xp, 1, 0)[:, b]   # 1,3..15
                    d2 = rowpair(xp, 2, 0)[:, b]   # 2,4..16
                    d3 = rowpair(xp, 2, 1)[:, b]   # 3,5..17
                    nc.vector.tensor_tensor(U[ci][:, 0, b], d0, d2, op=ALU.subtract)
                    nc.vector.tensor_tensor(U[ci][:, 1, b], d1, d2, op=ALU.add)
                    nc.vector.tensor_tensor(U[ci][:, 2, b], d2, d1, op=ALU.subtract)
                    nc.vector.tensor_tensor(U[ci][:, 3, b], d1, d3, op=ALU.subtract)
                    # p=1 window rows: d0'=d1, d1'=d2, d2'=d3:
                    # v0 = d0'-d1', v1 = d1' (XP view), v2 = d2'-d1'
                    nc.vector.tensor_tensor(V[ci][:, 0, b], d1, d2, op=ALU.subtract)
                    nc.vector.tensor_tensor(V[ci][:, 1, b], d3, d2, op=ALU.subtract)

            def v_rhs(ci, s, wsl):
                if s == 0:
                    return V[ci][:, 0, :, :, wsl]
                if s == 1:
                    return rowpair(XP[ci], 2, 0)[:, :, :, wsl]
                return V[ci][:, 1, :, :, wsl]

            def u_rhs(ci, s, wsl):
                return U[ci][:, s, :, :, wsl]

            def op1_dst(co, p, q, j):
                """conv1 drain dest: OP1 interior phase view [c, b, t, 16]."""
                par = 1 - q          # q=0 -> odd cols (1..31), q=1 -> even (2..32)
                wsl = slice(0, 16) if q == 0 else slice(1, 17)
                return OP1[co][:, :, 1:33].rearrange(
                    "c b (t four) par w -> c b t four par w", four=4)[
                        :, :, :, 2 * j + p, par, wsl]

            def csc_dst(co, p, q, j):
                iph = 2 * p + q
                return CSC[co][:, iph].rearrange(
                    "c b (t two) w -> c b t two w", two=2)[:, :, :, j, :]

            # ---- startup DMAs / init (first weight chunk ahead of x) ----
            wt_pre = wts.tile([128, 3, 512], bf, name="c1w0", tag="w5")
            nc.sync.dma_start(wt_pre[:], w1g_d[0:3].rearrange("l c m -> c l m"))
            emit_xp(0)
            emit_xp(1)
            nc.vector.memset(epsc[:], EPS)
            for co in range(NCO):
                nc.gpsimd.memset(OP1[co][:].rearrange("c b h p w -> c b (h p w)"), 0.0)
            emit_uv(0)
            emit_uv(1)

            # ---- conv5: phase / h-position / ci loops ----
            # Per s-position: 4 PSUM chains (one per co), accumulated over
            # (ci, aw). Drains fold the h-inverse incrementally, in place in
            # the bf16 destination:
            #   p=0: y0 = m0+m1+m2 ; y1 = m1-m2-m3
            #   p=1: y0 = m0+m1    ; y1 = m1+m2
            def conv5(wg_d, dst_fn, sums, sqs, wtag, prefetch=False, pre_wt=None):
                gofs = 0
                for iph, (p, q) in enumerate(PHASES):
                    aws = _aws(q)
                    nblk = len(aws)
                    rhs_fn = u_rhs if p == 0 else v_rhs
                    for s in range(_ht(p)):
                        pps = [ps.tile([128, 256], f32, name=f"{wtag}ps{iph}{s}_{co}",
                                       tag="psb") for co in range(NCO)]
                        for ci in range(NCI):
                            if prefetch and iph == 0 and s == 0 and ci + 2 < NCI:
                                emit_xp(ci + 2)
                                emit_uv(ci + 2)
                            if pre_wt is not None and iph == 0 and s == 0 and ci == 0:
                                wt = pre_wt
                            else:
                                wt = wts.tile([128, 3, 512], bf, name=f"{wtag}w",
                                              tag="w5")
                                nc.sync.dma_start(
                                    wt[:, :nblk, :],
                                    wg_d[gofs:gofs + nblk].rearrange("l c m -> c l m"))
                            gofs += nblk
                            for ai, aw in enumerate(aws):
                                rhs = rhs_fn(ci, s, slice(1 + aw, 17 + aw))
                                first = ci == 0 and ai == 0
                                last = ci == NCI - 1 and ai == nblk - 1
                                for co in range(NCO):
                                    nc.tensor.matmul(
                                        pps[co][:], wt[:, ai, co * 128:(co + 1) * 128],
                                        rhs, start=first, stop=last)
                        # ---- drain position s into y0/y1 (in-place partials) --
                        for co in range(NCO):
                            c0 = ((co * 4 + iph) * 2) * 2
                            for b in range(B_LOC):
                                m = pps[co][:, b * 128:(b + 1) * 128].rearrange(
                                    "c (t w) -> c t w", t=8)
                                y0 = dst_fn(co, p, q, 0)[:, b]
                                y1 = dst_fn(co, p, q, 1)[:, b]
                                s0 = sums[:, c0 + b:c0 + b + 1]
                                s1 = sums[:, c0 + 2 + b:c0 + 3 + b]
                                sg = 1.0
                                y0_ops, y1_ops = [], []
                                if s == 0:
                                    y0_ops = ["init"]
                                elif s == 1:
                                    y0_ops = ["acc"]
                                    y1_ops = ["init"]
                                elif s == 2:
                                    if p == 0:
                                        y0_ops = ["acc_fin"]
                                        y1_ops = ["neg"]
                                    else:
                                        y1_ops = ["acc_fin"]
                                else:
                                    y1_ops = ["neg_fin"]
                                if p == 1 and s == 1:
                                    y0_ops = ["acc_fin"]
                                for tgt, ops_, scol in ((y0, y0_ops, s0),
                                                        (y1, y1_ops, s1)):
                                    for opk in ops_:
                                        if opk == "init":
                                            nc.vector.tensor_scalar(
                                                tgt, m, 0.0, 0.0,
                                                op0=ALU.add, op1=ALU.add)
                                        else:
                                            fin = opk.endswith("fin")
                                            sgn = -1.0 if opk.startswith("neg") else 1.0
                                            nc.vector.scalar_tensor_tensor(
                                                tgt, m, sgn, tgt,
                                                op0=ALU.mult, op1=ALU.add,
                                                accum_out=(scol if fin else None))
                                if s == _ht(p) - 1:
                                    nc.scalar.activation(
                                        m128b(sqscr, 0, b), y0, AFT.Square,
                                        accum_out=sqs[:, c0 + b:c0 + b + 1])
                                    nc.scalar.activation(
                                        m128b(sqscr, 1, b), y1, AFT.Square,
                                        accum_out=sqs[:, c0 + 2 + b:c0 + 3 + b])

            # ================= conv1 =================
            conv5(w1g_d, op1_dst, sums1, sq1, "c1", prefetch=True, pre_wt=wt_pre)
            nc.sync.dma_start(gbv[:], gb_d)

            # ---- c1 stats -> AllReduce #1 (overlaps convsc) ----
            nc.vector.tensor_reduce(
                pack1[:, 0, :], sums1[:].rearrange("c (co x) -> c co x", x=16),
                axis=mybir.AxisListType.X, op=ALU.add)
            nc.vector.tensor_reduce(
                pack1[:, 1, :], sq1[:].rearrange("c (co x) -> c co x", x=16),
                axis=mybir.AxisListType.X, op=ALU.add)
            nc.sync.dma_start(ar1_in[:].rearrange("s (co c) -> c s co", c=128), pack1[:])
            nc.gpsimd.collective_compute(
                "AllReduce", ALU.add,
                replica_groups=[list(range(NCORES))],
                ins=[ar1_in.opt()], outs=[ar1_out.opt()])
            nc.sync.dma_start(st1[:], ar1_out[:].rearrange("s (co c) -> c s co", c=128))

            # ================= convsc =================
            conv5(wscg_d, csc_dst, sumssc, sqsc, "sc")

            # ---- BN1 scale/shift ----
            nc.vector.tensor_scalar_mul(st1[:], st1[:], 1.0 / CNT)
            m1 = st1[:, 0, :]
            nc.vector.tensor_tensor(tmpa[:], m1, m1, op=ALU.mult)
            nc.vector.tensor_tensor(tmpb[:], st1[:, 1, :], tmpa[:], op=ALU.subtract)
            nc.scalar.activation(tmpb[:], tmpb[:], AFT.Sqrt, bias=epsc[:])
            nc.vector.reciprocal(tmpa[:], tmpb[:])
            nc.vector.tensor_tensor(scale1[:], gbv[:, 0, :], tmpa[:], op=ALU.mult)
            nc.vector.tensor_tensor(tmpa[:], m1, scale1[:], op=ALU.mult)
            nc.vector.tensor_tensor(shift1[:], gbv[:, 1, :], tmpa[:], op=ALU.subtract)

            # ---- BN1 apply (in-place relu) + conv2 input transform ----
            UH = scr_pool.tile([128, 4, 2, 16, 34], bf, name="uh", tag="uh")
            for co in range(NCO):
                for par, wsl in ((0, slice(1, 17)), (1, slice(0, 16))):
                    for b in range(B_LOC):
                        v = OP1[co][:, b, 1:33, par, wsl]
                        nc.scalar.activation(v, v, AFT.Relu,
                                             bias=shift1[:, co:co + 1],
                                             scale=scale1[:, co:co + 1])
                # H-stage: window rows 2t..2t+3 of padded OP1 (34 rows)
                flat = OP1[co][:].rearrange("c b h par w -> c b h (par w)")

                def rows(lo, parity, b):
                    return flat[:, b, lo:lo + 32].rearrange(
                        "c (t two) w -> c t two w", two=2)[:, :, parity, :]

                for b in range(B_LOC):
                    d0 = rows(0, 0, b)
                    d1 = rows(1, 0, b)
                    d2 = rows(2, 0, b)
                    d3 = rows(2, 1, b)
                    nc.vector.tensor_tensor(UH[:, 0, b], d0, d2, op=ALU.subtract)
                    nc.vector.tensor_tensor(UH[:, 1, b], d1, d2, op=ALU.add)
                    nc.vector.tensor_tensor(UH[:, 2, b], d2, d1, op=ALU.subtract)
                    nc.vector.tensor_tensor(UH[:, 3, b], d1, d3, op=ALU.subtract)
                # W-stage: (par w) cols: even block wp0..16, odd block 17..33.
                # tile wt: d0 = E[wt], d1 = O[wt], d2 = E[wt+1], d3 = O[wt+1]
                for s in range(4):
                    E0 = UH[:, s, :, :, 0:16]
                    E1 = UH[:, s, :, :, 1:17]
                    O0 = UH[:, s, :, :, 17:33]
                    O1 = UH[:, s, :, :, 18:34]
                    nc.vector.tensor_tensor(U2[co][:, 4 * s + 0], E0, E1, op=ALU.subtract)
                    nc.vector.tensor_tensor(U2[co][:, 4 * s + 1], O0, E1, op=ALU.add)
                    nc.vector.tensor_tensor(U2[co][:, 4 * s + 2], E1, O0, op=ALU.subtract)
                    nc.vector.tensor_tensor(U2[co][:, 4 * s + 3], O0, O1, op=ALU.subtract)

            # ---- csc stats into pack2 rows 2,3 ----
            nc.vector.tensor_reduce(
                pack2[:, 2, :], sumssc[:].rearrange("c (co x) -> c co x", x=16),
                axis=mybir.AxisListType.X, op=ALU.add)
            nc.vector.tensor_reduce(
                pack2[:, 3, :], sqsc[:].rearrange("c (co x) -> c co x", x=16),
                axis=mybir.AxisListType.X, op=ALU.add)

            # ================= conv2 (2D Winograd F(2x2,3x3)) =================
            # co outer; per (co, s): 4 sw-chains accumulate over ci2.
            # w-inverse per (co, s) -> T[s, a]; h-inverse per co -> C2 + stats.
            for co in range(NCO):
                T = scr_pool.tile([128, 4, 2, 512], bf, name="T", tag="T", bufs=1)
                hy0 = scr_pool.tile([128, 2, 512], bf, name="hy0", tag="hy0")
                hy1 = scr_pool.tile([128, 2, 512], bf, name="hy1", tag="hy1")
                for s in range(4):
                    # weights in 2 half-chunks (deeper DMA pipeline)
                    wt2p = []
                    for k in range(2):
                        wt2 = w2p.tile([128, 8, 128], bf, name="c2w", tag="w2",
                                       bufs=4)
                        nc.sync.dma_start(
                            wt2[:].rearrange("c l m -> c (l m)"),
                            w2g_d[co * 4 + s][:, k * 1024:(k + 1) * 1024])
                        wt2p.append(wt2)
                    pps = [ps.tile([128, 512], f32, name=f"c2ps{co}{s}_{sw}",
                                   tag="psb") for sw in range(4)]
                    for ci in range(NCO):
                        for sw in range(4):
                            blk = ci * 4 + sw
                            nc.tensor.matmul(
                                pps[sw][:], wt2p[blk // 8][:, blk % 8, :],
                                U2[ci][:, 4 * s + sw].rearrange("c b t w -> c (b t w)"),
                                start=(ci == 0), stop=(ci == NCO - 1))
                    if s == 3:
                        # fuse w-inverse + h-inverse final directly from PSUM:
                        # y(1,0) = hy1[0] - (m0+m1+m2) -> ph 2
                        # y(1,1) = hy1[1] - (m1-m2-m3) -> ph 3
                        for a, chain in ((0, ((0, -1.0), (1, -1.0), (2, -1.0))),
                                         (1, ((1, -1.0), (2, 1.0), (3, 1.0)))):
                            ph = 2 + a
                            dst = C2[co][:, ph].rearrange("c b t w -> c (b t w)")
                            src = hy1[:, a]
                            for i, (sw, sgn) in enumerate(chain):
                                nc.vector.scalar_tensor_tensor(
                                    dst, pps[sw][:], sgn, src,
                                    op0=ALU.mult, op1=ALU.add,
                                    accum_out=(sums2[:, co * 4 + ph:
                                                     co * 4 + ph + 1]
                                               if i == 2 else None))
                                src = dst
                            nc.scalar.activation(
                                sqscr[:], dst, AFT.Square,
                                accum_out=sq2[:, co * 4 + ph:co * 4 + ph + 1])
                        continue
                    # w-inverse: T0 = m0+m1+m2 ; T1 = m1-m2-m3 (1 psum read/op)
                    a2 = scr_pool.tile([128, 512], f32, name="a2", tag="a2")
                    q0 = scr_pool.tile([128, 512], f32, name="q0", tag="q0")
                    nc.scalar.copy(a2[:], pps[1][:])
                    nc.vector.scalar_tensor_tensor(q0[:], pps[0][:], 1.0, a2[:],
                                                   op0=ALU.mult, op1=ALU.add)
                    nc.vector.scalar_tensor_tensor(T[:, s, 0], pps[2][:], 1.0,
                                                   q0[:], op0=ALU.mult, op1=ALU.add)
                    nc.vector.scalar_tensor_tensor(q0[:], pps[2][:], -1.0, a2[:],
                                                   op0=ALU.mult, op1=ALU.add)
                    nc.vector.scalar_tensor_tensor(T[:, s, 1], pps[3][:], -1.0,
                                                   q0[:], op0=ALU.mult, op1=ALU.add)
                    # incremental h-inverse: y(jh,a) -> C2 phase (2*jh + a)
                    if s == 1:
                        for a in range(2):
                            nc.vector.tensor_tensor(hy0[:, a], T[:, 0, a],
                                                    T[:, 1, a], op=ALU.add)
                    elif s == 2:
                        for a in range(2):
                            ph = a
                            dst = C2[co][:, ph].rearrange("c b t w -> c (b t w)")
                            nc.vector.scalar_tensor_tensor(
                                dst, T[:, 2, a], 1.0, hy0[:, a],
                                op0=ALU.mult, op1=ALU.add,
                                accum_out=sums2[:, co * 4 + ph:co * 4 + ph + 1])
                            nc.scalar.activation(
                                sqscr[:], dst, AFT.Square,
                                accum_out=sq2[:, co * 4 + ph:co * 4 + ph + 1])
                            nc.vector.tensor_tensor(hy1[:, a], T[:, 1, a],
                                                    T[:, 2, a], op=ALU.subtract)
                    elif s == 3:
                        for a in range(2):
                            ph = 2 + a
                            dst = C2[co][:, ph].rearrange("c b t w -> c (b t w)")
                            nc.vector.scalar_tensor_tensor(
                                dst, T[:, 3, a], -1.0, hy1[:, a],
                                op0=ALU.mult, op1=ALU.add,
                                accum_out=sums2[:, co * 4 + ph:co * 4 + ph + 1])
                            nc.scalar.activation(
                                sqscr[:], dst, AFT.Square,
                                accum_out=sq2[:, co * 4 + ph:co * 4 + ph + 1])

            # ---- stats of c2 -> AllReduce #2 ----
            nc.vector.tensor_reduce(
                pack2[:, 0, :], sums2[:].rearrange("c (co x) -> c co x", x=4),
                axis=mybir.AxisListType.X, op=ALU.add)
            nc.vector.tensor_reduce(
                pack2[:, 1, :], sq2[:].rearrange("c (co x) -> c co x", x=4),
                axis=mybir.AxisListType.X, op=ALU.add)
            nc.sync.dma_start(ar2_in[:].rearrange("s (co c) -> c s co", c=128), pack2[:])
            nc.gpsimd.collective_compute(
                "AllReduce", ALU.add,
                replica_groups=[list(range(NCORES))],
                ins=[ar2_in.opt()], outs=[ar2_out.opt()])
            nc.sync.dma_start(
                st2[:].rearrange("c g s co -> c (g s) co"),
                ar2_out[:].rearrange("s (co c) -> c s co", c=128))

            # ---- BN2 / BNsc scale+shift ----
            # final = relu(s2*(c2 + (ssc/s2)*csc) + (t2 + tsc))
            nc.vector.tensor_scalar_mul(st2[:], st2[:], 1.0 / CNT)
            means = st2[:, :, 0, :]
            e2s = st2[:, :, 1, :]
            gpair = gbv[:, 2:, :].rearrange("c (g s) co -> c g s co", s=2)
            nc.vector.tensor_tensor(m2b[:], means, means, op=ALU.mult)
            nc.vector.tensor_tensor(varb[:], e2s, m2b[:], op=ALU.subtract)
            nc.scalar.activation(varb[:], varb[:], AFT.Sqrt, bias=epsc[:])
            nc.vector.reciprocal(invb[:], varb[:])
            nc.vector.tensor_tensor(scaleb[:], gpair[:, :, 0, :], invb[:], op=ALU.mult)
            nc.vector.tensor_tensor(m2b[:], means, scaleb[:], op=ALU.mult)
            nc.vector.tensor_tensor(shiftb2[:], gpair[:, :, 1, :], m2b[:], op=ALU.subtract)
            nc.vector.tensor_tensor(shiftB[:], shiftb2[:, 0, :], shiftb2[:, 1, :], op=ALU.add)
            nc.vector.reciprocal(tmpa[:], scaleb[:, 0, :])
            nc.vector.tensor_tensor(rmix[:], scaleb[:, 1, :], tmpa[:], op=ALU.mult)

            # ---- final fuse: c2 += rmix*csc ; out = relu(scale2*c2 + shiftB) ----
            for co in range(NCO):
                for iph in range(4):
                    nc.vector.scalar_tensor_tensor(
                        C2[co][:, iph].rearrange("c b t w -> c (b t w)"),
                        CSC[co][:, iph].rearrange("c b t w -> c (b t w)"),
                        rmix[:, co:co + 1],
                        C2[co][:, iph].rearrange("c b t w -> c (b t w)"),
                        op0=ALU.mult, op1=ALU.add)
                for b in range(B_LOC):
                    fin = scr_pool.tile([128, 1024], f32, name="fin", tag="fin",
                                        bufs=2)
                    for p in range(2):
                        # phases (2p, 2p+1) together: src dims (q, th, tw)
                        dst = fin.rearrange(
                            "c (th p2 tw q2) -> c p2 q2 th tw",
                            th=16, p2=2, q2=2)[:, p]
                        nc.scalar.activation(dst, C2[co][:, 2 * p:2 * p + 2, b],
                                             AFT.Relu,
                                             bias=shiftB[:, co:co + 1],
                                             scale=scaleb[:, 0, co:co + 1])
                    nc.sync.dma_start(
                        out_d[b, co * 128:(co + 1) * 128].rearrange("c h w -> c (h w)"),
                        fin[:])

            for _f in _frees:
                _f()

    nc.compile()
    return nc


def _get_nc():
    if "nc" not in _CACHE:
        _CACHE["nc"] = _build_nc()
    return _CACHE["nc"]


def _prep_inputs(x, w1, w2, wsc, g1, b1, g2, b2, gsc, bsc):
    xpad = np.zeros((B, CIN, 18, 18), dtype=np.float32)
    xpad[:, :, 1:17, 1:17] = x
    xpad = xpad.astype(BF16)
    w1g = _w5_blocks(w1).astype(BF16)
    wscg = _w5_blocks(wsc).astype(BF16)
    w2g = _w2_blocks(w2).astype(BF16)
    gb = np.stack([g1, b1, g2, b2, gsc, bsc]).astype(np.float32)   # [6, 512]
    gbt = np.ascontiguousarray(gb.reshape(6, 4, 128).transpose(2, 0, 1))  # [128, 6, 4]
    return xpad, w1g, wscg, w2g, gbt


def kernel(x, w1, g1, b1, w2, g2, b2, wsc, gsc, bsc, _trace=False, **_kw):
    from concourse.bass_utils import run_bass_kernel_spmd

    x = np.asarray(x, dtype=np.float32)
    xpad, w1g, wscg, w2g, gbt = _prep_inputs(
        np.asarray(x), np.asarray(w1), np.asarray(w2), np.asarray(wsc),
        np.asarray(g1), np.asarray(b1), np.asarray(g2), np.asarray(b2),
        np.asarray(gsc), np.asarray(bsc))

    nc = _get_nc()
    in_maps = []
    for core in range(NCORES):
        in_maps.append({
            "xpad": xpad[core * B_LOC:(core + 1) * B_LOC],
            "w1g": w1g, "wscg": wscg, "w2g": w2g, "gb": gbt,
        })
    res = run_bass_kernel_spmd(nc, in_maps, list(range(NCORES)), trace=_trace)
    out = np.concatenate([res.results[i]["out"] for i in range(NCORES)], axis=0)
    if _trace:
        _CACHE["last_result"] = res
    return out


# revision 40
# speedup vs baseline: 1.3901x; 1.0003x over previous
"""Trainium2 Bass kernel for the Gudi UpProj block — Winograd + bf16.

Reference computation (per image, NCHW):
    xu  = zero_stuff_2x(x)                    # [B,1024,32,32]
    c1  = conv5x5(xu, w1, pad=2);  out1 = relu(BN(c1))
    c2  = conv3x3(out1, w2, pad=1)
    csc = conv5x5(xu, wsc, pad=2)
    out = relu(BN(c2) + BN(csc))              # BN: batch stats over (N,H,W)

Strategy:
  * Data-parallel over batch: 16 images -> 2 per core (8 cores).
  * Zero-stuffing: the 5x5 conv decomposes into 4 parity phases with
    3x3 / 3x2 / 2x3 / 2x2 kernels on the 16x16 grid.
  * 1D Winograd along h on each phase: F(2,3) for 3-tap rows, F(2,2) for
    2-tap rows; w taps stay direct -> 25 -> 17.5 effective taps/quad.
  * conv2 (3x3) via full 2D Winograd F(2x2,3x3): 9 -> 4 eff taps.
  * All matmuls in bf16 (full PE rate at any N; PSUM accumulates f32).
  * BN batch stats cross-core via two small AllReduces (c1; c2+csc).
"""

import numpy as np
import ml_dtypes

NCORES = 8
B = 16
B_LOC = B // NCORES
CIN, COUT = 1024, 512
NCI, NCO = CIN // 128, COUT // 128   # 8, 4
EPS = 1e-5
CNT = float(B * 32 * 32)
PHASES = [(0, 0), (0, 1), (1, 0), (1, 1)]
BF16 = ml_dtypes.bfloat16

# F(2,3): 3-tap kernel, 2 outputs, 4 positions; F(2,2): 2-tap, 2 out, 3 pos.
G3 = np.array([[1, 0, 0], [.5, .5, .5], [.5, -.5, .5], [0, 0, 1]], np.float64)
G2 = np.array([[1, 0], [1, 1], [0, 1]], np.float64)
# Output transforms hardcoded in drain code:
#   F(2,3): y0 = m0+m1+m2 ; y1 = m1-m2-m3
#   F(2,2): y0 = m0+m1    ; y1 = m1+m2

_CACHE = {}


def _aws(q):
    return (-1, 0, 1) if q == 0 else (0, 1)


def _ht(p):
    return 4 if p == 0 else 3


def _w5_order():
    """(phase, s, ci, aw) emission order for conv5 weight blocks."""
    order = []
    for (p, q) in PHASES:
        for s in range(_ht(p)):
            for ci in range(NCI):
                for aw in _aws(q):
                    order.append((p, q, ci, s, aw))
    return order


N_W5 = len(_w5_order())  # 280


def _w5_blocks(w):
    """w [Cout, Cin, 5, 5] f32 -> [280, 128, COUT] transformed blocks (f32)."""
    w = np.asarray(w, np.float64)
    out = np.empty((N_W5, 128, COUT), np.float32)
    for g, (p, q, ci, s, aw) in enumerate(_w5_order()):
        Gm = G3 if p == 0 else G2
        nk = 3 - p
        acc = np.zeros((COUT, 128), np.float64)
        for k in range(nk):
            kh = 2 * k + p
            kw = 2 * (aw + 1) if q == 0 else 2 * aw + 1
            acc += Gm[s, k] * w[:, ci * 128:(ci + 1) * 128, kh, kw]
        out[g] = acc.T
    return out


def _w2_blocks(w2):
    """w2 [Cout, Cin, 3, 3] -> [16, 128, 16*128]: per (co, s): [c, (ci sw), m].

    DMA'd per (co, s) with per-partition-contiguous 4KB lines.
    """
    W2p = np.einsum('sr,ocrv,wv->swoc', G3, np.asarray(w2, np.float64), G3)
    out = np.empty((16, 128, 16, 128), np.float32)
    for co in range(NCO):
        for s in range(4):
            for ci in range(4):
                for sw in range(4):
                    blk = W2p[s, sw, co * 128:(co + 1) * 128,
                              ci * 128:(ci + 1) * 128]   # [m, c]
                    out[co * 4 + s, :, ci * 4 + sw, :] = blk.T
    return out.reshape(16, 128, 16 * 128)


def _build_nc():
    import concourse.mybir as mybir
    import concourse.tile as tile
    from concourse import bacc

    f32 = mybir.dt.float32
    bf = mybir.dt.bfloat16
    ALU = mybir.AluOpType
    AFT = mybir.ActivationFunctionType

    nc = bacc.Bacc("TRN2", target_bir_lowering=False, debug=False)

    xpad_d = nc.dram_tensor("xpad", [B_LOC, CIN, 18, 18], bf, kind="ExternalInput").ap()
    w1g_d = nc.dram_tensor("w1g", [N_W5, 128, COUT], bf, kind="ExternalInput").ap()
    wscg_d = nc.dram_tensor("wscg", [N_W5, 128, COUT], bf, kind="ExternalInput").ap()
    w2g_d = nc.dram_tensor("w2g", [16, 128, 16 * 128], bf, kind="ExternalInput").ap()
    gb_d = nc.dram_tensor("gb", [128, 6, 4], f32, kind="ExternalInput").ap()
    out_d = nc.dram_tensor("out", [B_LOC, COUT, 32, 32], f32, kind="ExternalOutput").ap()

    def r256(ap):
        return ap.rearrange("c (b t w) -> c b t w", b=2, t=8, w=16)

    def m128b(t, half, b):
        return t[:, half * 256 + b * 128:half * 256 + (b + 1) * 128].rearrange(
            "c (t w) -> c t w", t=8)

    with tile.TileContext(nc) as tc:
        _frees = []
        ar1_in, _f = tc.tile([2, COUT], f32, space="DRAM", name="ar1_in"); _frees.append(_f)
        ar1_out, _f = tc.tile([2, COUT], f32, space="DRAM", addr_space="Shared", name="ar1_out"); _frees.append(_f)
        ar2_in, _f = tc.tile([4, COUT], f32, space="DRAM", name="ar2_in"); _frees.append(_f)
        ar2_out, _f = tc.tile([4, COUT], f32, space="DRAM", addr_space="Shared", name="ar2_out"); _frees.append(_f)

        with tc.tile_pool(name="xp", bufs=1) as xp_pool, \
             tc.tile_pool(name="uv", bufs=1) as uv_pool, \
             tc.tile_pool(name="acts", bufs=1) as acts, \
             tc.tile_pool(name="op1", bufs=1) as op1_pool, \
             tc.tile_pool(name="u2", bufs=1) as u2_pool, \
             tc.tile_pool(name="wts", bufs=4) as wts, \
             tc.tile_pool(name="w2p", bufs=2) as w2p, \
             tc.tile_pool(name="scr", bufs=1) as scr_pool, \
             tc.tile_pool(name="small", bufs=1) as small, \
             tc.tile_pool(name="ps", bufs=8, space="PSUM") as ps:

            # ---- persistent SBUF tensors ----
            XP = [xp_pool.tile([128, 2, 18, 18], bf, name=f"xp{i}", tag=f"xp{i}")
                  for i in range(NCI)]
            # U: h-Winograd F(2,3) positions for p=0 phases; V: F(2,2) (v0,v2)
            U = [uv_pool.tile([128, 4, 2, 8, 18], bf, name=f"u{i}", tag=f"u{i}")
                 for i in range(NCI)]
            V = [uv_pool.tile([128, 2, 2, 8, 18], bf, name=f"v{i}", tag=f"v{i}")
                 for i in range(NCI)]
            # OP1: BN1(relu) output, padded, parity-split cols:
            # [c, b, h(0..33), par, wp]; par0 = even cols 0..32, par1 = odd 1..33
            OP1 = [op1_pool.tile([128, 2, 34, 2, 17], bf, name=f"op1_{i}", tag=f"op1_{i}")
                   for i in range(NCO)]
            # CSC / C2: phase-major [c, ph(2p+q), b, i, j]
            CSC = [acts.tile([128, 4, 2, 16, 16], bf, name=f"csc{i}", tag=f"csc{i}")
                   for i in range(NCO)]
            C2 = [acts.tile([128, 4, 2, 16, 16], bf, name=f"c2_{i}", tag=f"c2_{i}")
                  for i in range(NCO)]
            # U2: conv2 2D-Winograd input transform, per OP1 tile (=ci2)
            U2 = [u2_pool.tile([128, 16, 2, 16, 16], bf, name=f"u2_{i}", tag=f"u2_{i}")
                  for i in range(NCO)]

            # stats: conv5 sums col = ((co*4 + iph)*2 + j)*2 + b (per-image,
            # since the split drain ops each carry their own accum); sq col =
            # (co*4 + iph)*2 + j; conv2 col = co*4 + ph
            sums1 = small.tile([128, 64], f32, name="sums1")
            sq1 = small.tile([128, 64], f32, name="sq1")
            sumssc = small.tile([128, 64], f32, name="sumssc")
            sqsc = small.tile([128, 64], f32, name="sqsc")
            sums2 = small.tile([128, 16], f32, name="sums2")
            sq2 = small.tile([128, 16], f32, name="sq2")
            pack1 = small.tile([128, 2, 4], f32, name="pack1")
            pack2 = small.tile([128, 4, 4], f32, name="pack2")
            st1 = small.tile([128, 2, 4], f32, name="st1")
            st2 = small.tile([128, 2, 2, 4], f32, name="st2")
            gbv = small.tile([128, 6, 4], f32, name="gbv")
            scale1 = small.tile([128, 4], f32, name="scale1")
            shift1 = small.tile([128, 4], f32, name="shift1")
            m2b = small.tile([128, 2, 4], f32, name="m2b")
            varb = small.tile([128, 2, 4], f32, name="varb")
            invb = small.tile([128, 2, 4], f32, name="invb")
            scaleb = small.tile([128, 2, 4], f32, name="scaleb")
            shiftb2 = small.tile([128, 2, 4], f32, name="shiftb2")
            shiftB = small.tile([128, 4], f32, name="shiftB")
            rmix = small.tile([128, 4], f32, name="rmix")
            tmpa = small.tile([128, 4], f32, name="tmpa")
            tmpb = small.tile([128, 4], f32, name="tmpb")
            epsc = small.tile([128, 1], f32, name="epsc")
            sqscr = small.tile([128, 512], f32, name="sqscr")

            # ---- helpers ----
            def rowpair(xp, lo, parity):
                """rows lo..lo+15 of an 18-row dim, split in pairs, pick one."""
                return xp[:, :, lo:lo + 16].rearrange(
                    "c b (t two) w -> c b t two w", two=2)[:, :, :, parity, :]

            def emit_xp(ci):
                nc.sync.dma_start(
                    XP[ci][:].rearrange("c b h w -> c b (h w)"),
                    xpad_d[:, ci * 128:(ci + 1) * 128].rearrange("b c h w -> c b (h w)"))

            def emit_uv(ci):
                # per-image ops (walrus caps DVE APs at 3 canonical dims),
                # position-major so consumers unblock position by position
                xp = XP[ci]
                dv = [(rowpair(xp, 0, 0)[:, b],    # xpad rows 0,2..14
                       rowpair(xp, 1, 0)[:, b],    # 1,3..15
                       rowpair(xp, 2, 0)[:, b],    # 2,4..16
                       rowpair(xp, 2, 1)[:, b])    # 3,5..17
                      for b in range(B_LOC)]
                # p=1 window rows: d0'=d1, d1'=d2, d2'=d3:
                # v0 = d0'-d1', v1 = d1' (XP view), v2 = d2'-d1'
                for (dstT, idx, ia, ib) in ((U, 0, 0, 2), (U, 1, 1, 2),
                                            (U, 2, 2, 1), (U, 3, 1, 3),
                                            (V, 0, 1, 2), (V, 1, 3, 2)):
                    op = ALU.add if (dstT is U and idx == 1) else ALU.subtract
                    for b in range(B_LOC):
                        nc.vector.tensor_tensor(dstT[ci][:, idx, b],
                                                dv[b][ia], dv[b][ib], op=op)

            def v_rhs(ci, s, wsl):
                if s == 0:
                    return V[ci][:, 0, :, :, wsl]
                if s == 1:
                    return rowpair(XP[ci], 2, 0)[:, :, :, wsl]
                return V[ci][:, 1, :, :, wsl]

            def u_rhs(ci, s, wsl):
                return U[ci][:, s, :, :, wsl]

            def op1_dst(co, p, q, j):
                """conv1 drain dest: OP1 interior phase view [c, b, t, 16]."""
                par = 1 - q          # q=0 -> odd cols (1..31), q=1 -> even (2..32)
                wsl = slice(0, 16) if q == 0 else slice(1, 17)
                return OP1[co][:, :, 1:33].rearrange(
                    "c b (t four) par w -> c b t four par w", four=4)[
                        :, :, :, 2 * j + p, par, wsl]

            def csc_dst(co, p, q, j):
                iph = 2 * p + q
                return CSC[co][:, iph].rearrange(
                    "c b (t two) w -> c b t two w", two=2)[:, :, :, j, :]

            # ---- startup DMAs / init (first weight chunk ahead of x) ----
            wt_pre = wts.tile([128, 3, 512], bf, name="c1w0", tag="w5")
            nc.sync.dma_start(wt_pre[:, 0:1, :], w1g_d[0:1].rearrange("l c m -> c l m"))
            nc.sync.dma_start(wt_pre[:, 1:3, :], w1g_d[1:3].rearrange("l c m -> c l m"))
            emit_xp(0)
            emit_xp(1)
            nc.vector.memset(epsc[:], EPS)
            for co in range(NCO):
                nc.gpsimd.memset(OP1[co][:].rearrange("c b h p w -> c b (h p w)"), 0.0)
            emit_uv(0)
            emit_uv(1)

            # ---- conv5: phase / h-position / ci loops ----
            # Per s-position: 4 PSUM chains (one per co), accumulated over
            # (ci, aw). Drains fold the h-inverse incrementally, in place in
            # the bf16 destination:
            #   p=0: y0 = m0+m1+m2 ; y1 = m1-m2-m3
            #   p=1: y0 = m0+m1    ; y1 = m1+m2
            def conv5(wg_d, dst_fn, sums, sqs, wtag, prefetch=False, pre_wt=None):
                gofs = 0
                for iph, (p, q) in enumerate(PHASES):
                    aws = _aws(q)
                    nblk = len(aws)
                    rhs_fn = u_rhs if p == 0 else v_rhs
                    for s in range(_ht(p)):
                        pps = [ps.tile([128, 256], f32, name=f"{wtag}ps{iph}{s}_{co}",
                                       tag="psb") for co in range(NCO)]
                        for ci in range(NCI):
                            if prefetch and iph == 0 and s == 0 and ci + 2 < NCI:
                                emit_xp(ci + 2)
                                emit_uv(ci + 2)
                            if pre_wt is not None and iph == 0 and s == 0 and ci == 0:
                                wt = pre_wt
                            else:
                                wt = wts.tile([128, 3, 512], bf, name=f"{wtag}w",
                                              tag="w5")
                                nc.sync.dma_start(
                                    wt[:, :nblk, :],
                                    wg_d[gofs:gofs + nblk].rearrange("l c m -> c l m"))
                            gofs += nblk
                            for ai, aw in enumerate(aws):
                                rhs = rhs_fn(ci, s, slice(1 + aw, 17 + aw))
                                first = ci == 0 and ai == 0
                                last = ci == NCI - 1 and ai == nblk - 1
                                for co in range(NCO):
                                    nc.tensor.matmul(
                                        pps[co][:], wt[:, ai, co * 128:(co + 1) * 128],
                                        rhs, start=first, stop=last)
                        # ---- drain position s into y0/y1 (in-place partials) --
                        for co in range(NCO):
                            c0 = ((co * 4 + iph) * 2) * 2
                            for b in range(B_LOC):
                                m = pps[co][:, b * 128:(b + 1) * 128].rearrange(
                                    "c (t w) -> c t w", t=8)
                                y0 = dst_fn(co, p, q, 0)[:, b]
                                y1 = dst_fn(co, p, q, 1)[:, b]
                                s0 = sums[:, c0 + b:c0 + b + 1]
                                s1 = sums[:, c0 + 2 + b:c0 + 3 + b]
                                sg = 1.0
                                y0_ops, y1_ops = [], []
                                if s == 0:
                                    y0_ops = ["init"]
                                elif s == 1:
                                    y0_ops = ["acc"]
                                    y1_ops = ["init"]
                                elif s == 2:
                                    if p == 0:
                                        y0_ops = ["acc_fin"]
                                        y1_ops = ["neg"]
                                    else:
                                        y1_ops = ["acc_fin"]
                                else:
                                    y1_ops = ["neg_fin"]
                                if p == 1 and s == 1:
                                    y0_ops = ["acc_fin"]
                                for tgt, ops_, scol in ((y0, y0_ops, s0),
                                                        (y1, y1_ops, s1)):
                                    for opk in ops_:
                                        if opk == "init":
                                            nc.vector.tensor_scalar(
                                                tgt, m, 0.0, 0.0,
                                                op0=ALU.add, op1=ALU.add)
                                        else:
                                            fin = opk.endswith("fin")
                                            sgn = -1.0 if opk.startswith("neg") else 1.0
                                            nc.vector.scalar_tensor_tensor(
                                                tgt, m, sgn, tgt,
                                                op0=ALU.mult, op1=ALU.add,
                                                accum_out=(scol if fin else None))
                                if s == _ht(p) - 1:
                                    nc.scalar.activation(
                                        m128b(sqscr, 0, b), y0, AFT.Square,
                                        accum_out=sqs[:, c0 + b:c0 + b + 1])
                                    nc.scalar.activation(
                                        m128b(sqscr, 1, b), y1, AFT.Square,
                                        accum_out=sqs[:, c0 + 2 + b:c0 + 3 + b])

            # ================= conv1 =================
            conv5(w1g_d, op1_dst, sums1, sq1, "c1", prefetch=True, pre_wt=wt_pre)
            nc.sync.dma_start(gbv[:], gb_d)

            # ---- c1 stats -> AllReduce #1 (overlaps convsc) ----
            nc.vector.tensor_reduce(
                pack1[:, 0, :], sums1[:].rearrange("c (co x) -> c co x", x=16),
                axis=mybir.AxisListType.X, op=ALU.add)
            nc.vector.tensor_reduce(
                pack1[:, 1, :], sq1[:].rearrange("c (co x) -> c co x", x=16),
                axis=mybir.AxisListType.X, op=ALU.add)
            nc.sync.dma_start(ar1_in[:].rearrange("s (co c) -> c s co", c=128), pack1[:])
            nc.gpsimd.collective_compute(
                "AllReduce", ALU.add,
                replica_groups=[list(range(NCORES))],
                ins=[ar1_in.opt()], outs=[ar1_out.opt()])
            nc.sync.dma_start(st1[:], ar1_out[:].rearrange("s (co c) -> c s co", c=128))

            # ================= convsc =================
            conv5(wscg_d, csc_dst, sumssc, sqsc, "sc")

            # ---- BN1 scale/shift ----
            nc.vector.tensor_scalar_mul(st1[:], st1[:], 1.0 / CNT)
            m1 = st1[:, 0, :]
            nc.vector.tensor_tensor(tmpa[:], m1, m1, op=ALU.mult)
            nc.vector.tensor_tensor(tmpb[:], st1[:, 1, :], tmpa[:], op=ALU.subtract)
            nc.scalar.activation(tmpb[:], tmpb[:], AFT.Sqrt, bias=epsc[:])
            nc.vector.reciprocal(tmpa[:], tmpb[:])
            nc.vector.tensor_tensor(scale1[:], gbv[:, 0, :], tmpa[:], op=ALU.mult)
            nc.vector.tensor_tensor(tmpa[:], m1, scale1[:], op=ALU.mult)
            nc.vector.tensor_tensor(shift1[:], gbv[:, 1, :], tmpa[:], op=ALU.subtract)

            # ---- BN1 apply (in-place relu) + conv2 input transform ----
            UH = scr_pool.tile([128, 4, 2, 16, 34], bf, name="uh", tag="uh")
            for co in range(NCO):
                for par, wsl in ((0, slice(1, 17)), (1, slice(0, 16))):
                    for b in range(B_LOC):
                        v = OP1[co][:, b, 1:33, par, wsl]
                        nc.scalar.activation(v, v, AFT.Relu,
                                             bias=shift1[:, co:co + 1],
                                             scale=scale1[:, co:co + 1])
                # H-stage: window rows 2t..2t+3 of padded OP1 (34 rows)
                flat = OP1[co][:].rearrange("c b h par w -> c b h (par w)")

                def rows(lo, parity, b):
                    return flat[:, b, lo:lo + 32].rearrange(
                        "c (t two) w -> c t two w", two=2)[:, :, parity, :]

                for b in range(B_LOC):
                    d0 = rows(0, 0, b)
                    d1 = rows(1, 0, b)
                    d2 = rows(2, 0, b)
                    d3 = rows(2, 1, b)
                    nc.vector.tensor_tensor(UH[:, 0, b], d0, d2, op=ALU.subtract)
                    nc.vector.tensor_tensor(UH[:, 1, b], d1, d2, op=ALU.add)
                    nc.vector.tensor_tensor(UH[:, 2, b], d2, d1, op=ALU.subtract)
                    nc.vector.tensor_tensor(UH[:, 3, b], d1, d3, op=ALU.subtract)
                # W-stage: (par w) cols: even block wp0..16, odd block 17..33.
                # tile wt: d0 = E[wt], d1 = O[wt], d2 = E[wt+1], d3 = O[wt+1]
                for s in range(4):
                    E0 = UH[:, s, :, :, 0:16]
                    E1 = UH[:, s, :, :, 1:17]
                    O0 = UH[:, s, :, :, 17:33]
                    O1 = UH[:, s, :, :, 18:34]
                    nc.vector.tensor_tensor(U2[co][:, 4 * s + 0], E0, E1, op=ALU.subtract)
                    nc.vector.tensor_tensor(U2[co][:, 4 * s + 1], O0, E1, op=ALU.add)
                    nc.vector.tensor_tensor(U2[co][:, 4 * s + 2], E1, O0, op=ALU.subtract)
                    nc.vector.tensor_tensor(U2[co][:, 4 * s + 3], O0, O1, op=ALU.subtract)

            # ---- csc stats into pack2 rows 2,3 ----
            nc.vector.tensor_reduce(
                pack2[:, 2, :], sumssc[:].rearrange("c (co x) -> c co x", x=16),
                axis=mybir.AxisListType.X, op=ALU.add)
            nc.vector.tensor_reduce(
                pack2[:, 3, :], sqsc[:].rearrange("c (co x) -> c co x", x=16),
                axis=mybir.AxisListType.X, op=ALU.add)

            # ================= conv2 (2D Winograd F(2x2,3x3)) =================
            # co outer; per (co, s): 4 sw-chains accumulate over ci2.
            # w-inverse per (co, s) -> T[s, a]; h-inverse per co -> C2 + stats.
            for co in range(NCO):
                T = scr_pool.tile([128, 4, 2, 512], bf, name="T", tag="T", bufs=1)
                hy0 = scr_pool.tile([128, 2, 512], bf, name="hy0", tag="hy0")
                hy1 = scr_pool.tile([128, 2, 512], bf, name="hy1", tag="hy1")
                for s in range(4):
                    # weights in 2 half-chunks (deeper DMA pipeline)
                    wt2p = []
                    for k in range(2):
                        wt2 = w2p.tile([128, 8, 128], bf, name="c2w", tag="w2",
                                       bufs=4)
                        nc.sync.dma_start(
                            wt2[:].rearrange("c l m -> c (l m)"),
                            w2g_d[co * 4 + s][:, k * 1024:(k + 1) * 1024])
                        wt2p.append(wt2)
                    pps = [ps.tile([128, 512], f32, name=f"c2ps{co}{s}_{sw}",
                                   tag="psb") for sw in range(4)]
                    for ci in range(NCO):
                        for sw in range(4):
                            blk = ci * 4 + sw
                            nc.tensor.matmul(
                                pps[sw][:], wt2p[blk // 8][:, blk % 8, :],
                                U2[ci][:, 4 * s + sw].rearrange("c b t w -> c (b t w)"),
                                start=(ci == 0), stop=(ci == NCO - 1))
                    if s == 3:
                        # fuse w-inverse + h-inverse final directly from PSUM:
                        # y(1,0) = hy1[0] - (m0+m1+m2) -> ph 2
                        # y(1,1) = hy1[1] - (m1-m2-m3) -> ph 3
                        for a, chain in ((0, ((0, -1.0), (1, -1.0), (2, -1.0))),
                                         (1, ((1, -1.0), (2, 1.0), (3, 1.0)))):
                            ph = 2 + a
                            dst = C2[co][:, ph].rearrange("c b t w -> c (b t w)")
                            src = hy1[:, a]
                            for i, (sw, sgn) in enumerate(chain):
                                nc.vector.scalar_tensor_tensor(
                                    dst, pps[sw][:], sgn, src,
                                    op0=ALU.mult, op1=ALU.add,
                                    accum_out=(sums2[:, co * 4 + ph:
                                                     co * 4 + ph + 1]
                                               if i == 2 else None))
                                src = dst
                            nc.scalar.activation(
                                sqscr[:], dst, AFT.Square,
                                accum_out=sq2[:, co * 4 + ph:co * 4 + ph + 1])
                        continue
                    # w-inverse: T0 = m0+m1+m2 ; T1 = m1-m2-m3 (1 psum read/op)
                    a2 = scr_pool.tile([128, 512], f32, name="a2", tag="a2")
                    q0 = scr_pool.tile([128, 512], f32, name="q0", tag="q0")
                    nc.scalar.copy(a2[:], pps[1][:])
                    nc.vector.scalar_tensor_tensor(q0[:], pps[0][:], 1.0, a2[:],
                                                   op0=ALU.mult, op1=ALU.add)
                    nc.vector.scalar_tensor_tensor(T[:, s, 0], pps[2][:], 1.0,
                                                   q0[:], op0=ALU.mult, op1=ALU.add)
                    nc.vector.scalar_tensor_tensor(q0[:], pps[2][:], -1.0, a2[:],
                                                   op0=ALU.mult, op1=ALU.add)
                    nc.vector.scalar_tensor_tensor(T[:, s, 1], pps[3][:], -1.0,
                                                   q0[:], op0=ALU.mult, op1=ALU.add)
                    # incremental h-inverse: y(jh,a) -> C2 phase (2*jh + a)
                    if s == 1:
                        for a in range(2):
                            nc.vector.tensor_tensor(hy0[:, a], T[:, 0, a],
                                                    T[:, 1, a], op=ALU.add)
                    elif s == 2:
                        for a in range(2):
                            ph = a
                            dst = C2[co][:, ph].rearrange("c b t w -> c (b t w)")
                            nc.vector.scalar_tensor_tensor(
                                dst, T[:, 2, a], 1.0, hy0[:, a],
                                op0=ALU.mult, op1=ALU.add,
                                accum_out=sums2[:, co * 4 + ph:co * 4 + ph + 1])
                            nc.scalar.activation(
                                sqscr[:], dst, AFT.Square,
                                accum_out=sq2[:, co * 4 + ph:co * 4 + ph + 1])
                            nc.vector.tensor_tensor(hy1[:, a], T[:, 1, a],
                                                    T[:, 2, a], op=ALU.subtract)
                    elif s == 3:
                        for a in range(2):
                            ph = 2 + a
                            dst = C2[co][:, ph].rearrange("c b t w -> c (b t w)")
                            nc.vector.scalar_tensor_tensor(
                                dst, T[:, 3, a], -1.0, hy1[:, a],
                                op0=ALU.mult, op1=ALU.add,
                                accum_out=sums2[:, co * 4 + ph:co * 4 + ph + 1])
                            nc.scalar.activation(
                                sqscr[:], dst, AFT.Square,
                                accum_out=sq2[:, co * 4 + ph:co * 4 + ph + 1])

            # ---- stats of c2 -> AllReduce #2 ----
            nc.vector.tensor_reduce(
                pack2[:, 0, :], sums2[:].rearrange("c (co x) -> c co x", x=4),
                axis=mybir.AxisListType.X, op=ALU.add)
            nc.vector.tensor_reduce(
                pack2[:, 1, :], sq2[:].rearrange("c (co x) -> c co x", x=4),
                axis=mybir.AxisListType.X, op=ALU.add)
            nc.sync.dma_start(ar2_in[:].rearrange("s (co c) -> c s co", c=128), pack2[:])
            nc.gpsimd.collective_compute(
                "AllReduce", ALU.add,
                replica_groups=[list(range(NCORES))],
                ins=[ar2_in.opt()], outs=[ar2_out.opt()])
            nc.sync.dma_start(
                st2[:].rearrange("c g s co -> c (g s) co"),
                ar2_out[:].rearrange("s (co c) -> c s co", c=128))

            # ---- BN2 / BNsc scale+shift ----
            # final = relu(s2*(c2 + (ssc/s2)*csc) + (t2 + tsc))
            nc.vector.tensor_scalar_mul(st2[:], st2[:], 1.0 / CNT)
            means = st2[:, :, 0, :]
            e2s = st2[:, :, 1, :]
            gpair = gbv[:, 2:, :].rearrange("c (g s) co -> c g s co", s=2)
            nc.vector.tensor_tensor(m2b[:], means, means, op=ALU.mult)
            nc.vector.tensor_tensor(varb[:], e2s, m2b[:], op=ALU.subtract)
            nc.scalar.activation(varb[:], varb[:], AFT.Sqrt, bias=epsc[:])
            nc.vector.reciprocal(invb[:], varb[:])
            nc.vector.tensor_tensor(scaleb[:], gpair[:, :, 0, :], invb[:], op=ALU.mult)
            nc.vector.tensor_tensor(m2b[:], means, scaleb[:], op=ALU.mult)
            nc.vector.tensor_tensor(shiftb2[:], gpair[:, :, 1, :], m2b[:], op=ALU.subtract)
            nc.vector.tensor_tensor(shiftB[:], shiftb2[:, 0, :], shiftb2[:, 1, :], op=ALU.add)
            nc.vector.reciprocal(tmpa[:], scaleb[:, 0, :])
            nc.vector.tensor_tensor(rmix[:], scaleb[:, 1, :], tmpa[:], op=ALU.mult)

            # ---- final fuse: c2 += rmix*csc ; out = relu(scale2*c2 + shiftB) ----
            for co in range(NCO):
                for iph in range(4):
                    nc.vector.scalar_tensor_tensor(
                        C2[co][:, iph].rearrange("c b t w -> c (b t w)"),
                        CSC[co][:, iph].rearrange("c b t w -> c (b t w)"),
                        rmix[:, co:co + 1],
                        C2[co][:, iph].rearrange("c b t w -> c (b t w)"),
                        op0=ALU.mult, op1=ALU.add)
                for b in range(B_LOC):
                    fin = scr_pool.tile([128, 1024], f32, name="fin", tag="fin",
                                        bufs=2)
                    for p in range(2):
                        # phases (2p, 2p+1) together: src dims (q, th, tw)
                        dst = fin.rearrange(
                            "c (th p2 tw q2) -> c p2 q2 th tw",
                            th=16, p2=2, q2=2)[:, p]
                        nc.scalar.activation(dst, C2[co][:, 2 * p:2 * p + 2, b],
                                             AFT.Relu,
                                             bias=shiftB[:, co:co + 1],
                                             scale=scaleb[:, 0, co:co + 1])
                    nc.sync.dma_start(
                        out_d[b, co * 128:(co + 1) * 128].rearrange("c h w -> c (h w)"),
                        fin[:])

            for _f in _frees:
                _f()

    nc.compile()
    return nc


def _get_nc():
    if "nc" not in _CACHE:
        _CACHE["nc"] = _build_nc()
    return _CACHE["nc"]


def _prep_inputs(x, w1, w2, wsc, g1, b1, g2, b2, gsc, bsc):
    xpad = np.zeros((B, CIN, 18, 18), dtype=np.float32)
    xpad[:, :, 1:17, 1:17] = x
    xpad = xpad.astype(BF16)
    w1g = _w5_blocks(w1).astype(BF16)
    wscg = _w5_blocks(wsc).astype(BF16)
    w2g = _w2_blocks(w2).astype(BF16)
    gb = np.stack([g1, b1, g2, b2, gsc, bsc]).astype(np.float32)   # [6, 512]
    gbt = np.ascontiguousarray(gb.reshape(6, 4, 128).transpose(2, 0, 1))  # [128, 6, 4]
    return xpad, w1g, wscg, w2g, gbt


def kernel(x, w1, g1, b1, w2, g2, b2, wsc, gsc, bsc, _trace=False, **_kw):
    from concourse.bass_utils import run_bass_kernel_spmd

    x = np.asarray(x, dtype=np.float32)
    xpad, w1g, wscg, w2g, gbt = _prep_inputs(
        np.asarray(x), np.asarray(w1), np.asarray(w2), np.asarray(wsc),
        np.asarray(g1), np.asarray(b1), np.asarray(g2), np.asarray(b2),
        np.asarray(gsc), np.asarray(bsc))

    nc = _get_nc()
    in_maps = []
    for core in range(NCORES):
        in_maps.append({
            "xpad": xpad[core * B_LOC:(core + 1) * B_LOC],
            "w1g": w1g, "wscg": wscg, "w2g": w2g, "gb": gbt,
        })
    res = run_bass_kernel_spmd(nc, in_maps, list(range(NCORES)), trace=_trace)
    out = np.concatenate([res.results[i]["out"] for i in range(NCORES)], axis=0)
    if _trace:
        _CACHE["last_result"] = res
    return out
